# revision 1
# baseline (speedup 1.0000x reference)
"""Trainium2 Bass kernel for nn_DDGCRN (gnn_message_passing).

DDGCRN: two 12-step GRU-style encoders over B=16, N=8600 nodes, HID=64,
with a global node-pooling term (GFS) inside each gate, plus conv heads.

Sharding: data-parallel over batch. B=16 / 8 cores = 2 batch elems per
core; the GFS node-pooling sum is per-batch-element, so no collectives.

Per-core layout: feature-major. All wide tensors are [D, W] where
W = 2*8600 = 17200 columns (col = b*N + n). State tile X is [66, W]:
rows 0:64 = hidden state, row 64 = x_t, row 65 = v (rank-1 pooling row).
Weight rows are host-permuted to match ([W_state; w_x; aug]).

The GFS pooled term affw*(pooled*scale) is rank-1 in (d, n) for the
given inputs (affw==1): C[d,n] = u[d]*v[n]. It is folded into the
"res" matmul by augmenting K: lhsT row 65 = pooled*u (written per cell
via PE-transpose), rhs row 65 = v (constant). Non-rank-1 / nonzero-affb
inputs fall back to explicit DVE ops with C/AB streamed from DRAM.
"""

import numpy as np
import ml_dtypes
from contextlib import ExitStack

import concourse.bass as bass
import concourse.bacc as bacc
import concourse.tile as tile
from concourse import mybir
from concourse.bass_utils import run_bass_kernel_spmd

F32 = mybir.dt.float32
F32R = mybir.dt.float32r
BF16 = mybir.dt.bfloat16
AX = mybir.AxisListType
OP = mybir.AluOpType
AF = mybir.ActivationFunctionType

# Problem constants (hardcoded; kernel.py must be self-contained)
B, T, N_FULL, HID, IN = 16, 12, 8600, 64, 1
GIN = IN + HID
NCORES = 8
BLOC = B // NCORES  # 2


def _chunks(total, size):
    out = []
    off = 0
    while off < total:
        w = min(size, total - off)
        out.append((off, w))
        off += w
    return out


def _rank1(C):
    """C [D, M] -> (u [D], v [M]) with C == outer(u, v), or None."""
    d0, m0 = np.unravel_index(np.argmax(np.abs(C)), C.shape)
    piv = C[d0, m0]
    if abs(piv) < 1e-30:
        return np.zeros(C.shape[0], np.float32), np.zeros(C.shape[1], np.float32)
    u = C[:, m0].astype(np.float64)
    v = C[d0, :].astype(np.float64) / piv
    if not np.allclose(np.outer(u, v), C, rtol=1e-5, atol=1e-7 * abs(piv)):
        return None
    return u.astype(np.float32), v.astype(np.float32)


def _prep_host(inputs, n=N_FULL, t_steps=T):
    """Host-side parameter prep. Returns dict of per-core-shared arrays
    plus flags. All weight matrices get the row permutation
    [rows 1:65 (state); row 0 (x); (aug row 0)]."""
    f32 = np.float32
    H = {"flags": {}}
    shared = {}

    def perm66(w):
        """[65, Dout] -> [66, Dout]: rows 0:64 = state weights, row 64 = 0
        (v-row slot / aug slot), row 65 = x weight."""
        z = np.zeros((1, w.shape[1]), f32)
        return np.concatenate([w[1:], z, w[0:1]], axis=0).astype(f32)

    for e in range(2):
        gaW = np.asarray(inputs["gate_alignW"][e], f32)   # [65, 128]
        gw = np.asarray(inputs["gate_w"][e], f32)         # [65, 128]
        gab = np.asarray(inputs["gate_alignb"][e], f32)   # [128]
        gb = np.asarray(inputs["gate_b"][e], f32)         # [128]
        uaW = np.asarray(inputs["upd_alignW"][e], f32)    # [65, 64]
        uw = np.asarray(inputs["upd_w"][e], f32)          # [65, 64]
        uab = np.asarray(inputs["upd_alignb"][e], f32)    # [64]
        ub = np.asarray(inputs["upd_b"][e], f32)          # [64]

        shared[f"wgh{e}"] = perm66(gw).astype(ml_dtypes.bfloat16)
        shared[f"wga{e}"] = perm66(gaW).astype(ml_dtypes.bfloat16)
        shared[f"wuh{e}"] = perm66(uw).astype(ml_dtypes.bfloat16)
        shared[f"wua{e}"] = perm66(uaW).astype(ml_dtypes.bfloat16)

        # rank-1 pooling factors: C[d, n] = affw[n, d] * aw[n] * nw[n]
        for kind, aff, aw, nw, dout in (
            ("g", inputs["gate_affw"][e], inputs["gate_aw"][e], inputs["gate_nw"][e], 128),
            ("u", inputs["upd_affw"][e], inputs["upd_aw"][e], inputs["upd_nw"][e], 64),
        ):
            scale = (np.asarray(aw, f32)[:, 0] * np.asarray(nw, f32)[0])  # [n]
            C = np.asarray(aff, f32).T * scale[None, :]  # [dout, n]
            r1 = _rank1(C)
            if r1 is None:
                H["flags"][f"cfull_{kind}{e}"] = True
                u = np.zeros(dout, f32)
                v = np.zeros(C.shape[1], f32)
                if kind == "g":
                    shared[f"cg{e}"] = np.ascontiguousarray(C)
                else:
                    shared[f"cu2_{e}"] = np.ascontiguousarray(
                        np.concatenate([C, C], axis=0))
            else:
                H["flags"][f"cfull_{kind}{e}"] = False
                u, v = r1
            if kind == "g":
                ug = u
                vg = v
            else:
                uu = u
                vu = v

        # affb fallback (AB tensors). gab/uab per-partition parts go in ACT bias.
        abg = np.asarray(inputs["gate_affb"][e], f32).T  # [128, n]
        abu = np.asarray(inputs["upd_affb"][e], f32).T   # [64, n]
        H["flags"][f"ab_g{e}"] = bool(np.any(abg))
        H["flags"][f"ab_u{e}"] = bool(np.any(abu))
        if H["flags"][f"ab_g{e}"]:
            shared[f"abg{e}"] = np.ascontiguousarray(abg)
        if H["flags"][f"ab_u{e}"]:
            shared[f"abu2_{e}"] = np.ascontiguousarray(
                np.concatenate([abu, abu], axis=0))

        # v rows, repeated per local batch elem
        shared[f"vg{e}"] = np.tile(vg[None, :], (1, BLOC)).astype(ml_dtypes.bfloat16)
        shared[f"vu{e}"] = np.tile(vu[None, :], (1, BLOC)).astype(ml_dtypes.bfloat16)

        # bias/scale vector columns
        H[f"gb{e}"] = gb
        H[f"bzr{e}"] = np.concatenate([gab[:64], -gab[64:]])
        H[f"ub2_{e}"] = np.concatenate([ub, ub])
        H[f"uab2_{e}"] = np.concatenate([uab, uab])
        H[f"ug{e}"] = ug
        H[f"uu2_{e}"] = np.concatenate([uu, uu])

    cw = np.asarray(inputs["conv_w"], f32)  # [3, 12, 64]
    cb = np.asarray(inputs["conv_b"], f32)  # [3, 12]
    # head order: [src1(12) | out1(12)] so src1 sits at partitions 0:12
    shared["cw01"] = np.concatenate([cw[1].T, cw[0].T], axis=1).astype(
        ml_dtypes.bfloat16)                                       # [64, 2T]
    shared["cw2"] = np.ascontiguousarray(cw[2].T).astype(ml_dtypes.bfloat16)
    shared["ident"] = np.eye(128, dtype=f32)

    # cvec columns
    ncol = 16
    cvec = np.zeros((128, ncol), f32)
    cvec[:64, 0] = 1.0
    cvec[64:, 0] = -1.0
    cols = {"szr": 0}
    ci = 1
    for e in range(2):
        for nm in ("gb", "bzr", "ub2_", "uab2_", "ug", "uu2_"):
            key = f"{nm}{e}"
            arr = H[key]
            cvec[: len(arr), ci] = arr
            cols[key] = ci
            ci += 1
    th = cb.shape[1]
    cvec[: 2 * th, ci] = np.concatenate([cb[1], cb[0]])
    cols["cb01"] = ci
    ci += 1
    cvec[:th, ci] = cb[2]
    cols["cb2"] = ci
    shared["cvec"] = cvec
    H["cols"] = cols
    H["shared"] = shared
    return H


def _build(H, n=N_FULL, t_steps=T, ch=1024, mmc=512):
    """Build the single-core Bass program (same for all cores)."""
    W = BLOC * n
    flags = H["flags"]
    cols = H["cols"]
    nc = bacc.Bacc("TRN2", target_bir_lowering=False, debug=False)

    dram = {}
    for name, arr in H["shared"].items():
        dram[name] = nc.declare_dram_parameter(
            name, list(arr.shape), mybir.dt.from_np(arr.dtype), isOutput=False)
    src32 = nc.declare_dram_parameter("src32", [t_steps, W], F32, isOutput=False)
    srcbf = nc.declare_dram_parameter("srcbf", [t_steps, W], BF16, isOutput=False)
    out_d = nc.declare_dram_parameter("out", [t_steps, W], F32, isOutput=True)
    o1d = nc.dram_tensor("o1d", [t_steps, W], F32)
    xsbf = nc.dram_tensor("xsbf", [t_steps, W], BF16)

    CH_B = _chunks(n, ch)       # chunks within one batch-half
    hch = min(ch, 1024)
    CH_W = _chunks(W, hch)      # chunks over full width (boundary passes)
    nchb = len(CH_B)

    with tile.TileContext(nc) as tc, ExitStack() as ctx:
        # ---- persistent tiles (one pool, distinct tags = one slot each) ----
        pers = ctx.enter_context(tc.tile_pool(name="pers", bufs=1))

        def ptile(shape, dtype, nm):
            return pers.tile(shape, dtype, name=nm, tag=nm)

        X = ptile([66, W], BF16, "X")
        X2 = ptile([66, W], BF16, "X2")
        ZQ = ptile([128, W], BF16, "ZQ")
        CVEC = ptile(list(H["shared"]["cvec"].shape), F32, "CVEC")
        IDENT = ptile([128, 128], F32, "IDENT")
        th = t_steps
        CW01 = ptile([64, 2 * th], BF16, "CW01")
        CW2 = ptile([64, th], BF16, "CW2")
        WGH = {}
        WGA = {}
        WUH = {}
        WUA = {}
        for e in range(2):
            WGH[e] = ptile([66, 128], BF16, f"twgh{e}")
            WUH[e] = ptile([66, 64], BF16, f"twuh{e}")
            WGA[e] = {}
            WUA[e] = {}
            for b in range(BLOC):
                WGA[e][b] = ptile([66, 128], BF16, f"twga{e}{b}")
                WUA[e][b] = ptile([66, 64], BF16, f"twua{e}{b}")

        # (Bacc.generate_event_semaphores legalizes wait counts at compile)
        nc.sync.dma_start(CVEC[:, :], dram["cvec"][:, :])
        nc.sync.dma_start(IDENT[:, :], dram["ident"][:, :])
        nc.sync.dma_start(CW01[:, :], dram["cw01"][:, :])
        nc.sync.dma_start(CW2[:, :], dram["cw2"][:, :])
        for e in range(2):
            nc.sync.dma_start(WGH[e][:, :], dram[f"wgh{e}"][:, :])
            nc.sync.dma_start(WUH[e][:, :], dram[f"wuh{e}"][:, :])
            for b in range(BLOC):
                nc.sync.dma_start(WGA[e][b][:, :], dram[f"wga{e}"][:, :])
                nc.sync.dma_start(WUA[e][b][:, :], dram[f"wua{e}"][:, :])

        def bias(key):
            return CVEC[:, cols[key]:cols[key] + 1]

        # ---- pools ----
        ps = ctx.enter_context(tc.tile_pool(name="ps", bufs=3, space="PSUM"))
        tps = ctx.enter_context(tc.tile_pool(name="tps", bufs=2, space="PSUM"))
        sb = ctx.enter_context(tc.tile_pool(name="sb", bufs=3))
        small = ctx.enter_context(tc.tile_pool(name="small", bufs=2))
        fpool = ctx.enter_context(tc.tile_pool(name="fpool", bufs=2))

        def mm_into(p, lhsT, rhs_tile, rhs_rows, coff, cw_, p_rows=None,
                    cast=None):
            """Matmul chunk [*, cw_] at col offset coff into psum tile p."""
            for m0, mw in _chunks(cw_, mmc):
                lhs_ap = lhsT
                rhs_ap = rhs_tile[rhs_rows, coff + m0:coff + m0 + mw]
                o = p[:, m0:m0 + mw] if p_rows is None else \
                    p[p_rows, m0:m0 + mw]
                if cast is not None:
                    lhs_ap = lhs_ap.bitcast(cast)
                    rhs_ap = rhs_ap.bitcast(cast)
                nc.tensor.matmul(o, lhs_ap, rhs_ap, start=True, stop=True)

        def gfs_gate(e):
            pooled_b = []
            for b in range(BLOC):
                boff = b * n
                parts = small.tile([128, nchb], F32, tag="parts")
                for ci, (c0, cw_) in enumerate(CH_B):
                    p = ps.tile([128, ch], F32, tag="ps")
                    mm_into(p, WGH[e][:, :], X, slice(0, 66), boff + c0, cw_)
                    if False:
                        # DVE relu+bias+accum for engine balance
                        nc.vector.tensor_scalar(p[:, :cw_], p[:, :cw_],
                                                bias(f"gb{e}"), 0.0,
                                                op0=OP.add, op1=OP.max,
                                                accum_out=parts[:, ci:ci + 1])
                    else:
                        nc.scalar.activation(p[:, :cw_], p[:, :cw_], AF.Relu,
                                             bias=bias(f"gb{e}"),
                                             accum_out=parts[:, ci:ci + 1])
                if not flags[f"cfull_g{e}"]:
                    # fused: scratch = parts*u, accum = sum = pooled*u
                    scr = small.tile([128, nchb], F32, tag="pscr")
                    pgs = small.tile([128, 1], F32, tag="pgs")
                    nc.vector.tensor_scalar(scr[:, :], parts[:, :],
                                            bias(f"ug{e}"), 0.0, op0=OP.mult,
                                            op1=OP.add, accum_out=pgs[:, :])
                    tp = tps.tile([1, 128], F32, tag="tp")
                    nc.tensor.transpose(tp[:, :], pgs[:, :], IDENT[:, :])
                    nc.vector.tensor_copy(WGA[e][b][64:65, 0:128],
                                          tp[0:1, 0:128])
                    pooled_b.append(None)
                else:
                    pooled = small.tile([128, 1], F32, tag="pooled")
                    nc.vector.tensor_reduce(pooled[:, :], parts[:, :],
                                            axis=AX.X, op=OP.add)
                    pooled_b.append(pooled)
            # pass 2: res(+aug) matmul -> sigmoid -> ZQ (z rows 0:64, q 64:128)
            for b in range(BLOC):
                boff = b * n
                for c0, cw_ in CH_B:
                    p = ps.tile([128, ch], F32, tag="ps")
                    mm_into(p, WGA[e][b][:, :], X, slice(0, 66), boff + c0,
                            cw_)
                    if flags[f"cfull_g{e}"]:
                        cgc = fpool.tile([128, ch], F32, tag="cgc")
                        nc.sync.dma_start(cgc[:, :cw_],
                                          dram[f"cg{e}"][:, c0:c0 + cw_])
                        nc.vector.scalar_tensor_tensor(
                            p[:, :cw_], cgc[:, :cw_], pooled_b[b][:, :],
                            p[:, :cw_], op0=OP.mult, op1=OP.add)
                    if flags[f"ab_g{e}"]:
                        abc = fpool.tile([128, ch], F32, tag="abc")
                        nc.sync.dma_start(abc[:, :cw_],
                                          dram[f"abg{e}"][:, c0:c0 + cw_])
                        nc.vector.tensor_add(p[:, :cw_], p[:, :cw_],
                                             abc[:, :cw_])
                    nc.scalar.activation(ZQ[:, boff + c0:boff + c0 + cw_],
                                         p[:, :cw_], AF.Sigmoid,
                                         bias=bias(f"bzr{e}"),
                                         scale=bias("szr"))
                    # zs = z * S -> X2 state rows (b0 on gpsimd for balance)
                    zeng = nc.gpsimd if b == 0 else nc.vector
                    zeng.tensor_mul(
                        X2[0:64, boff + c0:boff + c0 + cw_],
                        ZQ[0:64, boff + c0:boff + c0 + cw_],
                        X[0:64, boff + c0:boff + c0 + cw_])

        def gfs_upd(e):
            parts = small.tile([128, nchb], F32, tag="parts")
            for ci, (c0, cw_) in enumerate(CH_B):
                p = ps.tile([128, ch], F32, tag="ps")
                for b in range(BLOC):
                    mm_into(p, WUH[e][:, :], X2, slice(0, 66), b * n + c0,
                            cw_, p_rows=slice(b * 64, b * 64 + 64))
                if ci % 2 == 0:
                    nc.scalar.activation(p[:, :cw_], p[:, :cw_], AF.Relu,
                                         bias=bias(f"ub2_{e}"),
                                         accum_out=parts[:, ci:ci + 1])
                else:
                    # DVE version of relu+bias+accum to balance engines
                    nc.vector.tensor_scalar(p[:, :cw_], p[:, :cw_],
                                            bias(f"ub2_{e}"), 0.0,
                                            op0=OP.add, op1=OP.max,
                                            accum_out=parts[:, ci:ci + 1])
            pooled2 = None
            if not flags[f"cfull_u{e}"]:
                scr = small.tile([128, nchb], F32, tag="pscr")
                pgs = small.tile([128, 1], F32, tag="pgs")
                nc.vector.tensor_scalar(scr[:, :], parts[:, :],
                                        bias(f"uu2_{e}"), 0.0, op0=OP.mult,
                                        op1=OP.add, accum_out=pgs[:, :])
                tp = tps.tile([1, 128], F32, tag="tp")
                nc.tensor.transpose(tp[:, :], pgs[:, :], IDENT[:, :])
                nc.vector.tensor_copy(WUA[e][0][64:65, 0:64], tp[0:1, 0:64])
                nc.vector.tensor_copy(WUA[e][1][64:65, 0:64],
                                      tp[0:1, 64:128])
            else:
                pooled2 = small.tile([128, 1], F32, tag="pooled")
                nc.vector.tensor_reduce(pooled2[:, :], parts[:, :],
                                        axis=AX.X, op=OP.add)
            # full-width q realign: one SBUF->SBUF DMA per batch elem,
            # issued while U1 still runs (sigmoid is long done)
            qts = []
            for b in range(BLOC):
                qt = sb.tile([64, n], BF16, tag="qt", bufs=2)
                nc.gpsimd.dma_start(qt[:, :], ZQ[64:128, b * n:(b + 1) * n])
                qts.append(qt)
            hcf = sb.tile([128, n], BF16, tag="hcf", bufs=2)
            # pass 2: res2(+aug) -> tanh -> combine into state
            for c0, cw_ in CH_B:
                p = ps.tile([128, ch], F32, tag="ps")
                for b in range(BLOC):
                    mm_into(p, WUA[e][b][:, :], X2, slice(0, 66), b * n + c0,
                            cw_, p_rows=slice(b * 64, b * 64 + 64))
                if flags[f"cfull_u{e}"]:
                    cuc = fpool.tile([128, ch], F32, tag="cgc")
                    nc.sync.dma_start(cuc[:, :cw_],
                                      dram[f"cu2_{e}"][:, c0:c0 + cw_])
                    nc.vector.scalar_tensor_tensor(
                        p[:, :cw_], cuc[:, :cw_], pooled2[:, :], p[:, :cw_],
                        op0=OP.mult, op1=OP.add)
                if flags[f"ab_u{e}"]:
                    abc = fpool.tile([128, ch], F32, tag="abc")
                    nc.sync.dma_start(abc[:, :cw_],
                                      dram[f"abu2_{e}"][:, c0:c0 + cw_])
                    nc.vector.tensor_add(p[:, :cw_], p[:, :cw_], abc[:, :cw_])
                nc.scalar.activation(hcf[:, c0:c0 + cw_], p[:, :cw_], AF.Tanh,
                                     bias=bias(f"uab2_{e}"))
                # b0 combine chunk-rolls right behind tanh
                csl = slice(c0, c0 + cw_)
                nc.vector.tensor_sub(hcf[0:64, csl], hcf[0:64, csl],
                                     X[0:64, csl])
                nc.vector.tensor_mul(hcf[0:64, csl], qts[0][:, csl],
                                     hcf[0:64, csl])
                nc.vector.tensor_add(X[0:64, csl], X[0:64, csl],
                                     hcf[0:64, csl])
            # b1: chunked realign (SWDGE; Pool engine is otherwise idle)
            for c0, cw_ in CH_B:
                csl = slice(c0, c0 + cw_)
                xsl = slice(n + c0, n + c0 + cw_)
                hct = sb.tile([64, ch], BF16, tag="hct", bufs=2)
                nc.gpsimd.dma_start(hct[:, :cw_], hcf[64:128, csl])
                nc.vector.tensor_sub(hct[:, :cw_], hct[:, :cw_], X[0:64, xsl])
                nc.vector.tensor_mul(hct[:, :cw_], qts[1][:, csl],
                                     hct[:, :cw_])
                nc.vector.tensor_add(X[0:64, xsl], X[0:64, xsl],
                                     hct[:, :cw_])

        def absorb(src_tile, we):
            # dummy ldweights pre-consuming {engine, DMA} waits on the PE so
            # subsequent real matmuls stay within the 2-slot MM wait limit
            # (real matmuls are self-loading, so this clobbers nothing)
            nc.tensor.ldweights(src_tile[0:66, 0:1])

        def encoder(e, xbf_src):
            nc.vector.memset(X[0:64, :], 0.0)
            nc.sync.dma_start(X2[64:65, :], dram[f"vu{e}"][:, :])
            nc.sync.dma_start(X[64:65, :], dram[f"vg{e}"][:, :])
            absorb(X, WGH[e])
            absorb(X2, WUH[e])
            for t in range(t_steps):
                nc.sync.dma_start(X2[65:66, :], xbf_src[t:t + 1, :])
                nc.sync.dma_start(X[65:66, :], xbf_src[t:t + 1, :])
                absorb(X, WGH[e])
                gfs_gate(e)
                absorb(X2, WUH[e])
                gfs_upd(e)

        # ================= encoder 1 =================
        encoder(0, srcbf)

        # heads 1+2 and encoder-2 input build
        for c0, cw_ in CH_W:
            p = ps.tile([2 * th, hch], F32, tag="ps")
            mm_into(p, CW01[:, :], X, slice(0, 64), c0, cw_)
            oc = sb.tile([2 * th, hch], F32, tag="hc", bufs=2)
            nc.scalar.activation(oc[:, :cw_], p[: 2 * th, :cw_], AF.Identity,
                                 bias=CVEC[0: 2 * th, cols["cb01"]:cols["cb01"] + 1])
            # rows 0:12 = src1 (head 1), rows 12:24 = out1 (head 0)
            nc.sync.dma_start(o1d[:, c0:c0 + cw_], oc[th: 2 * th, :cw_])
            sc = small.tile([th, hch], F32, tag="srcc")
            nc.sync.dma_start(sc[:, :cw_], src32[:, c0:c0 + cw_])
            xbb = sb.tile([th, hch], BF16, tag="xbb", bufs=2)
            nc.vector.tensor_sub(xbb[:, :cw_], sc[:, :cw_], oc[0:th, :cw_])
            nc.sync.dma_start(xsbf[:, c0:c0 + cw_], xbb[:, :cw_])

        # ================= encoder 2 =================
        encoder(1, xsbf)

        # head 3 + final sum
        for c0, cw_ in CH_W:
            p = ps.tile([th, hch], F32, tag="ps")
            mm_into(p, CW2[:, :], X, slice(0, 64), c0, cw_)
            o2 = sb.tile([th, hch], F32, tag="hc", bufs=2)
            nc.scalar.activation(o2[:, :cw_], p[:th, :cw_], AF.Identity,
                                 bias=CVEC[0:th, cols["cb2"]:cols["cb2"] + 1])
            o1c = sb.tile([th, hch], F32, tag="d", bufs=2)
            nc.sync.dma_start(o1c[:, :cw_], o1d[:, c0:c0 + cw_])
            nc.vector.tensor_add(o2[:, :cw_], o2[:, :cw_], o1c[:, :cw_])
            nc.sync.dma_start(out_d[:, c0:c0 + cw_], o2[:, :cw_])

    nc.compile()
    return nc


def _make_in_maps(inputs, H, n=N_FULL, t_steps=T):
    src = np.asarray(inputs["source"], np.float32)[..., 0]  # (B, T, n)
    in_maps = []
    for c in range(NCORES):
        m = dict(H["shared"])
        # src32[t, b*n + i] = src[2c+b, t, i]
        blk = src[BLOC * c: BLOC * (c + 1)]          # (BLOC, T, n)
        s = np.ascontiguousarray(
            blk.transpose(1, 0, 2).reshape(t_steps, BLOC * n))
        m["src32"] = s
        m["srcbf"] = s.astype(ml_dtypes.bfloat16)
        in_maps.append(m)
    return in_maps


def _assemble(results, n=N_FULL, t_steps=T):
    full = np.zeros((B, t_steps, n, 1), np.float32)
    for c in range(NCORES):
        o = np.asarray(results[c]["out"])            # [T, BLOC*n]
        o = o.reshape(t_steps, BLOC, n).transpose(1, 0, 2)
        full[BLOC * c: BLOC * (c + 1), :, :, 0] = o
    return full


_PROG_CACHE = {}


def kernel(**inputs) -> np.ndarray:
    H = _prep_host(inputs)
    key = tuple(sorted(H["flags"].items()))
    if key not in _PROG_CACHE:
        _PROG_CACHE[key] = _build(H)
    nc = _PROG_CACHE[key]
    in_maps = _make_in_maps(inputs, H)
    res = run_bass_kernel_spmd(nc, in_maps, core_ids=list(range(NCORES)))
    return _assemble(res.results)



# revision 14
# speedup vs baseline: 2.1639x; 2.1639x over previous
"""Trainium2 Bass kernel for nn_DDGCRN (gnn_message_passing).

DDGCRN: two 12-step GRU-style encoders over B=16, N=8600 nodes, HID=64,
with a global node-pooling term (GFS) inside each gate, plus conv heads.

Sharding: data-parallel over batch. B=16 / 8 cores = 2 batch elems per
core; the GFS node-pooling sum is per-batch-element, so no collectives.

Per-core layout: feature-major. All wide tensors are [D, W] where
W = 2*8600 = 17200 columns (col = b*N + n). State tile X is [66, W]:
rows 0:64 = hidden state, row 64 = x_t, row 65 = v (rank-1 pooling row).
Weight rows are host-permuted to match ([W_state; w_x; aug]).

The GFS pooled term affw*(pooled*scale) is rank-1 in (d, n) for the
given inputs (affw==1): C[d,n] = u[d]*v[n]. It is folded into the
"res" matmul by augmenting K: lhsT row 65 = pooled*u (written per cell
via PE-transpose), rhs row 65 = v (constant). Non-rank-1 / nonzero-affb
inputs fall back to explicit DVE ops with C/AB streamed from DRAM.
"""

import numpy as np
import ml_dtypes
from contextlib import ExitStack

import concourse.bass as bass
import concourse.bacc as bacc
import concourse.tile as tile
from concourse import mybir
from concourse.bass_utils import run_bass_kernel_spmd

F32 = mybir.dt.float32
F32R = mybir.dt.float32r
BF16 = mybir.dt.bfloat16
AX = mybir.AxisListType
OP = mybir.AluOpType
AF = mybir.ActivationFunctionType

# Problem constants (hardcoded; kernel.py must be self-contained)
B, T, N_FULL, HID, IN = 16, 12, 8600, 64, 1
GIN = IN + HID
NCORES = 8
BLOC = B // NCORES  # 2


def _chunks(total, size):
    out = []
    off = 0
    while off < total:
        w = min(size, total - off)
        out.append((off, w))
        off += w
    return out


def _rank1(C):
    """C [D, M] -> (u [D], v [M]) with C == outer(u, v), or None."""
    d0, m0 = np.unravel_index(np.argmax(np.abs(C)), C.shape)
    piv = C[d0, m0]
    if abs(piv) < 1e-30:
        return np.zeros(C.shape[0], np.float32), np.zeros(C.shape[1], np.float32)
    u = C[:, m0].astype(np.float64)
    v = C[d0, :].astype(np.float64) / piv
    if not np.allclose(np.outer(u, v), C, rtol=1e-5, atol=1e-7 * abs(piv)):
        return None
    return u.astype(np.float32), v.astype(np.float32)


def _prep_host(inputs, n=N_FULL, t_steps=T):
    """Host-side parameter prep. Returns dict of per-core-shared arrays
    plus flags. All weight matrices get the row permutation
    [rows 1:65 (state); row 0 (x); (aug row 0)]."""
    f32 = np.float32
    H = {"flags": {}}
    shared = {}

    def perm66(w):
        """[65, Dout] -> [66, Dout]: rows 0:64 = state weights, row 64 = 0
        (v-row slot / aug slot), row 65 = x weight."""
        z = np.zeros((1, w.shape[1]), f32)
        return np.concatenate([w[1:], z, w[0:1]], axis=0).astype(f32)

    for e in range(2):
        gaW = np.asarray(inputs["gate_alignW"][e], f32)   # [65, 128]
        gw = np.asarray(inputs["gate_w"][e], f32)         # [65, 128]
        gab = np.asarray(inputs["gate_alignb"][e], f32)   # [128]
        gb = np.asarray(inputs["gate_b"][e], f32)         # [128]
        uaW = np.asarray(inputs["upd_alignW"][e], f32)    # [65, 64]
        uw = np.asarray(inputs["upd_w"][e], f32)          # [65, 64]
        uab = np.asarray(inputs["upd_alignb"][e], f32)    # [64]
        ub = np.asarray(inputs["upd_b"][e], f32)          # [64]

        shared[f"wgh{e}"] = perm66(gw).astype(ml_dtypes.bfloat16)
        shared[f"wga{e}"] = perm66(gaW).astype(ml_dtypes.bfloat16)
        shared[f"wuh{e}"] = perm66(uw).astype(ml_dtypes.bfloat16)
        shared[f"wua{e}"] = perm66(uaW).astype(ml_dtypes.bfloat16)

        # rank-1 pooling factors: C[d, n] = affw[n, d] * aw[n] * nw[n]
        for kind, aff, aw, nw, dout in (
            ("g", inputs["gate_affw"][e], inputs["gate_aw"][e], inputs["gate_nw"][e], 128),
            ("u", inputs["upd_affw"][e], inputs["upd_aw"][e], inputs["upd_nw"][e], 64),
        ):
            scale = (np.asarray(aw, f32)[:, 0] * np.asarray(nw, f32)[0])  # [n]
            C = np.asarray(aff, f32).T * scale[None, :]  # [dout, n]
            r1 = _rank1(C)
            if r1 is None:
                H["flags"][f"cfull_{kind}{e}"] = True
                u = np.zeros(dout, f32)
                v = np.zeros(C.shape[1], f32)
                if kind == "g":
                    shared[f"cg{e}"] = np.ascontiguousarray(C)
                else:
                    shared[f"cu2_{e}"] = np.ascontiguousarray(
                        np.concatenate([C, C], axis=0))
            else:
                H["flags"][f"cfull_{kind}{e}"] = False
                u, v = r1
            if kind == "g":
                ug = u
                vg = v
            else:
                uu = u
                vu = v

        # affb fallback (AB tensors). gab/uab per-partition parts go in ACT bias.
        abg = np.asarray(inputs["gate_affb"][e], f32).T  # [128, n]
        abu = np.asarray(inputs["upd_affb"][e], f32).T   # [64, n]
        H["flags"][f"ab_g{e}"] = bool(np.any(abg))
        H["flags"][f"ab_u{e}"] = bool(np.any(abu))
        if H["flags"][f"ab_g{e}"]:
            shared[f"abg{e}"] = np.ascontiguousarray(abg)
        if H["flags"][f"ab_u{e}"]:
            shared[f"abu2_{e}"] = np.ascontiguousarray(
                np.concatenate([abu, abu], axis=0))

        # v rows, repeated per local batch elem
        shared[f"vg{e}"] = np.tile(vg[None, :], (1, BLOC)).astype(ml_dtypes.bfloat16)
        shared[f"vu{e}"] = np.tile(vu[None, :], (1, BLOC)).astype(ml_dtypes.bfloat16)

        # bias/scale vector columns
        H[f"gb{e}"] = gb
        H[f"bzr{e}"] = np.concatenate([gab[:64], -gab[64:]])
        H[f"ub2_{e}"] = np.concatenate([ub, ub])
        H[f"uab2_{e}"] = np.concatenate([uab, uab])
        H[f"ug{e}"] = ug
        H[f"uu2_{e}"] = np.concatenate([uu, uu])

    cw = np.asarray(inputs["conv_w"], f32)  # [3, 12, 64]
    cb = np.asarray(inputs["conv_b"], f32)  # [3, 12]
    # head order: [src1(12) | out1(12)] so src1 sits at partitions 0:12
    shared["cw01"] = np.concatenate([cw[1].T, cw[0].T], axis=1).astype(
        ml_dtypes.bfloat16)                                       # [64, 2T]
    shared["cw2"] = np.ascontiguousarray(cw[2].T).astype(ml_dtypes.bfloat16)
    shared["ident"] = np.eye(128, dtype=f32)

    # cvec columns
    ncol = 16
    cvec = np.zeros((128, ncol), f32)
    cvec[:64, 0] = 1.0
    cvec[64:, 0] = -1.0
    cols = {"szr": 0}
    ci = 1
    for e in range(2):
        for nm in ("gb", "bzr", "ub2_", "uab2_", "ug", "uu2_"):
            key = f"{nm}{e}"
            arr = H[key]
            cvec[: len(arr), ci] = arr
            cols[key] = ci
            ci += 1
    th = cb.shape[1]
    cvec[: 2 * th, ci] = np.concatenate([cb[1], cb[0]])
    cols["cb01"] = ci
    ci += 1
    cvec[:th, ci] = cb[2]
    cols["cb2"] = ci
    shared["cvec"] = cvec
    H["cols"] = cols
    H["shared"] = shared
    return H


def _build(H, n=N_FULL, t_steps=T, ch=1024, mmc=512):
    """Build the single-core Bass program (same for all cores)."""
    W = BLOC * n
    flags = H["flags"]
    cols = H["cols"]
    nc = bacc.Bacc("TRN2", target_bir_lowering=False, debug=False)

    dram = {}
    for name, arr in H["shared"].items():
        dram[name] = nc.declare_dram_parameter(
            name, list(arr.shape), mybir.dt.from_np(arr.dtype), isOutput=False)
    src32 = nc.declare_dram_parameter("src32", [t_steps, W], F32, isOutput=False)
    srcbf = nc.declare_dram_parameter("srcbf", [t_steps, W], BF16, isOutput=False)
    out_d = nc.declare_dram_parameter("out", [t_steps, W], F32, isOutput=True)
    o1d = nc.dram_tensor("o1d", [t_steps, W], F32)
    xsbf = nc.dram_tensor("xsbf", [t_steps, W], BF16)

    CH_B = _chunks(n, ch)       # chunks within one batch-half
    hch = min(ch, 1024)
    CH_W = _chunks(W, hch)      # chunks over full width (boundary passes)
    nchb = len(CH_B)

    with tile.TileContext(nc) as tc, ExitStack() as ctx:
        # ---- persistent tiles (one pool, distinct tags = one slot each) ----
        pers = ctx.enter_context(tc.tile_pool(name="pers", bufs=1))

        def ptile(shape, dtype, nm):
            return pers.tile(shape, dtype, name=nm, tag=nm)

        X = ptile([66, W], BF16, "X")
        X2 = ptile([66, W], BF16, "X2")
        ZQ = ptile([128, W], BF16, "ZQ")
        CVEC = ptile(list(H["shared"]["cvec"].shape), F32, "CVEC")
        IDENT = ptile([128, 128], F32, "IDENT")
        th = t_steps
        CW01 = ptile([64, 2 * th], BF16, "CW01")
        CW2 = ptile([64, th], BF16, "CW2")
        WGH = {}
        WGA = {}
        WUH = {}
        WUA = {}
        for e in range(2):
            WGH[e] = ptile([66, 128], BF16, f"twgh{e}")
            WUH[e] = ptile([66, 64], BF16, f"twuh{e}")
            WGA[e] = {}
            WUA[e] = {}
            for b in range(BLOC):
                WGA[e][b] = ptile([66, 128], BF16, f"twga{e}{b}")
                WUA[e][b] = ptile([66, 64], BF16, f"twua{e}{b}")

        # (Bacc.generate_event_semaphores legalizes wait counts at compile)
        nc.sync.dma_start(CVEC[:, :], dram["cvec"][:, :])
        nc.sync.dma_start(IDENT[:, :], dram["ident"][:, :])
        nc.sync.dma_start(CW01[:, :], dram["cw01"][:, :])
        nc.sync.dma_start(CW2[:, :], dram["cw2"][:, :])
        for e in range(2):
            nc.sync.dma_start(WGH[e][:, :], dram[f"wgh{e}"][:, :])
            nc.sync.dma_start(WUH[e][:, :], dram[f"wuh{e}"][:, :])
            for b in range(BLOC):
                nc.sync.dma_start(WGA[e][b][:, :], dram[f"wga{e}"][:, :])
                nc.sync.dma_start(WUA[e][b][:, :], dram[f"wua{e}"][:, :])

        def bias(key):
            return CVEC[:, cols[key]:cols[key] + 1]

        # ---- pools ----
        ps = ctx.enter_context(tc.tile_pool(name="ps", bufs=3, space="PSUM"))
        tps = ctx.enter_context(tc.tile_pool(name="tps", bufs=2, space="PSUM"))
        sb = ctx.enter_context(tc.tile_pool(name="sb", bufs=3))
        small = ctx.enter_context(tc.tile_pool(name="small", bufs=2))
        fpool = ctx.enter_context(tc.tile_pool(name="fpool", bufs=2))

        def mm_into(p, lhsT, rhs_tile, rhs_rows, coff, cw_, p_rows=None,
                    cast=None):
            """Matmul chunk [*, cw_] at col offset coff into psum tile p."""
            for m0, mw in _chunks(cw_, mmc):
                lhs_ap = lhsT
                rhs_ap = rhs_tile[rhs_rows, coff + m0:coff + m0 + mw]
                o = p[:, m0:m0 + mw] if p_rows is None else \
                    p[p_rows, m0:m0 + mw]
                if cast is not None:
                    lhs_ap = lhs_ap.bitcast(cast)
                    rhs_ap = rhs_ap.bitcast(cast)
                nc.tensor.matmul(o, lhs_ap, rhs_ap, start=True, stop=True)

        def gfs_gate(e):
            pooled_b = []
            for b in range(BLOC):
                boff = b * n
                parts = small.tile([128, nchb], F32, tag="parts")
                for ci, (c0, cw_) in enumerate(CH_B):
                    p = ps.tile([128, ch], F32, tag="ps")
                    mm_into(p, WGH[e][:, :], X, slice(0, 66), boff + c0, cw_)
                    if False:
                        # DVE relu+bias+accum for engine balance
                        nc.vector.tensor_scalar(p[:, :cw_], p[:, :cw_],
                                                bias(f"gb{e}"), 0.0,
                                                op0=OP.add, op1=OP.max,
                                                accum_out=parts[:, ci:ci + 1])
                    else:
                        nc.scalar.activation(p[:, :cw_], p[:, :cw_], AF.Relu,
                                             bias=bias(f"gb{e}"),
                                             accum_out=parts[:, ci:ci + 1])
                if not flags[f"cfull_g{e}"]:
                    # fused: scratch = parts*u, accum = sum = pooled*u
                    scr = small.tile([128, nchb], F32, tag="pscr")
                    pgs = small.tile([128, 1], F32, tag="pgs")
                    nc.vector.tensor_scalar(scr[:, :], parts[:, :],
                                            bias(f"ug{e}"), 0.0, op0=OP.mult,
                                            op1=OP.add, accum_out=pgs[:, :])
                    tp = tps.tile([1, 128], F32, tag="tp")
                    nc.tensor.transpose(tp[:, :], pgs[:, :], IDENT[:, :])
                    nc.vector.tensor_copy(WGA[e][b][64:65, 0:128],
                                          tp[0:1, 0:128])
                    pooled_b.append(None)
                else:
                    pooled = small.tile([128, 1], F32, tag="pooled")
                    nc.vector.tensor_reduce(pooled[:, :], parts[:, :],
                                            axis=AX.X, op=OP.add)
                    pooled_b.append(pooled)
            # pass 2: res(+aug) matmul -> sigmoid -> ZQ (z rows 0:64, q 64:128)
            for b in range(BLOC):
                boff = b * n
                for c0, cw_ in CH_B:
                    p = ps.tile([128, ch], F32, tag="ps")
                    mm_into(p, WGA[e][b][:, :], X, slice(0, 66), boff + c0,
                            cw_)
                    if flags[f"cfull_g{e}"]:
                        cgc = fpool.tile([128, ch], F32, tag="cgc")
                        nc.sync.dma_start(cgc[:, :cw_],
                                          dram[f"cg{e}"][:, c0:c0 + cw_])
                        nc.vector.scalar_tensor_tensor(
                            p[:, :cw_], cgc[:, :cw_], pooled_b[b][:, :],
                            p[:, :cw_], op0=OP.mult, op1=OP.add)
                    if flags[f"ab_g{e}"]:
                        abc = fpool.tile([128, ch], F32, tag="abc")
                        nc.sync.dma_start(abc[:, :cw_],
                                          dram[f"abg{e}"][:, c0:c0 + cw_])
                        nc.vector.tensor_add(p[:, :cw_], p[:, :cw_],
                                             abc[:, :cw_])
                    nc.scalar.activation(ZQ[:, boff + c0:boff + c0 + cw_],
                                         p[:, :cw_], AF.Sigmoid,
                                         bias=bias(f"bzr{e}"),
                                         scale=bias("szr"))
                    # zs = z * S -> X2 state rows (b0 on gpsimd for balance)
                    zeng = nc.gpsimd if b == 0 else nc.vector
                    zeng.tensor_mul(
                        X2[0:64, boff + c0:boff + c0 + cw_],
                        ZQ[0:64, boff + c0:boff + c0 + cw_],
                        X[0:64, boff + c0:boff + c0 + cw_])

        def gfs_upd(e):
            parts = small.tile([128, nchb], F32, tag="parts")
            for ci, (c0, cw_) in enumerate(CH_B):
                p = ps.tile([128, ch], F32, tag="ps")
                for b in range(BLOC):
                    mm_into(p, WUH[e][:, :], X2, slice(0, 66), b * n + c0,
                            cw_, p_rows=slice(b * 64, b * 64 + 64))
                if ci % 2 == 0:
                    nc.scalar.activation(p[:, :cw_], p[:, :cw_], AF.Relu,
                                         bias=bias(f"ub2_{e}"),
                                         accum_out=parts[:, ci:ci + 1])
                else:
                    # DVE version of relu+bias+accum to balance engines
                    nc.vector.tensor_scalar(p[:, :cw_], p[:, :cw_],
                                            bias(f"ub2_{e}"), 0.0,
                                            op0=OP.add, op1=OP.max,
                                            accum_out=parts[:, ci:ci + 1])
            pooled2 = None
            if not flags[f"cfull_u{e}"]:
                scr = small.tile([128, nchb], F32, tag="pscr")
                pgs = small.tile([128, 1], F32, tag="pgs")
                nc.vector.tensor_scalar(scr[:, :], parts[:, :],
                                        bias(f"uu2_{e}"), 0.0, op0=OP.mult,
                                        op1=OP.add, accum_out=pgs[:, :])
                tp = tps.tile([1, 128], F32, tag="tp")
                nc.tensor.transpose(tp[:, :], pgs[:, :], IDENT[:, :])
                nc.vector.tensor_copy(WUA[e][0][64:65, 0:64], tp[0:1, 0:64])
                nc.vector.tensor_copy(WUA[e][1][64:65, 0:64],
                                      tp[0:1, 64:128])
            else:
                pooled2 = small.tile([128, 1], F32, tag="pooled")
                nc.vector.tensor_reduce(pooled2[:, :], parts[:, :],
                                        axis=AX.X, op=OP.add)
            # full-width q realign: one SBUF->SBUF DMA per batch elem,
            # issued while U1 still runs (sigmoid is long done)
            qts = []
            for b in range(BLOC):
                qt = sb.tile([64, n], BF16, tag="qt", bufs=2)
                nc.gpsimd.dma_start(qt[:, :], ZQ[64:128, b * n:(b + 1) * n])
                qts.append(qt)
            hcf = sb.tile([128, n], BF16, tag="hcf", bufs=2)
            # pass 2: res2(+aug) -> tanh -> combine into state
            for c0, cw_ in CH_B:
                p = ps.tile([128, ch], F32, tag="ps")
                for b in range(BLOC):
                    mm_into(p, WUA[e][b][:, :], X2, slice(0, 66), b * n + c0,
                            cw_, p_rows=slice(b * 64, b * 64 + 64))
                if flags[f"cfull_u{e}"]:
                    cuc = fpool.tile([128, ch], F32, tag="cgc")
                    nc.sync.dma_start(cuc[:, :cw_],
                                      dram[f"cu2_{e}"][:, c0:c0 + cw_])
                    nc.vector.scalar_tensor_tensor(
                        p[:, :cw_], cuc[:, :cw_], pooled2[:, :], p[:, :cw_],
                        op0=OP.mult, op1=OP.add)
                if flags[f"ab_u{e}"]:
                    abc = fpool.tile([128, ch], F32, tag="abc")
                    nc.sync.dma_start(abc[:, :cw_],
                                      dram[f"abu2_{e}"][:, c0:c0 + cw_])
                    nc.vector.tensor_add(p[:, :cw_], p[:, :cw_], abc[:, :cw_])
                nc.scalar.activation(hcf[:, c0:c0 + cw_], p[:, :cw_], AF.Tanh,
                                     bias=bias(f"uab2_{e}"))
                # b0 combine chunk-rolls right behind tanh
                csl = slice(c0, c0 + cw_)
                nc.vector.tensor_sub(hcf[0:64, csl], hcf[0:64, csl],
                                     X[0:64, csl])
                nc.vector.tensor_mul(hcf[0:64, csl], qts[0][:, csl],
                                     hcf[0:64, csl])
                nc.vector.tensor_add(X[0:64, csl], X[0:64, csl],
                                     hcf[0:64, csl])
            # b1: chunked realign (SWDGE; Pool engine is otherwise idle)
            for c0, cw_ in CH_B:
                csl = slice(c0, c0 + cw_)
                xsl = slice(n + c0, n + c0 + cw_)
                hct = sb.tile([64, ch], BF16, tag="hct", bufs=2)
                nc.gpsimd.dma_start(hct[:, :cw_], hcf[64:128, csl])
                nc.vector.tensor_sub(hct[:, :cw_], hct[:, :cw_], X[0:64, xsl])
                nc.vector.tensor_mul(hct[:, :cw_], qts[1][:, csl],
                                     hct[:, :cw_])
                nc.vector.tensor_add(X[0:64, xsl], X[0:64, xsl],
                                     hct[:, :cw_])

        def absorb(src_tile, we):
            # dummy ldweights pre-consuming {engine, DMA} waits on the PE so
            # subsequent real matmuls stay within the 2-slot MM wait limit
            # (real matmuls are self-loading, so this clobbers nothing)
            nc.tensor.ldweights(src_tile[0:66, 0:1])

        def encoder(e, xbf_src):
            nc.vector.memset(X[0:64, :], 0.0)
            nc.sync.dma_start(X2[64:65, :], dram[f"vu{e}"][:, :])
            nc.sync.dma_start(X[64:65, :], dram[f"vg{e}"][:, :])
            absorb(X, WGH[e])
            absorb(X2, WUH[e])
            for t in range(t_steps):
                nc.sync.dma_start(X2[65:66, :], xbf_src[t:t + 1, :])
                nc.sync.dma_start(X[65:66, :], xbf_src[t:t + 1, :])
                absorb(X, WGH[e])
                gfs_gate(e)
                absorb(X2, WUH[e])
                gfs_upd(e)

        # ================= encoder 1 =================
        encoder(0, srcbf)

        # heads 1+2 and encoder-2 input build
        for c0, cw_ in CH_W:
            p = ps.tile([2 * th, hch], F32, tag="ps")
            mm_into(p, CW01[:, :], X, slice(0, 64), c0, cw_)
            oc = sb.tile([2 * th, hch], F32, tag="hc", bufs=2)
            nc.scalar.activation(oc[:, :cw_], p[: 2 * th, :cw_], AF.Identity,
                                 bias=CVEC[0: 2 * th, cols["cb01"]:cols["cb01"] + 1])
            # rows 0:12 = src1 (head 1), rows 12:24 = out1 (head 0)
            nc.sync.dma_start(o1d[:, c0:c0 + cw_], oc[th: 2 * th, :cw_])
            sc = small.tile([th, hch], F32, tag="srcc")
            nc.sync.dma_start(sc[:, :cw_], src32[:, c0:c0 + cw_])
            xbb = sb.tile([th, hch], BF16, tag="xbb", bufs=2)
            nc.vector.tensor_sub(xbb[:, :cw_], sc[:, :cw_], oc[0:th, :cw_])
            nc.sync.dma_start(xsbf[:, c0:c0 + cw_], xbb[:, :cw_])

        # ================= encoder 2 =================
        encoder(1, xsbf)

        # head 3 + final sum
        for c0, cw_ in CH_W:
            p = ps.tile([th, hch], F32, tag="ps")
            mm_into(p, CW2[:, :], X, slice(0, 64), c0, cw_)
            o2 = sb.tile([th, hch], F32, tag="hc", bufs=2)
            nc.scalar.activation(o2[:, :cw_], p[:th, :cw_], AF.Identity,
                                 bias=CVEC[0:th, cols["cb2"]:cols["cb2"] + 1])
            o1c = sb.tile([th, hch], F32, tag="d", bufs=2)
            nc.sync.dma_start(o1c[:, :cw_], o1d[:, c0:c0 + cw_])
            nc.vector.tensor_add(o2[:, :cw_], o2[:, :cw_], o1c[:, :cw_])
            nc.sync.dma_start(out_d[:, c0:c0 + cw_], o2[:, :cw_])

    nc.compile()
    return nc


def _make_in_maps(inputs, H, n=N_FULL, t_steps=T):
    src = np.asarray(inputs["source"], np.float32)[..., 0]  # (B, T, n)
    in_maps = []
    for c in range(NCORES):
        m = dict(H["shared"])
        # src32[t, b*n + i] = src[2c+b, t, i]
        blk = src[BLOC * c: BLOC * (c + 1)]          # (BLOC, T, n)
        s = np.ascontiguousarray(
            blk.transpose(1, 0, 2).reshape(t_steps, BLOC * n))
        m["src32"] = s
        m["srcbf"] = s.astype(ml_dtypes.bfloat16)
        in_maps.append(m)
    return in_maps


def _assemble(results, n=N_FULL, t_steps=T):
    full = np.zeros((B, t_steps, n, 1), np.float32)
    for c in range(NCORES):
        o = np.asarray(results[c]["out"])            # [T, BLOC*n]
        o = o.reshape(t_steps, BLOC, n).transpose(1, 0, 2)
        full[BLOC * c: BLOC * (c + 1), :, :, 0] = o
    return full


# ---------------------------------------------------------------------------
# Fast path: GFS pooled term provably negligible -> drop both pooled matmul
# passes. State is b-packed [128, n] (partitions 64b+d); x_t contributions
# enter via K=1/K=2 accumulating matmuls from a resident [24, n] source tile
# (rows 2t+b), eliminating the per-step single-partition x-row DMAs.
# ---------------------------------------------------------------------------

TAU_MAX = 1e-3  # bound on the dropped pooled term's pre-activation magnitude


def _prep_fast(inputs):
    """Return fast-path host dict, or None if the pooled term is not
    provably negligible / affb nonzero."""
    f32 = np.float32
    n, th = N_FULL, T
    src = np.asarray(inputs["source"], f32)
    cw = np.asarray(inputs["conv_w"], f32)
    cb = np.asarray(inputs["conv_b"], f32)
    if np.any(np.asarray(inputs["gate_affb"])) or np.any(
            np.asarray(inputs["upd_affb"])):
        return None
    xmax0 = float(np.abs(src).max())
    s1max = float(np.max(np.sum(np.abs(cw[1]), axis=1) + np.abs(cb[1])))
    xmax = [xmax0, xmax0 + s1max]
    for e in range(2):
        for w_, b_, aff, aw, nw in (
            (inputs["gate_w"][e], inputs["gate_b"][e], inputs["gate_affw"][e],
             inputs["gate_aw"][e], inputs["gate_nw"][e]),
            (inputs["upd_w"][e], inputs["upd_b"][e], inputs["upd_affw"][e],
             inputs["upd_aw"][e], inputs["upd_nw"][e]),
        ):
            w_ = np.asarray(w_, f32)
            b_ = np.asarray(b_, f32)
            cmax = float(np.abs(np.asarray(aff, f32).T
                                * (np.asarray(aw, f32)[:, 0]
                                   * np.asarray(nw, f32)[0])[None, :]).max())
            premax = np.abs(b_) + np.abs(w_[0]) * xmax[e] + \
                np.sum(np.abs(w_[1:]), axis=0)
            tau = cmax * n * float(np.maximum(premax, 0.0).max())
            if tau > TAU_MAX:
                return None

    bf = ml_dtypes.bfloat16
    H = {"shared": {}}
    sh = H["shared"]
    for e in range(2):
        gaW = np.asarray(inputs["gate_alignW"][e], f32)   # [65, 128]
        gab = np.asarray(inputs["gate_alignb"][e], f32)   # [128]
        uaW = np.asarray(inputs["upd_alignW"][e], f32)    # [65, 64]
        uab = np.asarray(inputs["upd_alignb"][e], f32)    # [64]
        # b1 rows use swapped z/q column blocks so that z lands on psum rows
        # 64:128 for b1 (partition-aligned with the b-packed state) and q' on
        # 0:64; this keeps every gating TensorTensor op same-start-partition.
        swap = np.concatenate([gaW[:, 64:128], gaW[:, 0:64]], axis=1)
        wgp = np.zeros((128, 128), f32)
        wgp[0:64] = gaW[1:65]
        wgp[64:128] = swap[1:65]
        sh[f"wgp{e}"] = wgp.astype(bf)
        wxga = np.zeros((98, 128), f32)
        wxgb = np.zeros((98, 128), f32)
        for q in (0, 32, 64, 96):
            wxga[q] = gaW[0]          # K=1 row for gate b0
            wxgb[q + 1] = swap[0]     # K=2 rows [0; wx_swapped] for gate b1
        sh[f"wxga{e}"] = wxga.astype(bf)
        sh[f"wxgb{e}"] = wxgb.astype(bf)
        wubd = np.zeros((128, 128), f32)
        wubd[0:64, 0:64] = uaW[1:65]
        wubd[64:128, 64:128] = uaW[1:65]
        sh[f"wubd{e}"] = wubd.astype(bf)
        wxu = np.zeros((98, 128), f32)
        for q in (0, 32, 64, 96):
            wxu[q, 0:64] = uaW[0]
            wxu[q + 1, 64:128] = uaW[0]
        sh[f"wxu{e}"] = wxu.astype(bf)
        H[f"bzr{e}"] = np.concatenate([gab[:64], -gab[64:]])
        H[f"bzrS{e}"] = np.concatenate([-gab[64:], gab[:64]])
        H[f"uab2_{e}"] = np.concatenate([uab, uab])

    cwh1a = np.zeros((128, 24), f32)
    cwh1b = np.zeros((128, 24), f32)
    cwh2 = np.zeros((128, 24), f32)
    cb1p = np.zeros(24, f32)
    cb0p = np.zeros(24, f32)
    cb2p = np.zeros(24, f32)
    for b in range(2):
        for o in range(th):
            cwh1a[64 * b:64 * b + 64, 2 * o + b] = cw[1][o]
            cwh1b[64 * b:64 * b + 64, 2 * o + b] = cw[0][o]
            cwh2[64 * b:64 * b + 64, 2 * o + b] = cw[2][o]
            cb1p[2 * o + b] = cb[1][o]
            cb0p[2 * o + b] = cb[0][o]
            cb2p[2 * o + b] = cb[2][o]
    sh["cwh1a"] = cwh1a.astype(bf)
    sh["cwh1b"] = cwh1b.astype(bf)
    sh["cwh2"] = cwh2.astype(bf)

    cvec = np.zeros((128, 11), f32)
    cvec[0:64, 0] = 1.0        # szr (b0): z rows +, q rows -
    cvec[64:128, 0] = -1.0
    cvec[0:64, 1] = -1.0       # szrS (b1 swapped): q rows -, z rows +
    cvec[64:128, 1] = 1.0
    cvec[:, 2] = H["bzr0"]
    cvec[:, 3] = H["bzrS0"]
    cvec[:, 4] = H["bzr1"]
    cvec[:, 5] = H["bzrS1"]
    cvec[:, 6] = H["uab2_0"]
    cvec[:, 7] = H["uab2_1"]
    cvec[0:24, 8] = cb1p
    cvec[0:24, 9] = cb2p
    cvec[0:24, 10] = cb0p
    sh["cvec"] = cvec
    return H


def _build_fast(H, n=N_FULL, t_steps=T, ch=2048, mmc=512):
    nc = bacc.Bacc("TRN2", target_bir_lowering=False, debug=False)
    dram = {}
    for name, arr in H["shared"].items():
        dram[name] = nc.declare_dram_parameter(
            name, list(arr.shape), mybir.dt.from_np(arr.dtype), isOutput=False)
    srcp_d = nc.declare_dram_parameter("srcp", [2 * t_steps, n], BF16,
                                       isOutput=False)
    src32_d = nc.declare_dram_parameter("src32p", [2 * t_steps, n], F32,
                                        isOutput=False)
    out_d = nc.declare_dram_parameter("out", [2 * t_steps, n], F32,
                                      isOutput=True)
    xs2_d = nc.dram_tensor("xs2", [2 * t_steps, n], BF16)
    CH = _chunks(n, ch)

    with tile.TileContext(nc) as tc, ExitStack() as ctx:
        pers = ctx.enter_context(tc.tile_pool(name="pers", bufs=1))

        def ptile(shape, dtype, nm):
            return pers.tile(shape, dtype, name=nm, tag=nm)

        SP = ptile([128, n], BF16, "SP")
        X2P = ptile([128, n], BF16, "X2P")
        QP = ptile([128, n], BF16, "QP")
        HC = ptile([128, n], BF16, "HC")
        ZQ = ptile([128, 2 * n], BF16, "ZQ")
        XQ = ptile([98, n], BF16, "XQ")
        O1B = ptile([24, n], F32, "O1B")
        CVEC = ptile(list(H["shared"]["cvec"].shape), F32, "CVEC")
        CWH1A = ptile([128, 24], BF16, "CWH1A")
        CWH1B = ptile([128, 24], BF16, "CWH1B")
        CWH2 = ptile([128, 24], BF16, "CWH2")
        WGP, WXGA, WXGB, WUBD, WXU = {}, {}, {}, {}, {}
        for e in range(2):
            WGP[e] = ptile([128, 128], BF16, f"wgp{e}")
            WXGA[e] = ptile([98, 128], BF16, f"wxga{e}")
            WXGB[e] = ptile([98, 128], BF16, f"wxgb{e}")
            WUBD[e] = ptile([128, 128], BF16, f"wubd{e}")
            WXU[e] = ptile([98, 128], BF16, f"wxu{e}")

        nc.sync.dma_start(CVEC[:, :], dram["cvec"][:, :])
        nc.sync.dma_start(CWH1A[:, :], dram["cwh1a"][:, :])
        nc.sync.dma_start(CWH1B[:, :], dram["cwh1b"][:, :])
        nc.sync.dma_start(CWH2[:, :], dram["cwh2"][:, :])
        for e in range(2):
            nc.sync.dma_start(WGP[e][:, :], dram[f"wgp{e}"][:, :])
            nc.sync.dma_start(WXGA[e][:, :], dram[f"wxga{e}"][:, :])
            nc.sync.dma_start(WXGB[e][:, :], dram[f"wxgb{e}"][:, :])
            nc.sync.dma_start(WUBD[e][:, :], dram[f"wubd{e}"][:, :])
            nc.sync.dma_start(WXU[e][:, :], dram[f"wxu{e}"][:, :])

        ps = ctx.enter_context(tc.tile_pool(name="ps", bufs=2, space="PSUM"))
        sb = ctx.enter_context(tc.tile_pool(name="sb", bufs=2))

        def bias(i, p=128):
            return CVEC[0:p, i:i + 1]

        def xq_load(src_ap, t):
            q = 32 * (t % 4)
            nc.sync.dma_start(XQ[q:q + 2, :], src_ap[2 * t:2 * t + 2, :])

        def cell(e, t, src_ap):
            q = 32 * (t % 4)
            # ---- gate: ZQ b-width; b0 cols rows 0:64=z, 64:128=q';
            #      b1 cols (swapped weights) rows 0:64=q', 64:128=z ----
            for b in range(2):
                boff = b * n
                bp = slice(64 * b, 64 * b + 64)
                for c0, cw_ in CH:
                    p = ps.tile([128, ch], F32, tag="ps")
                    for m0, mw in _chunks(cw_, mmc):
                        msl = slice(c0 + m0, c0 + m0 + mw)
                        nc.tensor.matmul(p[:, m0:m0 + mw], WGP[e][bp, :],
                                         SP[bp, msl], start=True, stop=False)
                        if b == 0:
                            nc.tensor.matmul(p[:, m0:m0 + mw],
                                             WXGA[e][q:q + 1, :],
                                             XQ[q:q + 1, msl],
                                             start=False, stop=True,
                                             tile_position=(q, 0))
                        else:
                            nc.tensor.matmul(p[:, m0:m0 + mw],
                                             WXGB[e][q:q + 2, :],
                                             XQ[q:q + 2, msl],
                                             start=False, stop=True,
                                             tile_position=(q, 0))
                    osl = slice(boff + c0, boff + c0 + cw_)
                    nc.scalar.activation(ZQ[:, osl], p[:, :cw_], AF.Sigmoid,
                                         bias=bias(2 + 2 * e + b),
                                         scale=bias(b))
                    csl = slice(c0, c0 + cw_)
                    # zs: z rows co-located with the b-packed state half
                    zeng = nc.vector if b == 0 else nc.gpsimd
                    zeng.tensor_mul(X2P[bp, csl], ZQ[bp, osl], SP[bp, csl])
                    # realign q' into b-packed QP (cross-partition copy)
                    qp = slice(64 - 64 * b, 128 - 64 * b)
                    ceng = nc.gpsimd if b == 0 else nc.vector
                    ceng.tensor_copy(QP[bp, csl], ZQ[qp, osl])
            # ---- upd: hc = tanh(pre + uab2), b-packed; combine ----
            for c0, cw_ in CH:
                p = ps.tile([128, ch], F32, tag="ps")
                for m0, mw in _chunks(cw_, mmc):
                    msl = slice(c0 + m0, c0 + m0 + mw)
                    nc.tensor.matmul(p[:, m0:m0 + mw], WUBD[e][:, :],
                                     X2P[:, msl], start=True, stop=False)
                    nc.tensor.matmul(p[:, m0:m0 + mw], WXU[e][q:q + 2, :],
                                     XQ[q:q + 2, msl],
                                     start=False, stop=True,
                                     tile_position=(q, 0))
                csl = slice(c0, c0 + cw_)
                nc.scalar.activation(HC[:, csl], p[:, :cw_], AF.Tanh,
                                     bias=bias(6 + e))
                nc.vector.tensor_sub(HC[:, csl], HC[:, csl], SP[:, csl])
                nc.gpsimd.tensor_mul(HC[:, csl], QP[:, csl], HC[:, csl])
                nc.vector.tensor_add(SP[:, csl], SP[:, csl], HC[:, csl])
            # prefetch x for step t+4 into the quadrant this step just freed
            if t + 4 < t_steps:
                xq_load(src_ap, t + 4)

        # ================= encoder 1 =================
        nc.vector.memset(SP[:, :], 0.0)
        for t in range(min(4, t_steps)):
            xq_load(srcp_d, t)
        for t in range(t_steps):
            cell(0, t, srcp_d)

        # heads 1+2 and encoder-2 input build
        hch = 1024
        for c0, cw_ in _chunks(n, hch):
            p1 = ps.tile([24, hch], F32, tag="ps")
            p2 = ps.tile([24, hch], F32, tag="ps")
            for m0, mw in _chunks(cw_, mmc):
                msl = slice(c0 + m0, c0 + m0 + mw)
                nc.tensor.matmul(p1[:, m0:m0 + mw], CWH1A[:, :], SP[:, msl],
                                 start=True, stop=True)
                nc.tensor.matmul(p2[:, m0:m0 + mw], CWH1B[:, :], SP[:, msl],
                                 start=True, stop=True)
            csl = slice(c0, c0 + cw_)
            o1a = sb.tile([24, hch], F32, tag="o1a")
            nc.scalar.activation(o1a[:, :cw_], p1[:24, :cw_], AF.Identity,
                                 bias=bias(8, 24))
            nc.scalar.activation(O1B[:, csl], p2[:24, :cw_], AF.Identity,
                                 bias=bias(10, 24))
            sc = sb.tile([24, hch], F32, tag="srcc")
            nc.sync.dma_start(sc[:, :cw_], src32_d[:, csl])
            x2c = sb.tile([24, hch], BF16, tag="x2c")
            nc.vector.tensor_sub(x2c[:, :cw_], sc[:, :cw_], o1a[:, :cw_])
            nc.sync.dma_start(xs2_d[:, csl], x2c[:, :cw_])

        # ================= encoder 2 =================
        nc.vector.memset(SP[:, :], 0.0)
        for t in range(min(4, t_steps)):
            xq_load(xs2_d, t)
        for t in range(t_steps):
            cell(1, t, xs2_d)

        # head 3 + final sum
        hch = 1024
        for c0, cw_ in _chunks(n, hch):
            p = ps.tile([24, hch], F32, tag="ps")
            for m0, mw in _chunks(cw_, mmc):
                msl = slice(c0 + m0, c0 + m0 + mw)
                nc.tensor.matmul(p[0:24, m0:m0 + mw], CWH2[:, :], SP[:, msl],
                                 start=True, stop=True)
            csl = slice(c0, c0 + cw_)
            o2 = sb.tile([24, hch], F32, tag="o2")
            nc.scalar.activation(o2[:, :cw_], p[0:24, :cw_], AF.Identity,
                                 bias=bias(9, 24))
            nc.vector.tensor_add(o2[:, :cw_], o2[:, :cw_], O1B[:, csl])
            nc.sync.dma_start(out_d[:, csl], o2[:, :cw_])

    nc.compile()
    return nc


def _make_in_maps_fast(inputs, H, n=N_FULL, t_steps=T):
    src = np.asarray(inputs["source"], np.float32)[..., 0]  # (B, T, n)
    in_maps = []
    for c in range(NCORES):
        m = dict(H["shared"])
        s = np.zeros((2 * t_steps, n), np.float32)
        for b in range(BLOC):
            s[b::2] = src[BLOC * c + b]  # row 2t+b = src[2c+b, t]
        m["src32p"] = s
        m["srcp"] = s.astype(ml_dtypes.bfloat16)
        in_maps.append(m)
    return in_maps


def _assemble_fast(results, n=N_FULL, t_steps=T):
    full = np.zeros((B, t_steps, n, 1), np.float32)
    for c in range(NCORES):
        o = np.asarray(results[c]["out"])            # [2T, n], row 2o+b
        for b in range(BLOC):
            full[BLOC * c + b, :, :, 0] = o[b::2]
    return full


_PROG_CACHE = {}


def _prepare(inputs):
    """Pick fast/exact path; return (nc, in_maps, assemble_fn)."""
    HF = _prep_fast(inputs)
    if HF is not None:
        if "fast" not in _PROG_CACHE:
            _PROG_CACHE["fast"] = _build_fast(HF)
        return _PROG_CACHE["fast"], _make_in_maps_fast(inputs, HF), \
            _assemble_fast
    H = _prep_host(inputs)
    key = tuple(sorted(H["flags"].items()))
    if key not in _PROG_CACHE:
        _PROG_CACHE[key] = _build(H)
    return _PROG_CACHE[key], _make_in_maps(inputs, H), _assemble


def kernel(**inputs) -> np.ndarray:
    nc, in_maps, assemble = _prepare(inputs)
    res = run_bass_kernel_spmd(nc, in_maps, core_ids=list(range(NCORES)))
    return assemble(res.results)



# revision 15
# speedup vs baseline: 2.1741x; 1.0047x over previous
"""Trainium2 Bass kernel for nn_DDGCRN (gnn_message_passing).

DDGCRN: two 12-step GRU-style encoders over B=16, N=8600 nodes, HID=64,
with a global node-pooling term (GFS) inside each gate, plus conv heads.

Sharding: data-parallel over batch. B=16 / 8 cores = 2 batch elems per
core; the GFS node-pooling sum is per-batch-element, so no collectives.

Per-core layout: feature-major. All wide tensors are [D, W] where
W = 2*8600 = 17200 columns (col = b*N + n). State tile X is [66, W]:
rows 0:64 = hidden state, row 64 = x_t, row 65 = v (rank-1 pooling row).
Weight rows are host-permuted to match ([W_state; w_x; aug]).

The GFS pooled term affw*(pooled*scale) is rank-1 in (d, n) for the
given inputs (affw==1): C[d,n] = u[d]*v[n]. It is folded into the
"res" matmul by augmenting K: lhsT row 65 = pooled*u (written per cell
via PE-transpose), rhs row 65 = v (constant). Non-rank-1 / nonzero-affb
inputs fall back to explicit DVE ops with C/AB streamed from DRAM.
"""

import numpy as np
import ml_dtypes
from contextlib import ExitStack

import concourse.bass as bass
import concourse.bacc as bacc
import concourse.tile as tile
from concourse import mybir
from concourse.bass_utils import run_bass_kernel_spmd

F32 = mybir.dt.float32
F32R = mybir.dt.float32r
BF16 = mybir.dt.bfloat16
AX = mybir.AxisListType
OP = mybir.AluOpType
AF = mybir.ActivationFunctionType

# Problem constants (hardcoded; kernel.py must be self-contained)
B, T, N_FULL, HID, IN = 16, 12, 8600, 64, 1
GIN = IN + HID
NCORES = 8
BLOC = B // NCORES  # 2


def _chunks(total, size):
    out = []
    off = 0
    while off < total:
        w = min(size, total - off)
        out.append((off, w))
        off += w
    return out


def _rank1(C):
    """C [D, M] -> (u [D], v [M]) with C == outer(u, v), or None."""
    d0, m0 = np.unravel_index(np.argmax(np.abs(C)), C.shape)
    piv = C[d0, m0]
    if abs(piv) < 1e-30:
        return np.zeros(C.shape[0], np.float32), np.zeros(C.shape[1], np.float32)
    u = C[:, m0].astype(np.float64)
    v = C[d0, :].astype(np.float64) / piv
    if not np.allclose(np.outer(u, v), C, rtol=1e-5, atol=1e-7 * abs(piv)):
        return None
    return u.astype(np.float32), v.astype(np.float32)


def _prep_host(inputs, n=N_FULL, t_steps=T):
    """Host-side parameter prep. Returns dict of per-core-shared arrays
    plus flags. All weight matrices get the row permutation
    [rows 1:65 (state); row 0 (x); (aug row 0)]."""
    f32 = np.float32
    H = {"flags": {}}
    shared = {}

    def perm66(w):
        """[65, Dout] -> [66, Dout]: rows 0:64 = state weights, row 64 = 0
        (v-row slot / aug slot), row 65 = x weight."""
        z = np.zeros((1, w.shape[1]), f32)
        return np.concatenate([w[1:], z, w[0:1]], axis=0).astype(f32)

    for e in range(2):
        gaW = np.asarray(inputs["gate_alignW"][e], f32)   # [65, 128]
        gw = np.asarray(inputs["gate_w"][e], f32)         # [65, 128]
        gab = np.asarray(inputs["gate_alignb"][e], f32)   # [128]
        gb = np.asarray(inputs["gate_b"][e], f32)         # [128]
        uaW = np.asarray(inputs["upd_alignW"][e], f32)    # [65, 64]
        uw = np.asarray(inputs["upd_w"][e], f32)          # [65, 64]
        uab = np.asarray(inputs["upd_alignb"][e], f32)    # [64]
        ub = np.asarray(inputs["upd_b"][e], f32)          # [64]

        shared[f"wgh{e}"] = perm66(gw).astype(ml_dtypes.bfloat16)
        shared[f"wga{e}"] = perm66(gaW).astype(ml_dtypes.bfloat16)
        shared[f"wuh{e}"] = perm66(uw).astype(ml_dtypes.bfloat16)
        shared[f"wua{e}"] = perm66(uaW).astype(ml_dtypes.bfloat16)

        # rank-1 pooling factors: C[d, n] = affw[n, d] * aw[n] * nw[n]
        for kind, aff, aw, nw, dout in (
            ("g", inputs["gate_affw"][e], inputs["gate_aw"][e], inputs["gate_nw"][e], 128),
            ("u", inputs["upd_affw"][e], inputs["upd_aw"][e], inputs["upd_nw"][e], 64),
        ):
            scale = (np.asarray(aw, f32)[:, 0] * np.asarray(nw, f32)[0])  # [n]
            C = np.asarray(aff, f32).T * scale[None, :]  # [dout, n]
            r1 = _rank1(C)
            if r1 is None:
                H["flags"][f"cfull_{kind}{e}"] = True
                u = np.zeros(dout, f32)
                v = np.zeros(C.shape[1], f32)
                if kind == "g":
                    shared[f"cg{e}"] = np.ascontiguousarray(C)
                else:
                    shared[f"cu2_{e}"] = np.ascontiguousarray(
                        np.concatenate([C, C], axis=0))
            else:
                H["flags"][f"cfull_{kind}{e}"] = False
                u, v = r1
            if kind == "g":
                ug = u
                vg = v
            else:
                uu = u
                vu = v

        # affb fallback (AB tensors). gab/uab per-partition parts go in ACT bias.
        abg = np.asarray(inputs["gate_affb"][e], f32).T  # [128, n]
        abu = np.asarray(inputs["upd_affb"][e], f32).T   # [64, n]
        H["flags"][f"ab_g{e}"] = bool(np.any(abg))
        H["flags"][f"ab_u{e}"] = bool(np.any(abu))
        if H["flags"][f"ab_g{e}"]:
            shared[f"abg{e}"] = np.ascontiguousarray(abg)
        if H["flags"][f"ab_u{e}"]:
            shared[f"abu2_{e}"] = np.ascontiguousarray(
                np.concatenate([abu, abu], axis=0))

        # v rows, repeated per local batch elem
        shared[f"vg{e}"] = np.tile(vg[None, :], (1, BLOC)).astype(ml_dtypes.bfloat16)
        shared[f"vu{e}"] = np.tile(vu[None, :], (1, BLOC)).astype(ml_dtypes.bfloat16)

        # bias/scale vector columns
        H[f"gb{e}"] = gb
        H[f"bzr{e}"] = np.concatenate([gab[:64], -gab[64:]])
        H[f"ub2_{e}"] = np.concatenate([ub, ub])
        H[f"uab2_{e}"] = np.concatenate([uab, uab])
        H[f"ug{e}"] = ug
        H[f"uu2_{e}"] = np.concatenate([uu, uu])

    cw = np.asarray(inputs["conv_w"], f32)  # [3, 12, 64]
    cb = np.asarray(inputs["conv_b"], f32)  # [3, 12]
    # head order: [src1(12) | out1(12)] so src1 sits at partitions 0:12
    shared["cw01"] = np.concatenate([cw[1].T, cw[0].T], axis=1).astype(
        ml_dtypes.bfloat16)                                       # [64, 2T]
    shared["cw2"] = np.ascontiguousarray(cw[2].T).astype(ml_dtypes.bfloat16)
    shared["ident"] = np.eye(128, dtype=f32)

    # cvec columns
    ncol = 16
    cvec = np.zeros((128, ncol), f32)
    cvec[:64, 0] = 1.0
    cvec[64:, 0] = -1.0
    cols = {"szr": 0}
    ci = 1
    for e in range(2):
        for nm in ("gb", "bzr", "ub2_", "uab2_", "ug", "uu2_"):
            key = f"{nm}{e}"
            arr = H[key]
            cvec[: len(arr), ci] = arr
            cols[key] = ci
            ci += 1
    th = cb.shape[1]
    cvec[: 2 * th, ci] = np.concatenate([cb[1], cb[0]])
    cols["cb01"] = ci
    ci += 1
    cvec[:th, ci] = cb[2]
    cols["cb2"] = ci
    shared["cvec"] = cvec
    H["cols"] = cols
    H["shared"] = shared
    return H


def _build(H, n=N_FULL, t_steps=T, ch=1024, mmc=512):
    """Build the single-core Bass program (same for all cores)."""
    W = BLOC * n
    flags = H["flags"]
    cols = H["cols"]
    nc = bacc.Bacc("TRN2", target_bir_lowering=False, debug=False)

    dram = {}
    for name, arr in H["shared"].items():
        dram[name] = nc.declare_dram_parameter(
            name, list(arr.shape), mybir.dt.from_np(arr.dtype), isOutput=False)
    src32 = nc.declare_dram_parameter("src32", [t_steps, W], F32, isOutput=False)
    srcbf = nc.declare_dram_parameter("srcbf", [t_steps, W], BF16, isOutput=False)
    out_d = nc.declare_dram_parameter("out", [t_steps, W], F32, isOutput=True)
    o1d = nc.dram_tensor("o1d", [t_steps, W], F32)
    xsbf = nc.dram_tensor("xsbf", [t_steps, W], BF16)

    CH_B = _chunks(n, ch)       # chunks within one batch-half
    hch = min(ch, 1024)
    CH_W = _chunks(W, hch)      # chunks over full width (boundary passes)
    nchb = len(CH_B)

    with tile.TileContext(nc) as tc, ExitStack() as ctx:
        # ---- persistent tiles (one pool, distinct tags = one slot each) ----
        pers = ctx.enter_context(tc.tile_pool(name="pers", bufs=1))

        def ptile(shape, dtype, nm):
            return pers.tile(shape, dtype, name=nm, tag=nm)

        X = ptile([66, W], BF16, "X")
        X2 = ptile([66, W], BF16, "X2")
        ZQ = ptile([128, W], BF16, "ZQ")
        CVEC = ptile(list(H["shared"]["cvec"].shape), F32, "CVEC")
        IDENT = ptile([128, 128], F32, "IDENT")
        th = t_steps
        CW01 = ptile([64, 2 * th], BF16, "CW01")
        CW2 = ptile([64, th], BF16, "CW2")
        WGH = {}
        WGA = {}
        WUH = {}
        WUA = {}
        for e in range(2):
            WGH[e] = ptile([66, 128], BF16, f"twgh{e}")
            WUH[e] = ptile([66, 64], BF16, f"twuh{e}")
            WGA[e] = {}
            WUA[e] = {}
            for b in range(BLOC):
                WGA[e][b] = ptile([66, 128], BF16, f"twga{e}{b}")
                WUA[e][b] = ptile([66, 64], BF16, f"twua{e}{b}")

        # (Bacc.generate_event_semaphores legalizes wait counts at compile)
        nc.sync.dma_start(CVEC[:, :], dram["cvec"][:, :])
        nc.sync.dma_start(IDENT[:, :], dram["ident"][:, :])
        nc.sync.dma_start(CW01[:, :], dram["cw01"][:, :])
        nc.sync.dma_start(CW2[:, :], dram["cw2"][:, :])
        for e in range(2):
            nc.sync.dma_start(WGH[e][:, :], dram[f"wgh{e}"][:, :])
            nc.sync.dma_start(WUH[e][:, :], dram[f"wuh{e}"][:, :])
            for b in range(BLOC):
                nc.sync.dma_start(WGA[e][b][:, :], dram[f"wga{e}"][:, :])
                nc.sync.dma_start(WUA[e][b][:, :], dram[f"wua{e}"][:, :])

        def bias(key):
            return CVEC[:, cols[key]:cols[key] + 1]

        # ---- pools ----
        ps = ctx.enter_context(tc.tile_pool(name="ps", bufs=3, space="PSUM"))
        tps = ctx.enter_context(tc.tile_pool(name="tps", bufs=2, space="PSUM"))
        sb = ctx.enter_context(tc.tile_pool(name="sb", bufs=3))
        small = ctx.enter_context(tc.tile_pool(name="small", bufs=2))
        fpool = ctx.enter_context(tc.tile_pool(name="fpool", bufs=2))

        def mm_into(p, lhsT, rhs_tile, rhs_rows, coff, cw_, p_rows=None,
                    cast=None):
            """Matmul chunk [*, cw_] at col offset coff into psum tile p."""
            for m0, mw in _chunks(cw_, mmc):
                lhs_ap = lhsT
                rhs_ap = rhs_tile[rhs_rows, coff + m0:coff + m0 + mw]
                o = p[:, m0:m0 + mw] if p_rows is None else \
                    p[p_rows, m0:m0 + mw]
                if cast is not None:
                    lhs_ap = lhs_ap.bitcast(cast)
                    rhs_ap = rhs_ap.bitcast(cast)
                nc.tensor.matmul(o, lhs_ap, rhs_ap, start=True, stop=True)

        def gfs_gate(e):
            pooled_b = []
            for b in range(BLOC):
                boff = b * n
                parts = small.tile([128, nchb], F32, tag="parts")
                for ci, (c0, cw_) in enumerate(CH_B):
                    p = ps.tile([128, ch], F32, tag="ps")
                    mm_into(p, WGH[e][:, :], X, slice(0, 66), boff + c0, cw_)
                    if False:
                        # DVE relu+bias+accum for engine balance
                        nc.vector.tensor_scalar(p[:, :cw_], p[:, :cw_],
                                                bias(f"gb{e}"), 0.0,
                                                op0=OP.add, op1=OP.max,
                                                accum_out=parts[:, ci:ci + 1])
                    else:
                        nc.scalar.activation(p[:, :cw_], p[:, :cw_], AF.Relu,
                                             bias=bias(f"gb{e}"),
                                             accum_out=parts[:, ci:ci + 1])
                if not flags[f"cfull_g{e}"]:
                    # fused: scratch = parts*u, accum = sum = pooled*u
                    scr = small.tile([128, nchb], F32, tag="pscr")
                    pgs = small.tile([128, 1], F32, tag="pgs")
                    nc.vector.tensor_scalar(scr[:, :], parts[:, :],
                                            bias(f"ug{e}"), 0.0, op0=OP.mult,
                                            op1=OP.add, accum_out=pgs[:, :])
                    tp = tps.tile([1, 128], F32, tag="tp")
                    nc.tensor.transpose(tp[:, :], pgs[:, :], IDENT[:, :])
                    nc.vector.tensor_copy(WGA[e][b][64:65, 0:128],
                                          tp[0:1, 0:128])
                    pooled_b.append(None)
                else:
                    pooled = small.tile([128, 1], F32, tag="pooled")
                    nc.vector.tensor_reduce(pooled[:, :], parts[:, :],
                                            axis=AX.X, op=OP.add)
                    pooled_b.append(pooled)
            # pass 2: res(+aug) matmul -> sigmoid -> ZQ (z rows 0:64, q 64:128)
            for b in range(BLOC):
                boff = b * n
                for c0, cw_ in CH_B:
                    p = ps.tile([128, ch], F32, tag="ps")
                    mm_into(p, WGA[e][b][:, :], X, slice(0, 66), boff + c0,
                            cw_)
                    if flags[f"cfull_g{e}"]:
                        cgc = fpool.tile([128, ch], F32, tag="cgc")
                        nc.sync.dma_start(cgc[:, :cw_],
                                          dram[f"cg{e}"][:, c0:c0 + cw_])
                        nc.vector.scalar_tensor_tensor(
                            p[:, :cw_], cgc[:, :cw_], pooled_b[b][:, :],
                            p[:, :cw_], op0=OP.mult, op1=OP.add)
                    if flags[f"ab_g{e}"]:
                        abc = fpool.tile([128, ch], F32, tag="abc")
                        nc.sync.dma_start(abc[:, :cw_],
                                          dram[f"abg{e}"][:, c0:c0 + cw_])
                        nc.vector.tensor_add(p[:, :cw_], p[:, :cw_],
                                             abc[:, :cw_])
                    nc.scalar.activation(ZQ[:, boff + c0:boff + c0 + cw_],
                                         p[:, :cw_], AF.Sigmoid,
                                         bias=bias(f"bzr{e}"),
                                         scale=bias("szr"))
                    # zs = z * S -> X2 state rows (b0 on gpsimd for balance)
                    zeng = nc.gpsimd if b == 0 else nc.vector
                    zeng.tensor_mul(
                        X2[0:64, boff + c0:boff + c0 + cw_],
                        ZQ[0:64, boff + c0:boff + c0 + cw_],
                        X[0:64, boff + c0:boff + c0 + cw_])

        def gfs_upd(e):
            parts = small.tile([128, nchb], F32, tag="parts")
            for ci, (c0, cw_) in enumerate(CH_B):
                p = ps.tile([128, ch], F32, tag="ps")
                for b in range(BLOC):
                    mm_into(p, WUH[e][:, :], X2, slice(0, 66), b * n + c0,
                            cw_, p_rows=slice(b * 64, b * 64 + 64))
                if ci % 2 == 0:
                    nc.scalar.activation(p[:, :cw_], p[:, :cw_], AF.Relu,
                                         bias=bias(f"ub2_{e}"),
                                         accum_out=parts[:, ci:ci + 1])
                else:
                    # DVE version of relu+bias+accum to balance engines
                    nc.vector.tensor_scalar(p[:, :cw_], p[:, :cw_],
                                            bias(f"ub2_{e}"), 0.0,
                                            op0=OP.add, op1=OP.max,
                                            accum_out=parts[:, ci:ci + 1])
            pooled2 = None
            if not flags[f"cfull_u{e}"]:
                scr = small.tile([128, nchb], F32, tag="pscr")
                pgs = small.tile([128, 1], F32, tag="pgs")
                nc.vector.tensor_scalar(scr[:, :], parts[:, :],
                                        bias(f"uu2_{e}"), 0.0, op0=OP.mult,
                                        op1=OP.add, accum_out=pgs[:, :])
                tp = tps.tile([1, 128], F32, tag="tp")
                nc.tensor.transpose(tp[:, :], pgs[:, :], IDENT[:, :])
                nc.vector.tensor_copy(WUA[e][0][64:65, 0:64], tp[0:1, 0:64])
                nc.vector.tensor_copy(WUA[e][1][64:65, 0:64],
                                      tp[0:1, 64:128])
            else:
                pooled2 = small.tile([128, 1], F32, tag="pooled")
                nc.vector.tensor_reduce(pooled2[:, :], parts[:, :],
                                        axis=AX.X, op=OP.add)
            # full-width q realign: one SBUF->SBUF DMA per batch elem,
            # issued while U1 still runs (sigmoid is long done)
            qts = []
            for b in range(BLOC):
                qt = sb.tile([64, n], BF16, tag="qt", bufs=2)
                nc.gpsimd.dma_start(qt[:, :], ZQ[64:128, b * n:(b + 1) * n])
                qts.append(qt)
            hcf = sb.tile([128, n], BF16, tag="hcf", bufs=2)
            # pass 2: res2(+aug) -> tanh -> combine into state
            for c0, cw_ in CH_B:
                p = ps.tile([128, ch], F32, tag="ps")
                for b in range(BLOC):
                    mm_into(p, WUA[e][b][:, :], X2, slice(0, 66), b * n + c0,
                            cw_, p_rows=slice(b * 64, b * 64 + 64))
                if flags[f"cfull_u{e}"]:
                    cuc = fpool.tile([128, ch], F32, tag="cgc")
                    nc.sync.dma_start(cuc[:, :cw_],
                                      dram[f"cu2_{e}"][:, c0:c0 + cw_])
                    nc.vector.scalar_tensor_tensor(
                        p[:, :cw_], cuc[:, :cw_], pooled2[:, :], p[:, :cw_],
                        op0=OP.mult, op1=OP.add)
                if flags[f"ab_u{e}"]:
                    abc = fpool.tile([128, ch], F32, tag="abc")
                    nc.sync.dma_start(abc[:, :cw_],
                                      dram[f"abu2_{e}"][:, c0:c0 + cw_])
                    nc.vector.tensor_add(p[:, :cw_], p[:, :cw_], abc[:, :cw_])
                nc.scalar.activation(hcf[:, c0:c0 + cw_], p[:, :cw_], AF.Tanh,
                                     bias=bias(f"uab2_{e}"))
                # b0 combine chunk-rolls right behind tanh
                csl = slice(c0, c0 + cw_)
                nc.vector.tensor_sub(hcf[0:64, csl], hcf[0:64, csl],
                                     X[0:64, csl])
                nc.vector.tensor_mul(hcf[0:64, csl], qts[0][:, csl],
                                     hcf[0:64, csl])
                nc.vector.tensor_add(X[0:64, csl], X[0:64, csl],
                                     hcf[0:64, csl])
            # b1: chunked realign (SWDGE; Pool engine is otherwise idle)
            for c0, cw_ in CH_B:
                csl = slice(c0, c0 + cw_)
                xsl = slice(n + c0, n + c0 + cw_)
                hct = sb.tile([64, ch], BF16, tag="hct", bufs=2)
                nc.gpsimd.dma_start(hct[:, :cw_], hcf[64:128, csl])
                nc.vector.tensor_sub(hct[:, :cw_], hct[:, :cw_], X[0:64, xsl])
                nc.vector.tensor_mul(hct[:, :cw_], qts[1][:, csl],
                                     hct[:, :cw_])
                nc.vector.tensor_add(X[0:64, xsl], X[0:64, xsl],
                                     hct[:, :cw_])

        def absorb(src_tile, we):
            # dummy ldweights pre-consuming {engine, DMA} waits on the PE so
            # subsequent real matmuls stay within the 2-slot MM wait limit
            # (real matmuls are self-loading, so this clobbers nothing)
            nc.tensor.ldweights(src_tile[0:66, 0:1])

        def encoder(e, xbf_src):
            nc.vector.memset(X[0:64, :], 0.0)
            nc.sync.dma_start(X2[64:65, :], dram[f"vu{e}"][:, :])
            nc.sync.dma_start(X[64:65, :], dram[f"vg{e}"][:, :])
            absorb(X, WGH[e])
            absorb(X2, WUH[e])
            for t in range(t_steps):
                nc.sync.dma_start(X2[65:66, :], xbf_src[t:t + 1, :])
                nc.sync.dma_start(X[65:66, :], xbf_src[t:t + 1, :])
                absorb(X, WGH[e])
                gfs_gate(e)
                absorb(X2, WUH[e])
                gfs_upd(e)

        # ================= encoder 1 =================
        encoder(0, srcbf)

        # heads 1+2 and encoder-2 input build
        for c0, cw_ in CH_W:
            p = ps.tile([2 * th, hch], F32, tag="ps")
            mm_into(p, CW01[:, :], X, slice(0, 64), c0, cw_)
            oc = sb.tile([2 * th, hch], F32, tag="hc", bufs=2)
            nc.scalar.activation(oc[:, :cw_], p[: 2 * th, :cw_], AF.Identity,
                                 bias=CVEC[0: 2 * th, cols["cb01"]:cols["cb01"] + 1])
            # rows 0:12 = src1 (head 1), rows 12:24 = out1 (head 0)
            nc.sync.dma_start(o1d[:, c0:c0 + cw_], oc[th: 2 * th, :cw_])
            sc = small.tile([th, hch], F32, tag="srcc")
            nc.sync.dma_start(sc[:, :cw_], src32[:, c0:c0 + cw_])
            xbb = sb.tile([th, hch], BF16, tag="xbb", bufs=2)
            nc.vector.tensor_sub(xbb[:, :cw_], sc[:, :cw_], oc[0:th, :cw_])
            nc.sync.dma_start(xsbf[:, c0:c0 + cw_], xbb[:, :cw_])

        # ================= encoder 2 =================
        encoder(1, xsbf)

        # head 3 + final sum
        for c0, cw_ in CH_W:
            p = ps.tile([th, hch], F32, tag="ps")
            mm_into(p, CW2[:, :], X, slice(0, 64), c0, cw_)
            o2 = sb.tile([th, hch], F32, tag="hc", bufs=2)
            nc.scalar.activation(o2[:, :cw_], p[:th, :cw_], AF.Identity,
                                 bias=CVEC[0:th, cols["cb2"]:cols["cb2"] + 1])
            o1c = sb.tile([th, hch], F32, tag="d", bufs=2)
            nc.sync.dma_start(o1c[:, :cw_], o1d[:, c0:c0 + cw_])
            nc.vector.tensor_add(o2[:, :cw_], o2[:, :cw_], o1c[:, :cw_])
            nc.sync.dma_start(out_d[:, c0:c0 + cw_], o2[:, :cw_])

    nc.compile()
    return nc


def _make_in_maps(inputs, H, n=N_FULL, t_steps=T):
    src = np.asarray(inputs["source"], np.float32)[..., 0]  # (B, T, n)
    in_maps = []
    for c in range(NCORES):
        m = dict(H["shared"])
        # src32[t, b*n + i] = src[2c+b, t, i]
        blk = src[BLOC * c: BLOC * (c + 1)]          # (BLOC, T, n)
        s = np.ascontiguousarray(
            blk.transpose(1, 0, 2).reshape(t_steps, BLOC * n))
        m["src32"] = s
        m["srcbf"] = s.astype(ml_dtypes.bfloat16)
        in_maps.append(m)
    return in_maps


def _assemble(results, n=N_FULL, t_steps=T):
    full = np.zeros((B, t_steps, n, 1), np.float32)
    for c in range(NCORES):
        o = np.asarray(results[c]["out"])            # [T, BLOC*n]
        o = o.reshape(t_steps, BLOC, n).transpose(1, 0, 2)
        full[BLOC * c: BLOC * (c + 1), :, :, 0] = o
    return full


# ---------------------------------------------------------------------------
# Fast path: GFS pooled term provably negligible -> drop both pooled matmul
# passes. State is b-packed [128, n] (partitions 64b+d); x_t contributions
# enter via K=1/K=2 accumulating matmuls from a resident [24, n] source tile
# (rows 2t+b), eliminating the per-step single-partition x-row DMAs.
# ---------------------------------------------------------------------------

TAU_MAX = 1e-3  # bound on the dropped pooled term's pre-activation magnitude


def _prep_fast(inputs):
    """Return fast-path host dict, or None if the pooled term is not
    provably negligible / affb nonzero."""
    f32 = np.float32
    n, th = N_FULL, T
    src = np.asarray(inputs["source"], f32)
    cw = np.asarray(inputs["conv_w"], f32)
    cb = np.asarray(inputs["conv_b"], f32)
    if np.any(np.asarray(inputs["gate_affb"])) or np.any(
            np.asarray(inputs["upd_affb"])):
        return None
    xmax0 = float(np.abs(src).max())
    s1max = float(np.max(np.sum(np.abs(cw[1]), axis=1) + np.abs(cb[1])))
    xmax = [xmax0, xmax0 + s1max]
    for e in range(2):
        for w_, b_, aff, aw, nw in (
            (inputs["gate_w"][e], inputs["gate_b"][e], inputs["gate_affw"][e],
             inputs["gate_aw"][e], inputs["gate_nw"][e]),
            (inputs["upd_w"][e], inputs["upd_b"][e], inputs["upd_affw"][e],
             inputs["upd_aw"][e], inputs["upd_nw"][e]),
        ):
            w_ = np.asarray(w_, f32)
            b_ = np.asarray(b_, f32)
            cmax = float(np.abs(np.asarray(aff, f32).T
                                * (np.asarray(aw, f32)[:, 0]
                                   * np.asarray(nw, f32)[0])[None, :]).max())
            premax = np.abs(b_) + np.abs(w_[0]) * xmax[e] + \
                np.sum(np.abs(w_[1:]), axis=0)
            tau = cmax * n * float(np.maximum(premax, 0.0).max())
            if tau > TAU_MAX:
                return None

    bf = ml_dtypes.bfloat16
    H = {"shared": {}}
    sh = H["shared"]
    for e in range(2):
        gaW = np.asarray(inputs["gate_alignW"][e], f32)   # [65, 128]
        gab = np.asarray(inputs["gate_alignb"][e], f32)   # [128]
        uaW = np.asarray(inputs["upd_alignW"][e], f32)    # [65, 64]
        uab = np.asarray(inputs["upd_alignb"][e], f32)    # [64]
        # b1 rows use swapped z/q column blocks so that z lands on psum rows
        # 64:128 for b1 (partition-aligned with the b-packed state) and q' on
        # 0:64; this keeps every gating TensorTensor op same-start-partition.
        swap = np.concatenate([gaW[:, 64:128], gaW[:, 0:64]], axis=1)
        wgp = np.zeros((128, 128), f32)
        wgp[0:64] = gaW[1:65]
        wgp[64:128] = swap[1:65]
        sh[f"wgp{e}"] = wgp.astype(bf)
        wxga = np.zeros((98, 128), f32)
        wxgb = np.zeros((98, 128), f32)
        for q in (0, 32, 64, 96):
            wxga[q] = gaW[0]          # K=1 row for gate b0
            wxgb[q + 1] = swap[0]     # K=2 rows [0; wx_swapped] for gate b1
        sh[f"wxga{e}"] = wxga.astype(bf)
        sh[f"wxgb{e}"] = wxgb.astype(bf)
        wubd = np.zeros((128, 128), f32)
        wubd[0:64, 0:64] = uaW[1:65]
        wubd[64:128, 64:128] = uaW[1:65]
        sh[f"wubd{e}"] = wubd.astype(bf)
        wxu = np.zeros((98, 128), f32)
        for q in (0, 32, 64, 96):
            wxu[q, 0:64] = uaW[0]
            wxu[q + 1, 64:128] = uaW[0]
        sh[f"wxu{e}"] = wxu.astype(bf)
        H[f"bzr{e}"] = np.concatenate([gab[:64], -gab[64:]])
        H[f"bzrS{e}"] = np.concatenate([-gab[64:], gab[:64]])
        H[f"uab2_{e}"] = np.concatenate([uab, uab])

    cwh1a = np.zeros((128, 24), f32)
    cwh1b = np.zeros((128, 24), f32)
    cwh2 = np.zeros((128, 24), f32)
    cb1p = np.zeros(24, f32)
    cb0p = np.zeros(24, f32)
    cb2p = np.zeros(24, f32)
    for b in range(2):
        for o in range(th):
            cwh1a[64 * b:64 * b + 64, 2 * o + b] = cw[1][o]
            cwh1b[64 * b:64 * b + 64, 2 * o + b] = cw[0][o]
            cwh2[64 * b:64 * b + 64, 2 * o + b] = cw[2][o]
            cb1p[2 * o + b] = cb[1][o]
            cb0p[2 * o + b] = cb[0][o]
            cb2p[2 * o + b] = cb[2][o]
    sh["cwh1a"] = cwh1a.astype(bf)
    sh["cwh1b"] = cwh1b.astype(bf)
    sh["cwh2"] = cwh2.astype(bf)

    cvec = np.zeros((128, 11), f32)
    cvec[0:64, 0] = 1.0        # szr (b0): z rows +, q rows -
    cvec[64:128, 0] = -1.0
    cvec[0:64, 1] = -1.0       # szrS (b1 swapped): q rows -, z rows +
    cvec[64:128, 1] = 1.0
    cvec[:, 2] = H["bzr0"]
    cvec[:, 3] = H["bzrS0"]
    cvec[:, 4] = H["bzr1"]
    cvec[:, 5] = H["bzrS1"]
    cvec[:, 6] = H["uab2_0"]
    cvec[:, 7] = H["uab2_1"]
    cvec[0:24, 8] = cb1p
    cvec[0:24, 9] = cb2p
    cvec[0:24, 10] = cb0p
    sh["cvec"] = cvec
    return H


def _build_fast(H, n=N_FULL, t_steps=T, ch=2048, mmc=512):
    nc = bacc.Bacc("TRN2", target_bir_lowering=False, debug=False)
    dram = {}
    for name, arr in H["shared"].items():
        dram[name] = nc.declare_dram_parameter(
            name, list(arr.shape), mybir.dt.from_np(arr.dtype), isOutput=False)
    srcp_d = nc.declare_dram_parameter("srcp", [2 * t_steps, n], BF16,
                                       isOutput=False)
    src32_d = nc.declare_dram_parameter("src32p", [2 * t_steps, n], F32,
                                        isOutput=False)
    out_d = nc.declare_dram_parameter("out", [2 * t_steps, n], F32,
                                      isOutput=True)
    xs2_d = nc.dram_tensor("xs2", [2 * t_steps, n], BF16)
    CH = list(reversed(_chunks(n, ch)))

    with tile.TileContext(nc) as tc, ExitStack() as ctx:
        pers = ctx.enter_context(tc.tile_pool(name="pers", bufs=1))

        def ptile(shape, dtype, nm):
            return pers.tile(shape, dtype, name=nm, tag=nm)

        SP = ptile([128, n], BF16, "SP")
        X2P = ptile([128, n], BF16, "X2P")
        QP = ptile([128, n], BF16, "QP")
        HC = ptile([128, n], BF16, "HC")
        ZQ = ptile([128, 2 * n], BF16, "ZQ")
        XQ = ptile([98, n], BF16, "XQ")
        O1B = ptile([24, n], F32, "O1B")
        CVEC = ptile(list(H["shared"]["cvec"].shape), F32, "CVEC")
        CWH1A = ptile([128, 24], BF16, "CWH1A")
        CWH1B = ptile([128, 24], BF16, "CWH1B")
        CWH2 = ptile([128, 24], BF16, "CWH2")
        WGP, WXGA, WXGB, WUBD, WXU = {}, {}, {}, {}, {}
        for e in range(2):
            WGP[e] = ptile([128, 128], BF16, f"wgp{e}")
            WXGA[e] = ptile([98, 128], BF16, f"wxga{e}")
            WXGB[e] = ptile([98, 128], BF16, f"wxgb{e}")
            WUBD[e] = ptile([128, 128], BF16, f"wubd{e}")
            WXU[e] = ptile([98, 128], BF16, f"wxu{e}")

        nc.sync.dma_start(CVEC[:, :], dram["cvec"][:, :])
        nc.sync.dma_start(CWH1A[:, :], dram["cwh1a"][:, :])
        nc.sync.dma_start(CWH1B[:, :], dram["cwh1b"][:, :])
        nc.sync.dma_start(CWH2[:, :], dram["cwh2"][:, :])
        for e in range(2):
            nc.sync.dma_start(WGP[e][:, :], dram[f"wgp{e}"][:, :])
            nc.sync.dma_start(WXGA[e][:, :], dram[f"wxga{e}"][:, :])
            nc.sync.dma_start(WXGB[e][:, :], dram[f"wxgb{e}"][:, :])
            nc.sync.dma_start(WUBD[e][:, :], dram[f"wubd{e}"][:, :])
            nc.sync.dma_start(WXU[e][:, :], dram[f"wxu{e}"][:, :])

        ps = ctx.enter_context(tc.tile_pool(name="ps", bufs=2, space="PSUM"))
        sb = ctx.enter_context(tc.tile_pool(name="sb", bufs=2))

        def bias(i, p=128):
            return CVEC[0:p, i:i + 1]

        def xq_load(src_ap, t):
            q = 32 * (t % 4)
            nc.sync.dma_start(XQ[q:q + 2, :], src_ap[2 * t:2 * t + 2, :])

        def cell(e, t, src_ap):
            q = 32 * (t % 4)
            first = t == 0
            # ---- gate: ZQ b-width; b0 cols rows 0:64=z, 64:128=q';
            #      b1 cols (swapped weights) rows 0:64=q', 64:128=z ----
            for b in range(2):
                boff = b * n
                bp = slice(64 * b, 64 * b + 64)
                for c0, cw_ in CH:
                    p = ps.tile([128, ch], F32, tag="ps")
                    for m0, mw in _chunks(cw_, mmc):
                        msl = slice(c0 + m0, c0 + m0 + mw)
                        if not first:
                            nc.tensor.matmul(p[:, m0:m0 + mw], WGP[e][bp, :],
                                             SP[bp, msl], start=True,
                                             stop=False)
                        if b == 0:
                            nc.tensor.matmul(p[:, m0:m0 + mw],
                                             WXGA[e][q:q + 1, :],
                                             XQ[q:q + 1, msl],
                                             start=first, stop=True,
                                             tile_position=(q, 0))
                        else:
                            nc.tensor.matmul(p[:, m0:m0 + mw],
                                             WXGB[e][q:q + 2, :],
                                             XQ[q:q + 2, msl],
                                             start=first, stop=True,
                                             tile_position=(q, 0))
                    osl = slice(boff + c0, boff + c0 + cw_)
                    nc.scalar.activation(ZQ[:, osl], p[:, :cw_], AF.Sigmoid,
                                         bias=bias(2 + 2 * e + b),
                                         scale=bias(b))
                    csl = slice(c0, c0 + cw_)
                    if not first:
                        # zs: z rows co-located with the b-packed state half
                        zeng = nc.vector if b == 0 else nc.gpsimd
                        zeng.tensor_mul(X2P[bp, csl], ZQ[bp, osl],
                                        SP[bp, csl])
                    # realign q' into b-packed QP (cross-partition copy)
                    qp = slice(64 - 64 * b, 128 - 64 * b)
                    nc.vector.tensor_copy(QP[bp, csl], ZQ[qp, osl])
            # ---- upd: hc = tanh(pre + uab2), b-packed; combine ----
            for c0, cw_ in CH:
                p = ps.tile([128, ch], F32, tag="ps")
                for m0, mw in _chunks(cw_, mmc):
                    msl = slice(c0 + m0, c0 + m0 + mw)
                    if not first:
                        nc.tensor.matmul(p[:, m0:m0 + mw], WUBD[e][:, :],
                                         X2P[:, msl], start=True, stop=False)
                    nc.tensor.matmul(p[:, m0:m0 + mw], WXU[e][q:q + 2, :],
                                     XQ[q:q + 2, msl],
                                     start=first, stop=True,
                                     tile_position=(q, 0))
                csl = slice(c0, c0 + cw_)
                nc.scalar.activation(HC[:, csl], p[:, :cw_], AF.Tanh,
                                     bias=bias(6 + e))
                if first:
                    # state was zero: s1 = q' * hc
                    nc.gpsimd.tensor_mul(SP[:, csl], QP[:, csl], HC[:, csl])
                else:
                    nc.vector.tensor_sub(HC[:, csl], HC[:, csl], SP[:, csl])
                    nc.gpsimd.tensor_mul(HC[:, csl], QP[:, csl], HC[:, csl])
                    nc.vector.tensor_add(SP[:, csl], SP[:, csl], HC[:, csl])
            # prefetch x for step t+4 into the quadrant this step just freed
            if t + 4 < t_steps:
                xq_load(src_ap, t + 4)

        # ================= encoder 1 =================
        for t in range(min(4, t_steps)):
            xq_load(srcp_d, t)
        for t in range(t_steps):
            cell(0, t, srcp_d)

        # heads 1+2 and encoder-2 input build
        hch = 1024
        for c0, cw_ in _chunks(n, hch):
            p1 = ps.tile([24, hch], F32, tag="ps")
            p2 = ps.tile([24, hch], F32, tag="ps")
            for m0, mw in _chunks(cw_, mmc):
                msl = slice(c0 + m0, c0 + m0 + mw)
                nc.tensor.matmul(p1[:, m0:m0 + mw], CWH1A[:, :], SP[:, msl],
                                 start=True, stop=True)
                nc.tensor.matmul(p2[:, m0:m0 + mw], CWH1B[:, :], SP[:, msl],
                                 start=True, stop=True)
            csl = slice(c0, c0 + cw_)
            o1a = sb.tile([24, hch], F32, tag="o1a")
            nc.scalar.activation(o1a[:, :cw_], p1[:24, :cw_], AF.Identity,
                                 bias=bias(8, 24))
            nc.scalar.activation(O1B[:, csl], p2[:24, :cw_], AF.Identity,
                                 bias=bias(10, 24))
            sc = sb.tile([24, hch], F32, tag="srcc")
            nc.sync.dma_start(sc[:, :cw_], src32_d[:, csl])
            x2c = sb.tile([24, hch], BF16, tag="x2c")
            nc.vector.tensor_sub(x2c[:, :cw_], sc[:, :cw_], o1a[:, :cw_])
            nc.sync.dma_start(xs2_d[:, csl], x2c[:, :cw_])

        # ================= encoder 2 =================
        for t in range(min(4, t_steps)):
            xq_load(xs2_d, t)
        for t in range(t_steps):
            cell(1, t, xs2_d)

        # head 3 + final sum
        hch = 1024
        for c0, cw_ in _chunks(n, hch):
            p = ps.tile([24, hch], F32, tag="ps")
            for m0, mw in _chunks(cw_, mmc):
                msl = slice(c0 + m0, c0 + m0 + mw)
                nc.tensor.matmul(p[0:24, m0:m0 + mw], CWH2[:, :], SP[:, msl],
                                 start=True, stop=True)
            csl = slice(c0, c0 + cw_)
            o2 = sb.tile([24, hch], F32, tag="o2")
            nc.scalar.activation(o2[:, :cw_], p[0:24, :cw_], AF.Identity,
                                 bias=bias(9, 24))
            nc.vector.tensor_add(o2[:, :cw_], o2[:, :cw_], O1B[:, csl])
            nc.sync.dma_start(out_d[:, csl], o2[:, :cw_])

    nc.compile()
    return nc


def _make_in_maps_fast(inputs, H, n=N_FULL, t_steps=T):
    src = np.asarray(inputs["source"], np.float32)[..., 0]  # (B, T, n)
    in_maps = []
    for c in range(NCORES):
        m = dict(H["shared"])
        s = np.zeros((2 * t_steps, n), np.float32)
        for b in range(BLOC):
            s[b::2] = src[BLOC * c + b]  # row 2t+b = src[2c+b, t]
        m["src32p"] = s
        m["srcp"] = s.astype(ml_dtypes.bfloat16)
        in_maps.append(m)
    return in_maps


def _assemble_fast(results, n=N_FULL, t_steps=T):
    full = np.zeros((B, t_steps, n, 1), np.float32)
    for c in range(NCORES):
        o = np.asarray(results[c]["out"])            # [2T, n], row 2o+b
        for b in range(BLOC):
            full[BLOC * c + b, :, :, 0] = o[b::2]
    return full


_PROG_CACHE = {}


def _prepare(inputs):
    """Pick fast/exact path; return (nc, in_maps, assemble_fn)."""
    HF = _prep_fast(inputs)
    if HF is not None:
        if "fast" not in _PROG_CACHE:
            _PROG_CACHE["fast"] = _build_fast(HF)
        return _PROG_CACHE["fast"], _make_in_maps_fast(inputs, HF), \
            _assemble_fast
    H = _prep_host(inputs)
    key = tuple(sorted(H["flags"].items()))
    if key not in _PROG_CACHE:
        _PROG_CACHE[key] = _build(H)
    return _PROG_CACHE[key], _make_in_maps(inputs, H), _assemble


def kernel(**inputs) -> np.ndarray:
    nc, in_maps, assemble = _prepare(inputs)
    res = run_bass_kernel_spmd(nc, in_maps, core_ids=list(range(NCORES)))
    return assemble(res.results)



# revision 16
# speedup vs baseline: 2.3109x; 1.0629x over previous
"""Trainium2 Bass kernel for nn_DDGCRN (gnn_message_passing).

DDGCRN: two 12-step GRU-style encoders over B=16, N=8600 nodes, HID=64,
with a global node-pooling term (GFS) inside each gate, plus conv heads.

Sharding: data-parallel over batch. B=16 / 8 cores = 2 batch elems per
core; the GFS node-pooling sum is per-batch-element, so no collectives.

Per-core layout: feature-major. All wide tensors are [D, W] where
W = 2*8600 = 17200 columns (col = b*N + n). State tile X is [66, W]:
rows 0:64 = hidden state, row 64 = x_t, row 65 = v (rank-1 pooling row).
Weight rows are host-permuted to match ([W_state; w_x; aug]).

The GFS pooled term affw*(pooled*scale) is rank-1 in (d, n) for the
given inputs (affw==1): C[d,n] = u[d]*v[n]. It is folded into the
"res" matmul by augmenting K: lhsT row 65 = pooled*u (written per cell
via PE-transpose), rhs row 65 = v (constant). Non-rank-1 / nonzero-affb
inputs fall back to explicit DVE ops with C/AB streamed from DRAM.
"""

import numpy as np
import ml_dtypes
from contextlib import ExitStack

import concourse.bass as bass
import concourse.bacc as bacc
import concourse.tile as tile
from concourse import mybir
from concourse.bass_utils import run_bass_kernel_spmd

F32 = mybir.dt.float32
F32R = mybir.dt.float32r
BF16 = mybir.dt.bfloat16
AX = mybir.AxisListType
OP = mybir.AluOpType
AF = mybir.ActivationFunctionType

# Problem constants (hardcoded; kernel.py must be self-contained)
B, T, N_FULL, HID, IN = 16, 12, 8600, 64, 1
GIN = IN + HID
NCORES = 8
BLOC = B // NCORES  # 2


def _chunks(total, size):
    out = []
    off = 0
    while off < total:
        w = min(size, total - off)
        out.append((off, w))
        off += w
    return out


def _rank1(C):
    """C [D, M] -> (u [D], v [M]) with C == outer(u, v), or None."""
    d0, m0 = np.unravel_index(np.argmax(np.abs(C)), C.shape)
    piv = C[d0, m0]
    if abs(piv) < 1e-30:
        return np.zeros(C.shape[0], np.float32), np.zeros(C.shape[1], np.float32)
    u = C[:, m0].astype(np.float64)
    v = C[d0, :].astype(np.float64) / piv
    if not np.allclose(np.outer(u, v), C, rtol=1e-5, atol=1e-7 * abs(piv)):
        return None
    return u.astype(np.float32), v.astype(np.float32)


def _prep_host(inputs, n=N_FULL, t_steps=T):
    """Host-side parameter prep. Returns dict of per-core-shared arrays
    plus flags. All weight matrices get the row permutation
    [rows 1:65 (state); row 0 (x); (aug row 0)]."""
    f32 = np.float32
    H = {"flags": {}}
    shared = {}

    def perm66(w):
        """[65, Dout] -> [66, Dout]: rows 0:64 = state weights, row 64 = 0
        (v-row slot / aug slot), row 65 = x weight."""
        z = np.zeros((1, w.shape[1]), f32)
        return np.concatenate([w[1:], z, w[0:1]], axis=0).astype(f32)

    for e in range(2):
        gaW = np.asarray(inputs["gate_alignW"][e], f32)   # [65, 128]
        gw = np.asarray(inputs["gate_w"][e], f32)         # [65, 128]
        gab = np.asarray(inputs["gate_alignb"][e], f32)   # [128]
        gb = np.asarray(inputs["gate_b"][e], f32)         # [128]
        uaW = np.asarray(inputs["upd_alignW"][e], f32)    # [65, 64]
        uw = np.asarray(inputs["upd_w"][e], f32)          # [65, 64]
        uab = np.asarray(inputs["upd_alignb"][e], f32)    # [64]
        ub = np.asarray(inputs["upd_b"][e], f32)          # [64]

        shared[f"wgh{e}"] = perm66(gw).astype(ml_dtypes.bfloat16)
        shared[f"wga{e}"] = perm66(gaW).astype(ml_dtypes.bfloat16)
        shared[f"wuh{e}"] = perm66(uw).astype(ml_dtypes.bfloat16)
        shared[f"wua{e}"] = perm66(uaW).astype(ml_dtypes.bfloat16)

        # rank-1 pooling factors: C[d, n] = affw[n, d] * aw[n] * nw[n]
        for kind, aff, aw, nw, dout in (
            ("g", inputs["gate_affw"][e], inputs["gate_aw"][e], inputs["gate_nw"][e], 128),
            ("u", inputs["upd_affw"][e], inputs["upd_aw"][e], inputs["upd_nw"][e], 64),
        ):
            scale = (np.asarray(aw, f32)[:, 0] * np.asarray(nw, f32)[0])  # [n]
            C = np.asarray(aff, f32).T * scale[None, :]  # [dout, n]
            r1 = _rank1(C)
            if r1 is None:
                H["flags"][f"cfull_{kind}{e}"] = True
                u = np.zeros(dout, f32)
                v = np.zeros(C.shape[1], f32)
                if kind == "g":
                    shared[f"cg{e}"] = np.ascontiguousarray(C)
                else:
                    shared[f"cu2_{e}"] = np.ascontiguousarray(
                        np.concatenate([C, C], axis=0))
            else:
                H["flags"][f"cfull_{kind}{e}"] = False
                u, v = r1
            if kind == "g":
                ug = u
                vg = v
            else:
                uu = u
                vu = v

        # affb fallback (AB tensors). gab/uab per-partition parts go in ACT bias.
        abg = np.asarray(inputs["gate_affb"][e], f32).T  # [128, n]
        abu = np.asarray(inputs["upd_affb"][e], f32).T   # [64, n]
        H["flags"][f"ab_g{e}"] = bool(np.any(abg))
        H["flags"][f"ab_u{e}"] = bool(np.any(abu))
        if H["flags"][f"ab_g{e}"]:
            shared[f"abg{e}"] = np.ascontiguousarray(abg)
        if H["flags"][f"ab_u{e}"]:
            shared[f"abu2_{e}"] = np.ascontiguousarray(
                np.concatenate([abu, abu], axis=0))

        # v rows, repeated per local batch elem
        shared[f"vg{e}"] = np.tile(vg[None, :], (1, BLOC)).astype(ml_dtypes.bfloat16)
        shared[f"vu{e}"] = np.tile(vu[None, :], (1, BLOC)).astype(ml_dtypes.bfloat16)

        # bias/scale vector columns
        H[f"gb{e}"] = gb
        H[f"bzr{e}"] = np.concatenate([gab[:64], -gab[64:]])
        H[f"ub2_{e}"] = np.concatenate([ub, ub])
        H[f"uab2_{e}"] = np.concatenate([uab, uab])
        H[f"ug{e}"] = ug
        H[f"uu2_{e}"] = np.concatenate([uu, uu])

    cw = np.asarray(inputs["conv_w"], f32)  # [3, 12, 64]
    cb = np.asarray(inputs["conv_b"], f32)  # [3, 12]
    # head order: [src1(12) | out1(12)] so src1 sits at partitions 0:12
    shared["cw01"] = np.concatenate([cw[1].T, cw[0].T], axis=1).astype(
        ml_dtypes.bfloat16)                                       # [64, 2T]
    shared["cw2"] = np.ascontiguousarray(cw[2].T).astype(ml_dtypes.bfloat16)
    shared["ident"] = np.eye(128, dtype=f32)

    # cvec columns
    ncol = 16
    cvec = np.zeros((128, ncol), f32)
    cvec[:64, 0] = 1.0
    cvec[64:, 0] = -1.0
    cols = {"szr": 0}
    ci = 1
    for e in range(2):
        for nm in ("gb", "bzr", "ub2_", "uab2_", "ug", "uu2_"):
            key = f"{nm}{e}"
            arr = H[key]
            cvec[: len(arr), ci] = arr
            cols[key] = ci
            ci += 1
    th = cb.shape[1]
    cvec[: 2 * th, ci] = np.concatenate([cb[1], cb[0]])
    cols["cb01"] = ci
    ci += 1
    cvec[:th, ci] = cb[2]
    cols["cb2"] = ci
    shared["cvec"] = cvec
    H["cols"] = cols
    H["shared"] = shared
    return H


def _build(H, n=N_FULL, t_steps=T, ch=1024, mmc=512):
    """Build the single-core Bass program (same for all cores)."""
    W = BLOC * n
    flags = H["flags"]
    cols = H["cols"]
    nc = bacc.Bacc("TRN2", target_bir_lowering=False, debug=False)

    dram = {}
    for name, arr in H["shared"].items():
        dram[name] = nc.declare_dram_parameter(
            name, list(arr.shape), mybir.dt.from_np(arr.dtype), isOutput=False)
    src32 = nc.declare_dram_parameter("src32", [t_steps, W], F32, isOutput=False)
    srcbf = nc.declare_dram_parameter("srcbf", [t_steps, W], BF16, isOutput=False)
    out_d = nc.declare_dram_parameter("out", [t_steps, W], F32, isOutput=True)
    o1d = nc.dram_tensor("o1d", [t_steps, W], F32)
    xsbf = nc.dram_tensor("xsbf", [t_steps, W], BF16)

    CH_B = _chunks(n, ch)       # chunks within one batch-half
    hch = min(ch, 1024)
    CH_W = _chunks(W, hch)      # chunks over full width (boundary passes)
    nchb = len(CH_B)

    with tile.TileContext(nc) as tc, ExitStack() as ctx:
        # ---- persistent tiles (one pool, distinct tags = one slot each) ----
        pers = ctx.enter_context(tc.tile_pool(name="pers", bufs=1))

        def ptile(shape, dtype, nm):
            return pers.tile(shape, dtype, name=nm, tag=nm)

        X = ptile([66, W], BF16, "X")
        X2 = ptile([66, W], BF16, "X2")
        ZQ = ptile([128, W], BF16, "ZQ")
        CVEC = ptile(list(H["shared"]["cvec"].shape), F32, "CVEC")
        IDENT = ptile([128, 128], F32, "IDENT")
        th = t_steps
        CW01 = ptile([64, 2 * th], BF16, "CW01")
        CW2 = ptile([64, th], BF16, "CW2")
        WGH = {}
        WGA = {}
        WUH = {}
        WUA = {}
        for e in range(2):
            WGH[e] = ptile([66, 128], BF16, f"twgh{e}")
            WUH[e] = ptile([66, 64], BF16, f"twuh{e}")
            WGA[e] = {}
            WUA[e] = {}
            for b in range(BLOC):
                WGA[e][b] = ptile([66, 128], BF16, f"twga{e}{b}")
                WUA[e][b] = ptile([66, 64], BF16, f"twua{e}{b}")

        # (Bacc.generate_event_semaphores legalizes wait counts at compile)
        nc.sync.dma_start(CVEC[:, :], dram["cvec"][:, :])
        nc.sync.dma_start(IDENT[:, :], dram["ident"][:, :])
        nc.sync.dma_start(CW01[:, :], dram["cw01"][:, :])
        nc.sync.dma_start(CW2[:, :], dram["cw2"][:, :])
        for e in range(2):
            nc.sync.dma_start(WGH[e][:, :], dram[f"wgh{e}"][:, :])
            nc.sync.dma_start(WUH[e][:, :], dram[f"wuh{e}"][:, :])
            for b in range(BLOC):
                nc.sync.dma_start(WGA[e][b][:, :], dram[f"wga{e}"][:, :])
                nc.sync.dma_start(WUA[e][b][:, :], dram[f"wua{e}"][:, :])

        def bias(key):
            return CVEC[:, cols[key]:cols[key] + 1]

        # ---- pools ----
        ps = ctx.enter_context(tc.tile_pool(name="ps", bufs=3, space="PSUM"))
        tps = ctx.enter_context(tc.tile_pool(name="tps", bufs=2, space="PSUM"))
        sb = ctx.enter_context(tc.tile_pool(name="sb", bufs=3))
        small = ctx.enter_context(tc.tile_pool(name="small", bufs=2))
        fpool = ctx.enter_context(tc.tile_pool(name="fpool", bufs=2))

        def mm_into(p, lhsT, rhs_tile, rhs_rows, coff, cw_, p_rows=None,
                    cast=None):
            """Matmul chunk [*, cw_] at col offset coff into psum tile p."""
            for m0, mw in _chunks(cw_, mmc):
                lhs_ap = lhsT
                rhs_ap = rhs_tile[rhs_rows, coff + m0:coff + m0 + mw]
                o = p[:, m0:m0 + mw] if p_rows is None else \
                    p[p_rows, m0:m0 + mw]
                if cast is not None:
                    lhs_ap = lhs_ap.bitcast(cast)
                    rhs_ap = rhs_ap.bitcast(cast)
                nc.tensor.matmul(o, lhs_ap, rhs_ap, start=True, stop=True)

        def gfs_gate(e):
            pooled_b = []
            for b in range(BLOC):
                boff = b * n
                parts = small.tile([128, nchb], F32, tag="parts")
                for ci, (c0, cw_) in enumerate(CH_B):
                    p = ps.tile([128, ch], F32, tag="ps")
                    mm_into(p, WGH[e][:, :], X, slice(0, 66), boff + c0, cw_)
                    if False:
                        # DVE relu+bias+accum for engine balance
                        nc.vector.tensor_scalar(p[:, :cw_], p[:, :cw_],
                                                bias(f"gb{e}"), 0.0,
                                                op0=OP.add, op1=OP.max,
                                                accum_out=parts[:, ci:ci + 1])
                    else:
                        nc.scalar.activation(p[:, :cw_], p[:, :cw_], AF.Relu,
                                             bias=bias(f"gb{e}"),
                                             accum_out=parts[:, ci:ci + 1])
                if not flags[f"cfull_g{e}"]:
                    # fused: scratch = parts*u, accum = sum = pooled*u
                    scr = small.tile([128, nchb], F32, tag="pscr")
                    pgs = small.tile([128, 1], F32, tag="pgs")
                    nc.vector.tensor_scalar(scr[:, :], parts[:, :],
                                            bias(f"ug{e}"), 0.0, op0=OP.mult,
                                            op1=OP.add, accum_out=pgs[:, :])
                    tp = tps.tile([1, 128], F32, tag="tp")
                    nc.tensor.transpose(tp[:, :], pgs[:, :], IDENT[:, :])
                    nc.vector.tensor_copy(WGA[e][b][64:65, 0:128],
                                          tp[0:1, 0:128])
                    pooled_b.append(None)
                else:
                    pooled = small.tile([128, 1], F32, tag="pooled")
                    nc.vector.tensor_reduce(pooled[:, :], parts[:, :],
                                            axis=AX.X, op=OP.add)
                    pooled_b.append(pooled)
            # pass 2: res(+aug) matmul -> sigmoid -> ZQ (z rows 0:64, q 64:128)
            for b in range(BLOC):
                boff = b * n
                for c0, cw_ in CH_B:
                    p = ps.tile([128, ch], F32, tag="ps")
                    mm_into(p, WGA[e][b][:, :], X, slice(0, 66), boff + c0,
                            cw_)
                    if flags[f"cfull_g{e}"]:
                        cgc = fpool.tile([128, ch], F32, tag="cgc")
                        nc.sync.dma_start(cgc[:, :cw_],
                                          dram[f"cg{e}"][:, c0:c0 + cw_])
                        nc.vector.scalar_tensor_tensor(
                            p[:, :cw_], cgc[:, :cw_], pooled_b[b][:, :],
                            p[:, :cw_], op0=OP.mult, op1=OP.add)
                    if flags[f"ab_g{e}"]:
                        abc = fpool.tile([128, ch], F32, tag="abc")
                        nc.sync.dma_start(abc[:, :cw_],
                                          dram[f"abg{e}"][:, c0:c0 + cw_])
                        nc.vector.tensor_add(p[:, :cw_], p[:, :cw_],
                                             abc[:, :cw_])
                    nc.scalar.activation(ZQ[:, boff + c0:boff + c0 + cw_],
                                         p[:, :cw_], AF.Sigmoid,
                                         bias=bias(f"bzr{e}"),
                                         scale=bias("szr"))
                    # zs = z * S -> X2 state rows (b0 on gpsimd for balance)
                    zeng = nc.gpsimd if b == 0 else nc.vector
                    zeng.tensor_mul(
                        X2[0:64, boff + c0:boff + c0 + cw_],
                        ZQ[0:64, boff + c0:boff + c0 + cw_],
                        X[0:64, boff + c0:boff + c0 + cw_])

        def gfs_upd(e):
            parts = small.tile([128, nchb], F32, tag="parts")
            for ci, (c0, cw_) in enumerate(CH_B):
                p = ps.tile([128, ch], F32, tag="ps")
                for b in range(BLOC):
                    mm_into(p, WUH[e][:, :], X2, slice(0, 66), b * n + c0,
                            cw_, p_rows=slice(b * 64, b * 64 + 64))
                if ci % 2 == 0:
                    nc.scalar.activation(p[:, :cw_], p[:, :cw_], AF.Relu,
                                         bias=bias(f"ub2_{e}"),
                                         accum_out=parts[:, ci:ci + 1])
                else:
                    # DVE version of relu+bias+accum to balance engines
                    nc.vector.tensor_scalar(p[:, :cw_], p[:, :cw_],
                                            bias(f"ub2_{e}"), 0.0,
                                            op0=OP.add, op1=OP.max,
                                            accum_out=parts[:, ci:ci + 1])
            pooled2 = None
            if not flags[f"cfull_u{e}"]:
                scr = small.tile([128, nchb], F32, tag="pscr")
                pgs = small.tile([128, 1], F32, tag="pgs")
                nc.vector.tensor_scalar(scr[:, :], parts[:, :],
                                        bias(f"uu2_{e}"), 0.0, op0=OP.mult,
                                        op1=OP.add, accum_out=pgs[:, :])
                tp = tps.tile([1, 128], F32, tag="tp")
                nc.tensor.transpose(tp[:, :], pgs[:, :], IDENT[:, :])
                nc.vector.tensor_copy(WUA[e][0][64:65, 0:64], tp[0:1, 0:64])
                nc.vector.tensor_copy(WUA[e][1][64:65, 0:64],
                                      tp[0:1, 64:128])
            else:
                pooled2 = small.tile([128, 1], F32, tag="pooled")
                nc.vector.tensor_reduce(pooled2[:, :], parts[:, :],
                                        axis=AX.X, op=OP.add)
            # full-width q realign: one SBUF->SBUF DMA per batch elem,
            # issued while U1 still runs (sigmoid is long done)
            qts = []
            for b in range(BLOC):
                qt = sb.tile([64, n], BF16, tag="qt", bufs=2)
                nc.gpsimd.dma_start(qt[:, :], ZQ[64:128, b * n:(b + 1) * n])
                qts.append(qt)
            hcf = sb.tile([128, n], BF16, tag="hcf", bufs=2)
            # pass 2: res2(+aug) -> tanh -> combine into state
            for c0, cw_ in CH_B:
                p = ps.tile([128, ch], F32, tag="ps")
                for b in range(BLOC):
                    mm_into(p, WUA[e][b][:, :], X2, slice(0, 66), b * n + c0,
                            cw_, p_rows=slice(b * 64, b * 64 + 64))
                if flags[f"cfull_u{e}"]:
                    cuc = fpool.tile([128, ch], F32, tag="cgc")
                    nc.sync.dma_start(cuc[:, :cw_],
                                      dram[f"cu2_{e}"][:, c0:c0 + cw_])
                    nc.vector.scalar_tensor_tensor(
                        p[:, :cw_], cuc[:, :cw_], pooled2[:, :], p[:, :cw_],
                        op0=OP.mult, op1=OP.add)
                if flags[f"ab_u{e}"]:
                    abc = fpool.tile([128, ch], F32, tag="abc")
                    nc.sync.dma_start(abc[:, :cw_],
                                      dram[f"abu2_{e}"][:, c0:c0 + cw_])
                    nc.vector.tensor_add(p[:, :cw_], p[:, :cw_], abc[:, :cw_])
                nc.scalar.activation(hcf[:, c0:c0 + cw_], p[:, :cw_], AF.Tanh,
                                     bias=bias(f"uab2_{e}"))
                # b0 combine chunk-rolls right behind tanh
                csl = slice(c0, c0 + cw_)
                nc.vector.tensor_sub(hcf[0:64, csl], hcf[0:64, csl],
                                     X[0:64, csl])
                nc.vector.tensor_mul(hcf[0:64, csl], qts[0][:, csl],
                                     hcf[0:64, csl])
                nc.vector.tensor_add(X[0:64, csl], X[0:64, csl],
                                     hcf[0:64, csl])
            # b1: chunked realign (SWDGE; Pool engine is otherwise idle)
            for c0, cw_ in CH_B:
                csl = slice(c0, c0 + cw_)
                xsl = slice(n + c0, n + c0 + cw_)
                hct = sb.tile([64, ch], BF16, tag="hct", bufs=2)
                nc.gpsimd.dma_start(hct[:, :cw_], hcf[64:128, csl])
                nc.vector.tensor_sub(hct[:, :cw_], hct[:, :cw_], X[0:64, xsl])
                nc.vector.tensor_mul(hct[:, :cw_], qts[1][:, csl],
                                     hct[:, :cw_])
                nc.vector.tensor_add(X[0:64, xsl], X[0:64, xsl],
                                     hct[:, :cw_])

        def absorb(src_tile, we):
            # dummy ldweights pre-consuming {engine, DMA} waits on the PE so
            # subsequent real matmuls stay within the 2-slot MM wait limit
            # (real matmuls are self-loading, so this clobbers nothing)
            nc.tensor.ldweights(src_tile[0:66, 0:1])

        def encoder(e, xbf_src):
            nc.vector.memset(X[0:64, :], 0.0)
            nc.sync.dma_start(X2[64:65, :], dram[f"vu{e}"][:, :])
            nc.sync.dma_start(X[64:65, :], dram[f"vg{e}"][:, :])
            absorb(X, WGH[e])
            absorb(X2, WUH[e])
            for t in range(t_steps):
                nc.sync.dma_start(X2[65:66, :], xbf_src[t:t + 1, :])
                nc.sync.dma_start(X[65:66, :], xbf_src[t:t + 1, :])
                absorb(X, WGH[e])
                gfs_gate(e)
                absorb(X2, WUH[e])
                gfs_upd(e)

        # ================= encoder 1 =================
        encoder(0, srcbf)

        # heads 1+2 and encoder-2 input build
        for c0, cw_ in CH_W:
            p = ps.tile([2 * th, hch], F32, tag="ps")
            mm_into(p, CW01[:, :], X, slice(0, 64), c0, cw_)
            oc = sb.tile([2 * th, hch], F32, tag="hc", bufs=2)
            nc.scalar.activation(oc[:, :cw_], p[: 2 * th, :cw_], AF.Identity,
                                 bias=CVEC[0: 2 * th, cols["cb01"]:cols["cb01"] + 1])
            # rows 0:12 = src1 (head 1), rows 12:24 = out1 (head 0)
            nc.sync.dma_start(o1d[:, c0:c0 + cw_], oc[th: 2 * th, :cw_])
            sc = small.tile([th, hch], F32, tag="srcc")
            nc.sync.dma_start(sc[:, :cw_], src32[:, c0:c0 + cw_])
            xbb = sb.tile([th, hch], BF16, tag="xbb", bufs=2)
            nc.vector.tensor_sub(xbb[:, :cw_], sc[:, :cw_], oc[0:th, :cw_])
            nc.sync.dma_start(xsbf[:, c0:c0 + cw_], xbb[:, :cw_])

        # ================= encoder 2 =================
        encoder(1, xsbf)

        # head 3 + final sum
        for c0, cw_ in CH_W:
            p = ps.tile([th, hch], F32, tag="ps")
            mm_into(p, CW2[:, :], X, slice(0, 64), c0, cw_)
            o2 = sb.tile([th, hch], F32, tag="hc", bufs=2)
            nc.scalar.activation(o2[:, :cw_], p[:th, :cw_], AF.Identity,
                                 bias=CVEC[0:th, cols["cb2"]:cols["cb2"] + 1])
            o1c = sb.tile([th, hch], F32, tag="d", bufs=2)
            nc.sync.dma_start(o1c[:, :cw_], o1d[:, c0:c0 + cw_])
            nc.vector.tensor_add(o2[:, :cw_], o2[:, :cw_], o1c[:, :cw_])
            nc.sync.dma_start(out_d[:, c0:c0 + cw_], o2[:, :cw_])

    nc.compile()
    return nc


def _make_in_maps(inputs, H, n=N_FULL, t_steps=T):
    src = np.asarray(inputs["source"], np.float32)[..., 0]  # (B, T, n)
    in_maps = []
    for c in range(NCORES):
        m = dict(H["shared"])
        # src32[t, b*n + i] = src[2c+b, t, i]
        blk = src[BLOC * c: BLOC * (c + 1)]          # (BLOC, T, n)
        s = np.ascontiguousarray(
            blk.transpose(1, 0, 2).reshape(t_steps, BLOC * n))
        m["src32"] = s
        m["srcbf"] = s.astype(ml_dtypes.bfloat16)
        in_maps.append(m)
    return in_maps


def _assemble(results, n=N_FULL, t_steps=T):
    full = np.zeros((B, t_steps, n, 1), np.float32)
    for c in range(NCORES):
        o = np.asarray(results[c]["out"])            # [T, BLOC*n]
        o = o.reshape(t_steps, BLOC, n).transpose(1, 0, 2)
        full[BLOC * c: BLOC * (c + 1), :, :, 0] = o
    return full


# ---------------------------------------------------------------------------
# Fast path: GFS pooled term provably negligible -> drop both pooled matmul
# passes. State is b-packed [128, n] (partitions 64b+d); x_t contributions
# enter via K=1/K=2 accumulating matmuls from a resident [24, n] source tile
# (rows 2t+b), eliminating the per-step single-partition x-row DMAs.
# ---------------------------------------------------------------------------

TAU_MAX = 1e-3  # bound on the dropped pooled term's pre-activation magnitude


def _prep_fast(inputs):
    """Return fast-path host dict, or None if the pooled term is not
    provably negligible / affb nonzero."""
    f32 = np.float32
    n, th = N_FULL, T
    src = np.asarray(inputs["source"], f32)
    cw = np.asarray(inputs["conv_w"], f32)
    cb = np.asarray(inputs["conv_b"], f32)
    if np.any(np.asarray(inputs["gate_affb"])) or np.any(
            np.asarray(inputs["upd_affb"])):
        return None
    xmax0 = float(np.abs(src).max())
    s1max = float(np.max(np.sum(np.abs(cw[1]), axis=1) + np.abs(cb[1])))
    xmax = [xmax0, xmax0 + s1max]
    for e in range(2):
        for w_, b_, aff, aw, nw in (
            (inputs["gate_w"][e], inputs["gate_b"][e], inputs["gate_affw"][e],
             inputs["gate_aw"][e], inputs["gate_nw"][e]),
            (inputs["upd_w"][e], inputs["upd_b"][e], inputs["upd_affw"][e],
             inputs["upd_aw"][e], inputs["upd_nw"][e]),
        ):
            w_ = np.asarray(w_, f32)
            b_ = np.asarray(b_, f32)
            cmax = float(np.abs(np.asarray(aff, f32).T
                                * (np.asarray(aw, f32)[:, 0]
                                   * np.asarray(nw, f32)[0])[None, :]).max())
            premax = np.abs(b_) + np.abs(w_[0]) * xmax[e] + \
                np.sum(np.abs(w_[1:]), axis=0)
            tau = cmax * n * float(np.maximum(premax, 0.0).max())
            if tau > TAU_MAX:
                return None

    bf = ml_dtypes.bfloat16
    H = {"shared": {}}
    sh = H["shared"]
    for e in range(2):
        gaW = np.asarray(inputs["gate_alignW"][e], f32)   # [65, 128]
        gab = np.asarray(inputs["gate_alignb"][e], f32)   # [128]
        uaW = np.asarray(inputs["upd_alignW"][e], f32)    # [65, 64]
        uab = np.asarray(inputs["upd_alignb"][e], f32)    # [64]
        # b1 rows use swapped z/q column blocks so that z lands on psum rows
        # 64:128 for b1 (partition-aligned with the b-packed state) and q' on
        # 0:64; this keeps every gating TensorTensor op same-start-partition.
        swap = np.concatenate([gaW[:, 64:128], gaW[:, 0:64]], axis=1)
        wgp = np.zeros((128, 128), f32)
        wgp[0:64] = gaW[1:65]
        wgp[64:128] = swap[1:65]
        sh[f"wgp{e}"] = wgp.astype(bf)
        wxga = np.zeros((98, 128), f32)
        wxgb = np.zeros((98, 128), f32)
        for q in (0, 32, 64, 96):
            wxga[q] = gaW[0]          # K=1 row for gate b0
            wxgb[q + 1] = swap[0]     # K=2 rows [0; wx_swapped] for gate b1
        sh[f"wxga{e}"] = wxga.astype(bf)
        sh[f"wxgb{e}"] = wxgb.astype(bf)
        wubd = np.zeros((128, 128), f32)
        wubd[0:64, 0:64] = uaW[1:65]
        wubd[64:128, 64:128] = uaW[1:65]
        sh[f"wubd{e}"] = wubd.astype(bf)
        wxu = np.zeros((98, 128), f32)
        for q in (0, 32, 64, 96):
            wxu[q, 0:64] = uaW[0]
            wxu[q + 1, 64:128] = uaW[0]
        sh[f"wxu{e}"] = wxu.astype(bf)
        H[f"bzr{e}"] = np.concatenate([gab[:64], -gab[64:]])
        H[f"bzrS{e}"] = np.concatenate([-gab[64:], gab[:64]])
        H[f"uab2_{e}"] = np.concatenate([uab, uab])

    cwh1a = np.zeros((128, 24), f32)
    cwh1b = np.zeros((128, 24), f32)
    cwh2 = np.zeros((128, 24), f32)
    cb1p = np.zeros(24, f32)
    cb0p = np.zeros(24, f32)
    cb2p = np.zeros(24, f32)
    for b in range(2):
        for o in range(th):
            cwh1a[64 * b:64 * b + 64, 2 * o + b] = cw[1][o]
            cwh1b[64 * b:64 * b + 64, 2 * o + b] = cw[0][o]
            cwh2[64 * b:64 * b + 64, 2 * o + b] = cw[2][o]
            cb1p[2 * o + b] = cb[1][o]
            cb0p[2 * o + b] = cb[0][o]
            cb2p[2 * o + b] = cb[2][o]
    sh["cwh1a"] = cwh1a.astype(bf)
    sh["cwh1b"] = cwh1b.astype(bf)
    sh["cwh2"] = cwh2.astype(bf)

    cvec = np.zeros((128, 11), f32)
    cvec[0:64, 0] = 1.0        # szr (b0): z rows +, q rows -
    cvec[64:128, 0] = -1.0
    cvec[0:64, 1] = -1.0       # szrS (b1 swapped): q rows -, z rows +
    cvec[64:128, 1] = 1.0
    cvec[:, 2] = H["bzr0"]
    cvec[:, 3] = H["bzrS0"]
    cvec[:, 4] = H["bzr1"]
    cvec[:, 5] = H["bzrS1"]
    cvec[:, 6] = H["uab2_0"]
    cvec[:, 7] = H["uab2_1"]
    cvec[0:24, 8] = cb1p
    cvec[0:24, 9] = cb2p
    cvec[0:24, 10] = cb0p
    sh["cvec"] = cvec
    return H


def _build_fast(H, n=N_FULL, t_steps=T, ch=1024, mmc=512):
    nc = bacc.Bacc("TRN2", target_bir_lowering=False, debug=False)
    dram = {}
    for name, arr in H["shared"].items():
        dram[name] = nc.declare_dram_parameter(
            name, list(arr.shape), mybir.dt.from_np(arr.dtype), isOutput=False)
    srcp_d = nc.declare_dram_parameter("srcp", [2 * t_steps, n], BF16,
                                       isOutput=False)
    src32_d = nc.declare_dram_parameter("src32p", [2 * t_steps, n], F32,
                                        isOutput=False)
    out_d = nc.declare_dram_parameter("out", [2 * t_steps, n], F32,
                                      isOutput=True)
    xs2_d = nc.dram_tensor("xs2", [2 * t_steps, n], BF16)
    CH = list(reversed(_chunks(n, ch)))

    with tile.TileContext(nc) as tc, ExitStack() as ctx:
        pers = ctx.enter_context(tc.tile_pool(name="pers", bufs=1))

        def ptile(shape, dtype, nm):
            return pers.tile(shape, dtype, name=nm, tag=nm)

        SP = ptile([128, n], BF16, "SP")
        X2P = ptile([128, n], BF16, "X2P")
        QP = ptile([128, n], BF16, "QP")
        HC = ptile([128, n], BF16, "HC")
        ZQ = ptile([128, 2 * n], BF16, "ZQ")
        XQ = ptile([98, n], BF16, "XQ")
        O1B = ptile([24, n], F32, "O1B")
        CVEC = ptile(list(H["shared"]["cvec"].shape), F32, "CVEC")
        CWH1A = ptile([128, 24], BF16, "CWH1A")
        CWH1B = ptile([128, 24], BF16, "CWH1B")
        CWH2 = ptile([128, 24], BF16, "CWH2")
        WGP, WXGA, WXGB, WUBD, WXU = {}, {}, {}, {}, {}
        for e in range(2):
            WGP[e] = ptile([128, 128], BF16, f"wgp{e}")
            WXGA[e] = ptile([98, 128], BF16, f"wxga{e}")
            WXGB[e] = ptile([98, 128], BF16, f"wxgb{e}")
            WUBD[e] = ptile([128, 128], BF16, f"wubd{e}")
            WXU[e] = ptile([98, 128], BF16, f"wxu{e}")

        nc.sync.dma_start(CVEC[:, :], dram["cvec"][:, :])
        nc.sync.dma_start(CWH1A[:, :], dram["cwh1a"][:, :])
        nc.sync.dma_start(CWH1B[:, :], dram["cwh1b"][:, :])
        nc.sync.dma_start(CWH2[:, :], dram["cwh2"][:, :])
        for e in range(2):
            nc.sync.dma_start(WGP[e][:, :], dram[f"wgp{e}"][:, :])
            nc.sync.dma_start(WXGA[e][:, :], dram[f"wxga{e}"][:, :])
            nc.sync.dma_start(WXGB[e][:, :], dram[f"wxgb{e}"][:, :])
            nc.sync.dma_start(WUBD[e][:, :], dram[f"wubd{e}"][:, :])
            nc.sync.dma_start(WXU[e][:, :], dram[f"wxu{e}"][:, :])

        ps = ctx.enter_context(tc.tile_pool(name="ps", bufs=4, space="PSUM"))
        sb = ctx.enter_context(tc.tile_pool(name="sb", bufs=2))

        def bias(i, p=128):
            return CVEC[0:p, i:i + 1]

        def xq_load(src_ap, t):
            q = 32 * (t % 4)
            nc.sync.dma_start(XQ[q:q + 2, :], src_ap[2 * t:2 * t + 2, :])

        def cell(e, t, src_ap):
            q = 32 * (t % 4)
            first = t == 0
            # ---- gate: ZQ b-width; b0 cols rows 0:64=z, 64:128=q';
            #      b1 cols (swapped weights) rows 0:64=q', 64:128=z ----
            for b in range(2):
                boff = b * n
                bp = slice(64 * b, 64 * b + 64)
                for c0, cw_ in CH:
                    p = ps.tile([128, ch], F32, tag="ps")
                    for m0, mw in _chunks(cw_, mmc):
                        msl = slice(c0 + m0, c0 + m0 + mw)
                        if not first:
                            nc.tensor.matmul(p[:, m0:m0 + mw], WGP[e][bp, :],
                                             SP[bp, msl], start=True,
                                             stop=False)
                        if b == 0:
                            nc.tensor.matmul(p[:, m0:m0 + mw],
                                             WXGA[e][q:q + 1, :],
                                             XQ[q:q + 1, msl],
                                             start=first, stop=True,
                                             tile_position=(q, 0))
                        else:
                            nc.tensor.matmul(p[:, m0:m0 + mw],
                                             WXGB[e][q:q + 2, :],
                                             XQ[q:q + 2, msl],
                                             start=first, stop=True,
                                             tile_position=(q, 0))
                    osl = slice(boff + c0, boff + c0 + cw_)
                    nc.scalar.activation(ZQ[:, osl], p[:, :cw_], AF.Sigmoid,
                                         bias=bias(2 + 2 * e + b),
                                         scale=bias(b))
                    csl = slice(c0, c0 + cw_)
                    if not first:
                        # zs: z rows co-located with the b-packed state half
                        zeng = nc.vector if b == 0 else nc.gpsimd
                        zeng.tensor_mul(X2P[bp, csl], ZQ[bp, osl],
                                        SP[bp, csl])
                    # realign q' into b-packed QP (cross-partition copy)
                    qp = slice(64 - 64 * b, 128 - 64 * b)
                    nc.vector.tensor_copy(QP[bp, csl], ZQ[qp, osl])
            # ---- upd: hc = tanh(pre + uab2), b-packed; combine ----
            for c0, cw_ in CH:
                p = ps.tile([128, ch], F32, tag="ps")
                for m0, mw in _chunks(cw_, mmc):
                    msl = slice(c0 + m0, c0 + m0 + mw)
                    if not first:
                        nc.tensor.matmul(p[:, m0:m0 + mw], WUBD[e][:, :],
                                         X2P[:, msl], start=True, stop=False)
                    nc.tensor.matmul(p[:, m0:m0 + mw], WXU[e][q:q + 2, :],
                                     XQ[q:q + 2, msl],
                                     start=first, stop=True,
                                     tile_position=(q, 0))
                csl = slice(c0, c0 + cw_)
                nc.scalar.activation(HC[:, csl], p[:, :cw_], AF.Tanh,
                                     bias=bias(6 + e))
                if first:
                    # state was zero: s1 = q' * hc
                    nc.gpsimd.tensor_mul(SP[:, csl], QP[:, csl], HC[:, csl])
                else:
                    nc.vector.tensor_sub(HC[:, csl], HC[:, csl], SP[:, csl])
                    nc.gpsimd.tensor_mul(HC[:, csl], QP[:, csl], HC[:, csl])
                    nc.vector.tensor_add(SP[:, csl], SP[:, csl], HC[:, csl])
            # prefetch x for step t+4 into the quadrant this step just freed
            if t + 4 < t_steps:
                xq_load(src_ap, t + 4)

        # ================= encoder 1 =================
        for t in range(min(4, t_steps)):
            xq_load(srcp_d, t)
        for t in range(t_steps):
            cell(0, t, srcp_d)

        # heads 1+2 and encoder-2 input build
        hch = 1024
        for c0, cw_ in _chunks(n, hch):
            p1 = ps.tile([24, hch], F32, tag="ps")
            p2 = ps.tile([24, hch], F32, tag="ps")
            for m0, mw in _chunks(cw_, mmc):
                msl = slice(c0 + m0, c0 + m0 + mw)
                nc.tensor.matmul(p1[:, m0:m0 + mw], CWH1A[:, :], SP[:, msl],
                                 start=True, stop=True)
                nc.tensor.matmul(p2[:, m0:m0 + mw], CWH1B[:, :], SP[:, msl],
                                 start=True, stop=True)
            csl = slice(c0, c0 + cw_)
            o1a = sb.tile([24, hch], F32, tag="o1a")
            nc.scalar.activation(o1a[:, :cw_], p1[:24, :cw_], AF.Identity,
                                 bias=bias(8, 24))
            nc.scalar.activation(O1B[:, csl], p2[:24, :cw_], AF.Identity,
                                 bias=bias(10, 24))
            sc = sb.tile([24, hch], F32, tag="srcc")
            nc.sync.dma_start(sc[:, :cw_], src32_d[:, csl])
            x2c = sb.tile([24, hch], BF16, tag="x2c")
            nc.vector.tensor_sub(x2c[:, :cw_], sc[:, :cw_], o1a[:, :cw_])
            nc.sync.dma_start(xs2_d[:, csl], x2c[:, :cw_])

        # ================= encoder 2 =================
        for t in range(min(4, t_steps)):
            xq_load(xs2_d, t)
        for t in range(t_steps):
            cell(1, t, xs2_d)

        # head 3 + final sum
        hch = 1024
        for c0, cw_ in _chunks(n, hch):
            p = ps.tile([24, hch], F32, tag="ps")
            for m0, mw in _chunks(cw_, mmc):
                msl = slice(c0 + m0, c0 + m0 + mw)
                nc.tensor.matmul(p[0:24, m0:m0 + mw], CWH2[:, :], SP[:, msl],
                                 start=True, stop=True)
            csl = slice(c0, c0 + cw_)
            o2 = sb.tile([24, hch], F32, tag="o2")
            nc.scalar.activation(o2[:, :cw_], p[0:24, :cw_], AF.Identity,
                                 bias=bias(9, 24))
            nc.vector.tensor_add(o2[:, :cw_], o2[:, :cw_], O1B[:, csl])
            nc.sync.dma_start(out_d[:, csl], o2[:, :cw_])

    nc.compile()
    return nc


def _make_in_maps_fast(inputs, H, n=N_FULL, t_steps=T):
    src = np.asarray(inputs["source"], np.float32)[..., 0]  # (B, T, n)
    in_maps = []
    for c in range(NCORES):
        m = dict(H["shared"])
        s = np.zeros((2 * t_steps, n), np.float32)
        for b in range(BLOC):
            s[b::2] = src[BLOC * c + b]  # row 2t+b = src[2c+b, t]
        m["src32p"] = s
        m["srcp"] = s.astype(ml_dtypes.bfloat16)
        in_maps.append(m)
    return in_maps


def _assemble_fast(results, n=N_FULL, t_steps=T):
    full = np.zeros((B, t_steps, n, 1), np.float32)
    for c in range(NCORES):
        o = np.asarray(results[c]["out"])            # [2T, n], row 2o+b
        for b in range(BLOC):
            full[BLOC * c + b, :, :, 0] = o[b::2]
    return full


_PROG_CACHE = {}


def _prepare(inputs):
    """Pick fast/exact path; return (nc, in_maps, assemble_fn)."""
    HF = _prep_fast(inputs)
    if HF is not None:
        if "fast" not in _PROG_CACHE:
            _PROG_CACHE["fast"] = _build_fast(HF)
        return _PROG_CACHE["fast"], _make_in_maps_fast(inputs, HF), \
            _assemble_fast
    H = _prep_host(inputs)
    key = tuple(sorted(H["flags"].items()))
    if key not in _PROG_CACHE:
        _PROG_CACHE[key] = _build(H)
    return _PROG_CACHE[key], _make_in_maps(inputs, H), _assemble


def kernel(**inputs) -> np.ndarray:
    nc, in_maps, assemble = _prepare(inputs)
    res = run_bass_kernel_spmd(nc, in_maps, core_ids=list(range(NCORES)))
    return assemble(res.results)



# revision 17
# speedup vs baseline: 2.3183x; 1.0032x over previous
"""Trainium2 Bass kernel for nn_DDGCRN (gnn_message_passing).

DDGCRN: two 12-step GRU-style encoders over B=16, N=8600 nodes, HID=64,
with a global node-pooling term (GFS) inside each gate, plus conv heads.

Sharding: data-parallel over batch. B=16 / 8 cores = 2 batch elems per
core; the GFS node-pooling sum is per-batch-element, so no collectives.

Per-core layout: feature-major. All wide tensors are [D, W] where
W = 2*8600 = 17200 columns (col = b*N + n). State tile X is [66, W]:
rows 0:64 = hidden state, row 64 = x_t, row 65 = v (rank-1 pooling row).
Weight rows are host-permuted to match ([W_state; w_x; aug]).

The GFS pooled term affw*(pooled*scale) is rank-1 in (d, n) for the
given inputs (affw==1): C[d,n] = u[d]*v[n]. It is folded into the
"res" matmul by augmenting K: lhsT row 65 = pooled*u (written per cell
via PE-transpose), rhs row 65 = v (constant). Non-rank-1 / nonzero-affb
inputs fall back to explicit DVE ops with C/AB streamed from DRAM.
"""

import numpy as np
import ml_dtypes
from contextlib import ExitStack

import concourse.bass as bass
import concourse.bacc as bacc
import concourse.tile as tile
from concourse import mybir
from concourse.bass_utils import run_bass_kernel_spmd

F32 = mybir.dt.float32
F32R = mybir.dt.float32r
BF16 = mybir.dt.bfloat16
AX = mybir.AxisListType
OP = mybir.AluOpType
AF = mybir.ActivationFunctionType

# Problem constants (hardcoded; kernel.py must be self-contained)
B, T, N_FULL, HID, IN = 16, 12, 8600, 64, 1
GIN = IN + HID
NCORES = 8
BLOC = B // NCORES  # 2


def _chunks(total, size):
    out = []
    off = 0
    while off < total:
        w = min(size, total - off)
        out.append((off, w))
        off += w
    return out


def _rank1(C):
    """C [D, M] -> (u [D], v [M]) with C == outer(u, v), or None."""
    d0, m0 = np.unravel_index(np.argmax(np.abs(C)), C.shape)
    piv = C[d0, m0]
    if abs(piv) < 1e-30:
        return np.zeros(C.shape[0], np.float32), np.zeros(C.shape[1], np.float32)
    u = C[:, m0].astype(np.float64)
    v = C[d0, :].astype(np.float64) / piv
    if not np.allclose(np.outer(u, v), C, rtol=1e-5, atol=1e-7 * abs(piv)):
        return None
    return u.astype(np.float32), v.astype(np.float32)


def _prep_host(inputs, n=N_FULL, t_steps=T):
    """Host-side parameter prep. Returns dict of per-core-shared arrays
    plus flags. All weight matrices get the row permutation
    [rows 1:65 (state); row 0 (x); (aug row 0)]."""
    f32 = np.float32
    H = {"flags": {}}
    shared = {}

    def perm66(w):
        """[65, Dout] -> [66, Dout]: rows 0:64 = state weights, row 64 = 0
        (v-row slot / aug slot), row 65 = x weight."""
        z = np.zeros((1, w.shape[1]), f32)
        return np.concatenate([w[1:], z, w[0:1]], axis=0).astype(f32)

    for e in range(2):
        gaW = np.asarray(inputs["gate_alignW"][e], f32)   # [65, 128]
        gw = np.asarray(inputs["gate_w"][e], f32)         # [65, 128]
        gab = np.asarray(inputs["gate_alignb"][e], f32)   # [128]
        gb = np.asarray(inputs["gate_b"][e], f32)         # [128]
        uaW = np.asarray(inputs["upd_alignW"][e], f32)    # [65, 64]
        uw = np.asarray(inputs["upd_w"][e], f32)          # [65, 64]
        uab = np.asarray(inputs["upd_alignb"][e], f32)    # [64]
        ub = np.asarray(inputs["upd_b"][e], f32)          # [64]

        shared[f"wgh{e}"] = perm66(gw).astype(ml_dtypes.bfloat16)
        shared[f"wga{e}"] = perm66(gaW).astype(ml_dtypes.bfloat16)
        shared[f"wuh{e}"] = perm66(uw).astype(ml_dtypes.bfloat16)
        shared[f"wua{e}"] = perm66(uaW).astype(ml_dtypes.bfloat16)

        # rank-1 pooling factors: C[d, n] = affw[n, d] * aw[n] * nw[n]
        for kind, aff, aw, nw, dout in (
            ("g", inputs["gate_affw"][e], inputs["gate_aw"][e], inputs["gate_nw"][e], 128),
            ("u", inputs["upd_affw"][e], inputs["upd_aw"][e], inputs["upd_nw"][e], 64),
        ):
            scale = (np.asarray(aw, f32)[:, 0] * np.asarray(nw, f32)[0])  # [n]
            C = np.asarray(aff, f32).T * scale[None, :]  # [dout, n]
            r1 = _rank1(C)
            if r1 is None:
                H["flags"][f"cfull_{kind}{e}"] = True
                u = np.zeros(dout, f32)
                v = np.zeros(C.shape[1], f32)
                if kind == "g":
                    shared[f"cg{e}"] = np.ascontiguousarray(C)
                else:
                    shared[f"cu2_{e}"] = np.ascontiguousarray(
                        np.concatenate([C, C], axis=0))
            else:
                H["flags"][f"cfull_{kind}{e}"] = False
                u, v = r1
            if kind == "g":
                ug = u
                vg = v
            else:
                uu = u
                vu = v

        # affb fallback (AB tensors). gab/uab per-partition parts go in ACT bias.
        abg = np.asarray(inputs["gate_affb"][e], f32).T  # [128, n]
        abu = np.asarray(inputs["upd_affb"][e], f32).T   # [64, n]
        H["flags"][f"ab_g{e}"] = bool(np.any(abg))
        H["flags"][f"ab_u{e}"] = bool(np.any(abu))
        if H["flags"][f"ab_g{e}"]:
            shared[f"abg{e}"] = np.ascontiguousarray(abg)
        if H["flags"][f"ab_u{e}"]:
            shared[f"abu2_{e}"] = np.ascontiguousarray(
                np.concatenate([abu, abu], axis=0))

        # v rows, repeated per local batch elem
        shared[f"vg{e}"] = np.tile(vg[None, :], (1, BLOC)).astype(ml_dtypes.bfloat16)
        shared[f"vu{e}"] = np.tile(vu[None, :], (1, BLOC)).astype(ml_dtypes.bfloat16)

        # bias/scale vector columns
        H[f"gb{e}"] = gb
        H[f"bzr{e}"] = np.concatenate([gab[:64], -gab[64:]])
        H[f"ub2_{e}"] = np.concatenate([ub, ub])
        H[f"uab2_{e}"] = np.concatenate([uab, uab])
        H[f"ug{e}"] = ug
        H[f"uu2_{e}"] = np.concatenate([uu, uu])

    cw = np.asarray(inputs["conv_w"], f32)  # [3, 12, 64]
    cb = np.asarray(inputs["conv_b"], f32)  # [3, 12]
    # head order: [src1(12) | out1(12)] so src1 sits at partitions 0:12
    shared["cw01"] = np.concatenate([cw[1].T, cw[0].T], axis=1).astype(
        ml_dtypes.bfloat16)                                       # [64, 2T]
    shared["cw2"] = np.ascontiguousarray(cw[2].T).astype(ml_dtypes.bfloat16)
    shared["ident"] = np.eye(128, dtype=f32)

    # cvec columns
    ncol = 16
    cvec = np.zeros((128, ncol), f32)
    cvec[:64, 0] = 1.0
    cvec[64:, 0] = -1.0
    cols = {"szr": 0}
    ci = 1
    for e in range(2):
        for nm in ("gb", "bzr", "ub2_", "uab2_", "ug", "uu2_"):
            key = f"{nm}{e}"
            arr = H[key]
            cvec[: len(arr), ci] = arr
            cols[key] = ci
            ci += 1
    th = cb.shape[1]
    cvec[: 2 * th, ci] = np.concatenate([cb[1], cb[0]])
    cols["cb01"] = ci
    ci += 1
    cvec[:th, ci] = cb[2]
    cols["cb2"] = ci
    shared["cvec"] = cvec
    H["cols"] = cols
    H["shared"] = shared
    return H


def _build(H, n=N_FULL, t_steps=T, ch=1024, mmc=512):
    """Build the single-core Bass program (same for all cores)."""
    W = BLOC * n
    flags = H["flags"]
    cols = H["cols"]
    nc = bacc.Bacc("TRN2", target_bir_lowering=False, debug=False)

    dram = {}
    for name, arr in H["shared"].items():
        dram[name] = nc.declare_dram_parameter(
            name, list(arr.shape), mybir.dt.from_np(arr.dtype), isOutput=False)
    src32 = nc.declare_dram_parameter("src32", [t_steps, W], F32, isOutput=False)
    srcbf = nc.declare_dram_parameter("srcbf", [t_steps, W], BF16, isOutput=False)
    out_d = nc.declare_dram_parameter("out", [t_steps, W], F32, isOutput=True)
    o1d = nc.dram_tensor("o1d", [t_steps, W], F32)
    xsbf = nc.dram_tensor("xsbf", [t_steps, W], BF16)

    CH_B = _chunks(n, ch)       # chunks within one batch-half
    hch = min(ch, 1024)
    CH_W = _chunks(W, hch)      # chunks over full width (boundary passes)
    nchb = len(CH_B)

    with tile.TileContext(nc) as tc, ExitStack() as ctx:
        # ---- persistent tiles (one pool, distinct tags = one slot each) ----
        pers = ctx.enter_context(tc.tile_pool(name="pers", bufs=1))

        def ptile(shape, dtype, nm):
            return pers.tile(shape, dtype, name=nm, tag=nm)

        X = ptile([66, W], BF16, "X")
        X2 = ptile([66, W], BF16, "X2")
        ZQ = ptile([128, W], BF16, "ZQ")
        CVEC = ptile(list(H["shared"]["cvec"].shape), F32, "CVEC")
        IDENT = ptile([128, 128], F32, "IDENT")
        th = t_steps
        CW01 = ptile([64, 2 * th], BF16, "CW01")
        CW2 = ptile([64, th], BF16, "CW2")
        WGH = {}
        WGA = {}
        WUH = {}
        WUA = {}
        for e in range(2):
            WGH[e] = ptile([66, 128], BF16, f"twgh{e}")
            WUH[e] = ptile([66, 64], BF16, f"twuh{e}")
            WGA[e] = {}
            WUA[e] = {}
            for b in range(BLOC):
                WGA[e][b] = ptile([66, 128], BF16, f"twga{e}{b}")
                WUA[e][b] = ptile([66, 64], BF16, f"twua{e}{b}")

        # (Bacc.generate_event_semaphores legalizes wait counts at compile)
        nc.sync.dma_start(CVEC[:, :], dram["cvec"][:, :])
        nc.sync.dma_start(IDENT[:, :], dram["ident"][:, :])
        nc.sync.dma_start(CW01[:, :], dram["cw01"][:, :])
        nc.sync.dma_start(CW2[:, :], dram["cw2"][:, :])
        for e in range(2):
            nc.sync.dma_start(WGH[e][:, :], dram[f"wgh{e}"][:, :])
            nc.sync.dma_start(WUH[e][:, :], dram[f"wuh{e}"][:, :])
            for b in range(BLOC):
                nc.sync.dma_start(WGA[e][b][:, :], dram[f"wga{e}"][:, :])
                nc.sync.dma_start(WUA[e][b][:, :], dram[f"wua{e}"][:, :])

        def bias(key):
            return CVEC[:, cols[key]:cols[key] + 1]

        # ---- pools ----
        ps = ctx.enter_context(tc.tile_pool(name="ps", bufs=3, space="PSUM"))
        tps = ctx.enter_context(tc.tile_pool(name="tps", bufs=2, space="PSUM"))
        sb = ctx.enter_context(tc.tile_pool(name="sb", bufs=3))
        small = ctx.enter_context(tc.tile_pool(name="small", bufs=2))
        fpool = ctx.enter_context(tc.tile_pool(name="fpool", bufs=2))

        def mm_into(p, lhsT, rhs_tile, rhs_rows, coff, cw_, p_rows=None,
                    cast=None):
            """Matmul chunk [*, cw_] at col offset coff into psum tile p."""
            for m0, mw in _chunks(cw_, mmc):
                lhs_ap = lhsT
                rhs_ap = rhs_tile[rhs_rows, coff + m0:coff + m0 + mw]
                o = p[:, m0:m0 + mw] if p_rows is None else \
                    p[p_rows, m0:m0 + mw]
                if cast is not None:
                    lhs_ap = lhs_ap.bitcast(cast)
                    rhs_ap = rhs_ap.bitcast(cast)
                nc.tensor.matmul(o, lhs_ap, rhs_ap, start=True, stop=True)

        def gfs_gate(e):
            pooled_b = []
            for b in range(BLOC):
                boff = b * n
                parts = small.tile([128, nchb], F32, tag="parts")
                for ci, (c0, cw_) in enumerate(CH_B):
                    p = ps.tile([128, ch], F32, tag="ps")
                    mm_into(p, WGH[e][:, :], X, slice(0, 66), boff + c0, cw_)
                    if False:
                        # DVE relu+bias+accum for engine balance
                        nc.vector.tensor_scalar(p[:, :cw_], p[:, :cw_],
                                                bias(f"gb{e}"), 0.0,
                                                op0=OP.add, op1=OP.max,
                                                accum_out=parts[:, ci:ci + 1])
                    else:
                        nc.scalar.activation(p[:, :cw_], p[:, :cw_], AF.Relu,
                                             bias=bias(f"gb{e}"),
                                             accum_out=parts[:, ci:ci + 1])
                if not flags[f"cfull_g{e}"]:
                    # fused: scratch = parts*u, accum = sum = pooled*u
                    scr = small.tile([128, nchb], F32, tag="pscr")
                    pgs = small.tile([128, 1], F32, tag="pgs")
                    nc.vector.tensor_scalar(scr[:, :], parts[:, :],
                                            bias(f"ug{e}"), 0.0, op0=OP.mult,
                                            op1=OP.add, accum_out=pgs[:, :])
                    tp = tps.tile([1, 128], F32, tag="tp")
                    nc.tensor.transpose(tp[:, :], pgs[:, :], IDENT[:, :])
                    nc.vector.tensor_copy(WGA[e][b][64:65, 0:128],
                                          tp[0:1, 0:128])
                    pooled_b.append(None)
                else:
                    pooled = small.tile([128, 1], F32, tag="pooled")
                    nc.vector.tensor_reduce(pooled[:, :], parts[:, :],
                                            axis=AX.X, op=OP.add)
                    pooled_b.append(pooled)
            # pass 2: res(+aug) matmul -> sigmoid -> ZQ (z rows 0:64, q 64:128)
            for b in range(BLOC):
                boff = b * n
                for c0, cw_ in CH_B:
                    p = ps.tile([128, ch], F32, tag="ps")
                    mm_into(p, WGA[e][b][:, :], X, slice(0, 66), boff + c0,
                            cw_)
                    if flags[f"cfull_g{e}"]:
                        cgc = fpool.tile([128, ch], F32, tag="cgc")
                        nc.sync.dma_start(cgc[:, :cw_],
                                          dram[f"cg{e}"][:, c0:c0 + cw_])
                        nc.vector.scalar_tensor_tensor(
                            p[:, :cw_], cgc[:, :cw_], pooled_b[b][:, :],
                            p[:, :cw_], op0=OP.mult, op1=OP.add)
                    if flags[f"ab_g{e}"]:
                        abc = fpool.tile([128, ch], F32, tag="abc")
                        nc.sync.dma_start(abc[:, :cw_],
                                          dram[f"abg{e}"][:, c0:c0 + cw_])
                        nc.vector.tensor_add(p[:, :cw_], p[:, :cw_],
                                             abc[:, :cw_])
                    nc.scalar.activation(ZQ[:, boff + c0:boff + c0 + cw_],
                                         p[:, :cw_], AF.Sigmoid,
                                         bias=bias(f"bzr{e}"),
                                         scale=bias("szr"))
                    # zs = z * S -> X2 state rows (b0 on gpsimd for balance)
                    zeng = nc.gpsimd if b == 0 else nc.vector
                    zeng.tensor_mul(
                        X2[0:64, boff + c0:boff + c0 + cw_],
                        ZQ[0:64, boff + c0:boff + c0 + cw_],
                        X[0:64, boff + c0:boff + c0 + cw_])

        def gfs_upd(e):
            parts = small.tile([128, nchb], F32, tag="parts")
            for ci, (c0, cw_) in enumerate(CH_B):
                p = ps.tile([128, ch], F32, tag="ps")
                for b in range(BLOC):
                    mm_into(p, WUH[e][:, :], X2, slice(0, 66), b * n + c0,
                            cw_, p_rows=slice(b * 64, b * 64 + 64))
                if ci % 2 == 0:
                    nc.scalar.activation(p[:, :cw_], p[:, :cw_], AF.Relu,
                                         bias=bias(f"ub2_{e}"),
                                         accum_out=parts[:, ci:ci + 1])
                else:
                    # DVE version of relu+bias+accum to balance engines
                    nc.vector.tensor_scalar(p[:, :cw_], p[:, :cw_],
                                            bias(f"ub2_{e}"), 0.0,
                                            op0=OP.add, op1=OP.max,
                                            accum_out=parts[:, ci:ci + 1])
            pooled2 = None
            if not flags[f"cfull_u{e}"]:
                scr = small.tile([128, nchb], F32, tag="pscr")
                pgs = small.tile([128, 1], F32, tag="pgs")
                nc.vector.tensor_scalar(scr[:, :], parts[:, :],
                                        bias(f"uu2_{e}"), 0.0, op0=OP.mult,
                                        op1=OP.add, accum_out=pgs[:, :])
                tp = tps.tile([1, 128], F32, tag="tp")
                nc.tensor.transpose(tp[:, :], pgs[:, :], IDENT[:, :])
                nc.vector.tensor_copy(WUA[e][0][64:65, 0:64], tp[0:1, 0:64])
                nc.vector.tensor_copy(WUA[e][1][64:65, 0:64],
                                      tp[0:1, 64:128])
            else:
                pooled2 = small.tile([128, 1], F32, tag="pooled")
                nc.vector.tensor_reduce(pooled2[:, :], parts[:, :],
                                        axis=AX.X, op=OP.add)
            # full-width q realign: one SBUF->SBUF DMA per batch elem,
            # issued while U1 still runs (sigmoid is long done)
            qts = []
            for b in range(BLOC):
                qt = sb.tile([64, n], BF16, tag="qt", bufs=2)
                nc.gpsimd.dma_start(qt[:, :], ZQ[64:128, b * n:(b + 1) * n])
                qts.append(qt)
            hcf = sb.tile([128, n], BF16, tag="hcf", bufs=2)
            # pass 2: res2(+aug) -> tanh -> combine into state
            for c0, cw_ in CH_B:
                p = ps.tile([128, ch], F32, tag="ps")
                for b in range(BLOC):
                    mm_into(p, WUA[e][b][:, :], X2, slice(0, 66), b * n + c0,
                            cw_, p_rows=slice(b * 64, b * 64 + 64))
                if flags[f"cfull_u{e}"]:
                    cuc = fpool.tile([128, ch], F32, tag="cgc")
                    nc.sync.dma_start(cuc[:, :cw_],
                                      dram[f"cu2_{e}"][:, c0:c0 + cw_])
                    nc.vector.scalar_tensor_tensor(
                        p[:, :cw_], cuc[:, :cw_], pooled2[:, :], p[:, :cw_],
                        op0=OP.mult, op1=OP.add)
                if flags[f"ab_u{e}"]:
                    abc = fpool.tile([128, ch], F32, tag="abc")
                    nc.sync.dma_start(abc[:, :cw_],
                                      dram[f"abu2_{e}"][:, c0:c0 + cw_])
                    nc.vector.tensor_add(p[:, :cw_], p[:, :cw_], abc[:, :cw_])
                nc.scalar.activation(hcf[:, c0:c0 + cw_], p[:, :cw_], AF.Tanh,
                                     bias=bias(f"uab2_{e}"))
                # b0 combine chunk-rolls right behind tanh
                csl = slice(c0, c0 + cw_)
                nc.vector.tensor_sub(hcf[0:64, csl], hcf[0:64, csl],
                                     X[0:64, csl])
                nc.vector.tensor_mul(hcf[0:64, csl], qts[0][:, csl],
                                     hcf[0:64, csl])
                nc.vector.tensor_add(X[0:64, csl], X[0:64, csl],
                                     hcf[0:64, csl])
            # b1: chunked realign (SWDGE; Pool engine is otherwise idle)
            for c0, cw_ in CH_B:
                csl = slice(c0, c0 + cw_)
                xsl = slice(n + c0, n + c0 + cw_)
                hct = sb.tile([64, ch], BF16, tag="hct", bufs=2)
                nc.gpsimd.dma_start(hct[:, :cw_], hcf[64:128, csl])
                nc.vector.tensor_sub(hct[:, :cw_], hct[:, :cw_], X[0:64, xsl])
                nc.vector.tensor_mul(hct[:, :cw_], qts[1][:, csl],
                                     hct[:, :cw_])
                nc.vector.tensor_add(X[0:64, xsl], X[0:64, xsl],
                                     hct[:, :cw_])

        def absorb(src_tile, we):
            # dummy ldweights pre-consuming {engine, DMA} waits on the PE so
            # subsequent real matmuls stay within the 2-slot MM wait limit
            # (real matmuls are self-loading, so this clobbers nothing)
            nc.tensor.ldweights(src_tile[0:66, 0:1])

        def encoder(e, xbf_src):
            nc.vector.memset(X[0:64, :], 0.0)
            nc.sync.dma_start(X2[64:65, :], dram[f"vu{e}"][:, :])
            nc.sync.dma_start(X[64:65, :], dram[f"vg{e}"][:, :])
            absorb(X, WGH[e])
            absorb(X2, WUH[e])
            for t in range(t_steps):
                nc.sync.dma_start(X2[65:66, :], xbf_src[t:t + 1, :])
                nc.sync.dma_start(X[65:66, :], xbf_src[t:t + 1, :])
                absorb(X, WGH[e])
                gfs_gate(e)
                absorb(X2, WUH[e])
                gfs_upd(e)

        # ================= encoder 1 =================
        encoder(0, srcbf)

        # heads 1+2 and encoder-2 input build
        for c0, cw_ in CH_W:
            p = ps.tile([2 * th, hch], F32, tag="ps")
            mm_into(p, CW01[:, :], X, slice(0, 64), c0, cw_)
            oc = sb.tile([2 * th, hch], F32, tag="hc", bufs=2)
            nc.scalar.activation(oc[:, :cw_], p[: 2 * th, :cw_], AF.Identity,
                                 bias=CVEC[0: 2 * th, cols["cb01"]:cols["cb01"] + 1])
            # rows 0:12 = src1 (head 1), rows 12:24 = out1 (head 0)
            nc.sync.dma_start(o1d[:, c0:c0 + cw_], oc[th: 2 * th, :cw_])
            sc = small.tile([th, hch], F32, tag="srcc")
            nc.sync.dma_start(sc[:, :cw_], src32[:, c0:c0 + cw_])
            xbb = sb.tile([th, hch], BF16, tag="xbb", bufs=2)
            nc.vector.tensor_sub(xbb[:, :cw_], sc[:, :cw_], oc[0:th, :cw_])
            nc.sync.dma_start(xsbf[:, c0:c0 + cw_], xbb[:, :cw_])

        # ================= encoder 2 =================
        encoder(1, xsbf)

        # head 3 + final sum
        for c0, cw_ in CH_W:
            p = ps.tile([th, hch], F32, tag="ps")
            mm_into(p, CW2[:, :], X, slice(0, 64), c0, cw_)
            o2 = sb.tile([th, hch], F32, tag="hc", bufs=2)
            nc.scalar.activation(o2[:, :cw_], p[:th, :cw_], AF.Identity,
                                 bias=CVEC[0:th, cols["cb2"]:cols["cb2"] + 1])
            o1c = sb.tile([th, hch], F32, tag="d", bufs=2)
            nc.sync.dma_start(o1c[:, :cw_], o1d[:, c0:c0 + cw_])
            nc.vector.tensor_add(o2[:, :cw_], o2[:, :cw_], o1c[:, :cw_])
            nc.sync.dma_start(out_d[:, c0:c0 + cw_], o2[:, :cw_])

    nc.compile()
    return nc


def _make_in_maps(inputs, H, n=N_FULL, t_steps=T):
    src = np.asarray(inputs["source"], np.float32)[..., 0]  # (B, T, n)
    in_maps = []
    for c in range(NCORES):
        m = dict(H["shared"])
        # src32[t, b*n + i] = src[2c+b, t, i]
        blk = src[BLOC * c: BLOC * (c + 1)]          # (BLOC, T, n)
        s = np.ascontiguousarray(
            blk.transpose(1, 0, 2).reshape(t_steps, BLOC * n))
        m["src32"] = s
        m["srcbf"] = s.astype(ml_dtypes.bfloat16)
        in_maps.append(m)
    return in_maps


def _assemble(results, n=N_FULL, t_steps=T):
    full = np.zeros((B, t_steps, n, 1), np.float32)
    for c in range(NCORES):
        o = np.asarray(results[c]["out"])            # [T, BLOC*n]
        o = o.reshape(t_steps, BLOC, n).transpose(1, 0, 2)
        full[BLOC * c: BLOC * (c + 1), :, :, 0] = o
    return full


# ---------------------------------------------------------------------------
# Fast path: GFS pooled term provably negligible -> drop both pooled matmul
# passes. State is b-packed [128, n] (partitions 64b+d); x_t contributions
# enter via K=1/K=2 accumulating matmuls from a resident [24, n] source tile
# (rows 2t+b), eliminating the per-step single-partition x-row DMAs.
# ---------------------------------------------------------------------------

TAU_MAX = 1e-3  # bound on the dropped pooled term's pre-activation magnitude


def _prep_fast(inputs):
    """Return fast-path host dict, or None if the pooled term is not
    provably negligible / affb nonzero."""
    f32 = np.float32
    n, th = N_FULL, T
    src = np.asarray(inputs["source"], f32)
    cw = np.asarray(inputs["conv_w"], f32)
    cb = np.asarray(inputs["conv_b"], f32)
    if np.any(np.asarray(inputs["gate_affb"])) or np.any(
            np.asarray(inputs["upd_affb"])):
        return None
    xmax0 = float(np.abs(src).max())
    s1max = float(np.max(np.sum(np.abs(cw[1]), axis=1) + np.abs(cb[1])))
    xmax = [xmax0, xmax0 + s1max]
    for e in range(2):
        for w_, b_, aff, aw, nw in (
            (inputs["gate_w"][e], inputs["gate_b"][e], inputs["gate_affw"][e],
             inputs["gate_aw"][e], inputs["gate_nw"][e]),
            (inputs["upd_w"][e], inputs["upd_b"][e], inputs["upd_affw"][e],
             inputs["upd_aw"][e], inputs["upd_nw"][e]),
        ):
            w_ = np.asarray(w_, f32)
            b_ = np.asarray(b_, f32)
            cmax = float(np.abs(np.asarray(aff, f32).T
                                * (np.asarray(aw, f32)[:, 0]
                                   * np.asarray(nw, f32)[0])[None, :]).max())
            premax = np.abs(b_) + np.abs(w_[0]) * xmax[e] + \
                np.sum(np.abs(w_[1:]), axis=0)
            tau = cmax * n * float(np.maximum(premax, 0.0).max())
            if tau > TAU_MAX:
                return None

    bf = ml_dtypes.bfloat16
    H = {"shared": {}}
    sh = H["shared"]
    for e in range(2):
        gaW = np.asarray(inputs["gate_alignW"][e], f32)   # [65, 128]
        gab = np.asarray(inputs["gate_alignb"][e], f32)   # [128]
        uaW = np.asarray(inputs["upd_alignW"][e], f32)    # [65, 64]
        uab = np.asarray(inputs["upd_alignb"][e], f32)    # [64]
        # b1 rows use swapped z/q column blocks so that z lands on psum rows
        # 64:128 for b1 (partition-aligned with the b-packed state) and q' on
        # 0:64; this keeps every gating TensorTensor op same-start-partition.
        swap = np.concatenate([gaW[:, 64:128], gaW[:, 0:64]], axis=1)
        wgp = np.zeros((128, 128), f32)
        wgp[0:64] = gaW[1:65]
        wgp[64:128] = swap[1:65]
        sh[f"wgp{e}"] = wgp.astype(bf)
        wxga = np.zeros((98, 128), f32)
        wxgb = np.zeros((98, 128), f32)
        for q in (0, 32, 64, 96):
            wxga[q] = gaW[0]          # K=1 row for gate b0
            wxgb[q + 1] = swap[0]     # K=2 rows [0; wx_swapped] for gate b1
        sh[f"wxga{e}"] = wxga.astype(bf)
        sh[f"wxgb{e}"] = wxgb.astype(bf)
        wubd = np.zeros((128, 128), f32)
        wubd[0:64, 0:64] = uaW[1:65]
        wubd[64:128, 64:128] = uaW[1:65]
        sh[f"wubd{e}"] = wubd.astype(bf)
        wxu = np.zeros((98, 128), f32)
        for q in (0, 32, 64, 96):
            wxu[q, 0:64] = uaW[0]
            wxu[q + 1, 64:128] = uaW[0]
        sh[f"wxu{e}"] = wxu.astype(bf)
        H[f"bzr{e}"] = np.concatenate([gab[:64], -gab[64:]])
        H[f"bzrS{e}"] = np.concatenate([-gab[64:], gab[:64]])
        H[f"uab2_{e}"] = np.concatenate([uab, uab])

    cwh1a = np.zeros((128, 24), f32)
    cwh1b = np.zeros((128, 24), f32)
    cwh2 = np.zeros((128, 24), f32)
    cb1p = np.zeros(24, f32)
    cb0p = np.zeros(24, f32)
    cb2p = np.zeros(24, f32)
    for b in range(2):
        for o in range(th):
            cwh1a[64 * b:64 * b + 64, 2 * o + b] = cw[1][o]
            cwh1b[64 * b:64 * b + 64, 2 * o + b] = cw[0][o]
            cwh2[64 * b:64 * b + 64, 2 * o + b] = cw[2][o]
            cb1p[2 * o + b] = cb[1][o]
            cb0p[2 * o + b] = cb[0][o]
            cb2p[2 * o + b] = cb[2][o]
    sh["cwh1a"] = cwh1a.astype(bf)
    sh["cwh1b"] = cwh1b.astype(bf)
    sh["cwh2"] = cwh2.astype(bf)

    cvec = np.zeros((128, 11), f32)
    cvec[0:64, 0] = 1.0        # szr (b0): z rows +, q rows -
    cvec[64:128, 0] = -1.0
    cvec[0:64, 1] = -1.0       # szrS (b1 swapped): q rows -, z rows +
    cvec[64:128, 1] = 1.0
    cvec[:, 2] = H["bzr0"]
    cvec[:, 3] = H["bzrS0"]
    cvec[:, 4] = H["bzr1"]
    cvec[:, 5] = H["bzrS1"]
    cvec[:, 6] = H["uab2_0"]
    cvec[:, 7] = H["uab2_1"]
    cvec[0:24, 8] = cb1p
    cvec[0:24, 9] = cb2p
    cvec[0:24, 10] = cb0p
    sh["cvec"] = cvec
    return H


def _build_fast(H, n=N_FULL, t_steps=T, ch=1024, mmc=512):
    nc = bacc.Bacc("TRN2", target_bir_lowering=False, debug=False)
    dram = {}
    for name, arr in H["shared"].items():
        dram[name] = nc.declare_dram_parameter(
            name, list(arr.shape), mybir.dt.from_np(arr.dtype), isOutput=False)
    srcp_d = nc.declare_dram_parameter("srcp", [2 * t_steps, n], BF16,
                                       isOutput=False)
    src32_d = nc.declare_dram_parameter("src32p", [2 * t_steps, n], F32,
                                        isOutput=False)
    out_d = nc.declare_dram_parameter("out", [2 * t_steps, n], F32,
                                      isOutput=True)
    xs2_d = nc.dram_tensor("xs2", [2 * t_steps, n], BF16)
    CH = list(reversed(_chunks(n, ch)))

    with tile.TileContext(nc) as tc, ExitStack() as ctx:
        pers = ctx.enter_context(tc.tile_pool(name="pers", bufs=1))

        def ptile(shape, dtype, nm):
            return pers.tile(shape, dtype, name=nm, tag=nm)

        SP = ptile([128, n], BF16, "SP")
        X2P = ptile([128, n], BF16, "X2P")
        QP = ptile([128, n], BF16, "QP")
        HC = ptile([128, n], BF16, "HC")
        ZQ = ptile([128, 2 * n], BF16, "ZQ")
        XQ = ptile([98, n], BF16, "XQ")
        O1B = ptile([24, n], F32, "O1B")
        CVEC = ptile(list(H["shared"]["cvec"].shape), F32, "CVEC")
        CWH1A = ptile([128, 24], BF16, "CWH1A")
        CWH1B = ptile([128, 24], BF16, "CWH1B")
        CWH2 = ptile([128, 24], BF16, "CWH2")
        WGP, WXGA, WXGB, WUBD, WXU = {}, {}, {}, {}, {}
        for e in range(2):
            WGP[e] = ptile([128, 128], BF16, f"wgp{e}")
            WXGA[e] = ptile([98, 128], BF16, f"wxga{e}")
            WXGB[e] = ptile([98, 128], BF16, f"wxgb{e}")
            WUBD[e] = ptile([128, 128], BF16, f"wubd{e}")
            WXU[e] = ptile([98, 128], BF16, f"wxu{e}")

        nc.sync.dma_start(CVEC[:, :], dram["cvec"][:, :])
        nc.sync.dma_start(CWH1A[:, :], dram["cwh1a"][:, :])
        nc.sync.dma_start(CWH1B[:, :], dram["cwh1b"][:, :])
        nc.sync.dma_start(CWH2[:, :], dram["cwh2"][:, :])
        for e in range(2):
            nc.sync.dma_start(WGP[e][:, :], dram[f"wgp{e}"][:, :])
            nc.sync.dma_start(WXGA[e][:, :], dram[f"wxga{e}"][:, :])
            nc.sync.dma_start(WXGB[e][:, :], dram[f"wxgb{e}"][:, :])
            nc.sync.dma_start(WUBD[e][:, :], dram[f"wubd{e}"][:, :])
            nc.sync.dma_start(WXU[e][:, :], dram[f"wxu{e}"][:, :])

        ps = ctx.enter_context(tc.tile_pool(name="ps", bufs=4, space="PSUM"))
        sb = ctx.enter_context(tc.tile_pool(name="sb", bufs=2))

        def bias(i, p=128):
            return CVEC[0:p, i:i + 1]

        def xq_load(src_ap, t):
            q = 32 * (t % 4)
            nc.sync.dma_start(XQ[q:q + 2, :], src_ap[2 * t:2 * t + 2, :])

        def cell(e, t, src_ap):
            q = 32 * (t % 4)
            first = t == 0
            # ---- gate: ZQ b-width; b0 cols rows 0:64=z, 64:128=q';
            #      b1 cols (swapped weights) rows 0:64=q', 64:128=z ----
            for b in range(2):
                boff = b * n
                bp = slice(64 * b, 64 * b + 64)
                for c0, cw_ in CH:
                    p = ps.tile([128, ch], F32, tag="ps")
                    for m0, mw in _chunks(cw_, mmc):
                        msl = slice(c0 + m0, c0 + m0 + mw)
                        if not first:
                            nc.tensor.matmul(p[:, m0:m0 + mw], WGP[e][bp, :],
                                             SP[bp, msl], start=True,
                                             stop=False)
                        if b == 0:
                            nc.tensor.matmul(p[:, m0:m0 + mw],
                                             WXGA[e][q:q + 1, :],
                                             XQ[q:q + 1, msl],
                                             start=first, stop=True,
                                             tile_position=(q, 0))
                        else:
                            nc.tensor.matmul(p[:, m0:m0 + mw],
                                             WXGB[e][q:q + 2, :],
                                             XQ[q:q + 2, msl],
                                             start=first, stop=True,
                                             tile_position=(q, 0))
                    osl = slice(boff + c0, boff + c0 + cw_)
                    nc.scalar.activation(ZQ[:, osl], p[:, :cw_], AF.Sigmoid,
                                         bias=bias(2 + 2 * e + b),
                                         scale=bias(b))
                    csl = slice(c0, c0 + cw_)
                    if not first:
                        # zs: z rows co-located with the b-packed state half
                        zeng = nc.vector if b == 0 else nc.gpsimd
                        zeng.tensor_mul(X2P[bp, csl], ZQ[bp, osl],
                                        SP[bp, csl])
                    # realign q' into b-packed QP (cross-partition copy)
                    qp = slice(64 - 64 * b, 128 - 64 * b)
                    nc.vector.tensor_copy(QP[bp, csl], ZQ[qp, osl])
            # ---- upd: hc = tanh(pre + uab2), b-packed; combine ----
            for c0, cw_ in CH:
                p = ps.tile([128, ch], F32, tag="ps")
                for m0, mw in _chunks(cw_, mmc):
                    msl = slice(c0 + m0, c0 + m0 + mw)
                    if not first:
                        nc.tensor.matmul(p[:, m0:m0 + mw], WUBD[e][:, :],
                                         X2P[:, msl], start=True, stop=False)
                    nc.tensor.matmul(p[:, m0:m0 + mw], WXU[e][q:q + 2, :],
                                     XQ[q:q + 2, msl],
                                     start=first, stop=True,
                                     tile_position=(q, 0))
                csl = slice(c0, c0 + cw_)
                nc.scalar.activation(HC[:, csl], p[:, :cw_], AF.Tanh,
                                     bias=bias(6 + e))
                if first:
                    # state was zero: s1 = q' * hc
                    nc.gpsimd.tensor_mul(SP[:, csl], QP[:, csl], HC[:, csl])
                else:
                    nc.vector.tensor_sub(HC[:, csl], HC[:, csl], SP[:, csl])
                    nc.gpsimd.tensor_mul(HC[:, csl], QP[:, csl], HC[:, csl])
                    nc.vector.tensor_add(SP[:, csl], SP[:, csl], HC[:, csl])
            # prefetch x for step t+4 into the quadrant this step just freed
            if t + 4 < t_steps:
                xq_load(src_ap, t + 4)

        # ================= encoder 1 =================
        for t in range(min(4, t_steps)):
            xq_load(srcp_d, t)
        for t in range(t_steps):
            cell(0, t, srcp_d)

        # heads 1+2 and encoder-2 input build
        hch = 1024
        for c0, cw_ in _chunks(n, hch):
            p1 = ps.tile([24, hch], F32, tag="ps")
            p2 = ps.tile([24, hch], F32, tag="ps")
            for m0, mw in _chunks(cw_, mmc):
                msl = slice(c0 + m0, c0 + m0 + mw)
                nc.tensor.matmul(p1[:, m0:m0 + mw], CWH1A[:, :], SP[:, msl],
                                 start=True, stop=True)
                nc.tensor.matmul(p2[:, m0:m0 + mw], CWH1B[:, :], SP[:, msl],
                                 start=True, stop=True)
            csl = slice(c0, c0 + cw_)
            o1a = sb.tile([24, hch], F32, tag="o1a")
            nc.scalar.activation(o1a[:, :cw_], p1[:24, :cw_], AF.Identity,
                                 bias=bias(8, 24))
            nc.scalar.activation(O1B[:, csl], p2[:24, :cw_], AF.Identity,
                                 bias=bias(10, 24))
            sc = sb.tile([24, hch], F32, tag="srcc")
            nc.sync.dma_start(sc[:, :cw_], src32_d[:, csl])
            x2c = sb.tile([24, hch], BF16, tag="x2c")
            nc.vector.tensor_sub(x2c[:, :cw_], sc[:, :cw_], o1a[:, :cw_])
            nc.sync.dma_start(xs2_d[:, csl], x2c[:, :cw_])
            # feed enc2's t=0 x rows straight from SBUF so encoder 2 can
            # begin before the full xs2 round-trip completes
            nc.sync.dma_start(XQ[0:2, csl], x2c[0:2, :cw_])

        # ================= encoder 2 =================
        for t in range(1, min(4, t_steps)):
            xq_load(xs2_d, t)
        for t in range(t_steps):
            cell(1, t, xs2_d)

        # head 3 + final sum
        hch = 1024
        for c0, cw_ in _chunks(n, hch):
            p = ps.tile([24, hch], F32, tag="ps")
            for m0, mw in _chunks(cw_, mmc):
                msl = slice(c0 + m0, c0 + m0 + mw)
                nc.tensor.matmul(p[0:24, m0:m0 + mw], CWH2[:, :], SP[:, msl],
                                 start=True, stop=True)
            csl = slice(c0, c0 + cw_)
            o2 = sb.tile([24, hch], F32, tag="o2")
            nc.scalar.activation(o2[:, :cw_], p[0:24, :cw_], AF.Identity,
                                 bias=bias(9, 24))
            nc.vector.tensor_add(o2[:, :cw_], o2[:, :cw_], O1B[:, csl])
            nc.sync.dma_start(out_d[:, csl], o2[:, :cw_])

    nc.compile()
    return nc


def _make_in_maps_fast(inputs, H, n=N_FULL, t_steps=T):
    src = np.asarray(inputs["source"], np.float32)[..., 0]  # (B, T, n)
    in_maps = []
    for c in range(NCORES):
        m = dict(H["shared"])
        s = np.zeros((2 * t_steps, n), np.float32)
        for b in range(BLOC):
            s[b::2] = src[BLOC * c + b]  # row 2t+b = src[2c+b, t]
        m["src32p"] = s
        m["srcp"] = s.astype(ml_dtypes.bfloat16)
        in_maps.append(m)
    return in_maps


def _assemble_fast(results, n=N_FULL, t_steps=T):
    full = np.zeros((B, t_steps, n, 1), np.float32)
    for c in range(NCORES):
        o = np.asarray(results[c]["out"])            # [2T, n], row 2o+b
        for b in range(BLOC):
            full[BLOC * c + b, :, :, 0] = o[b::2]
    return full


_PROG_CACHE = {}


def _prepare(inputs):
    """Pick fast/exact path; return (nc, in_maps, assemble_fn)."""
    HF = _prep_fast(inputs)
    if HF is not None:
        if "fast" not in _PROG_CACHE:
            _PROG_CACHE["fast"] = _build_fast(HF)
        return _PROG_CACHE["fast"], _make_in_maps_fast(inputs, HF), \
            _assemble_fast
    H = _prep_host(inputs)
    key = tuple(sorted(H["flags"].items()))
    if key not in _PROG_CACHE:
        _PROG_CACHE[key] = _build(H)
    return _PROG_CACHE[key], _make_in_maps(inputs, H), _assemble


def kernel(**inputs) -> np.ndarray:
    nc, in_maps, assemble = _prepare(inputs)
    res = run_bass_kernel_spmd(nc, in_maps, core_ids=list(range(NCORES)))
    return assemble(res.results)



# revision 18
# speedup vs baseline: 2.3472x; 1.0125x over previous
"""Trainium2 Bass kernel for nn_DDGCRN (gnn_message_passing).

DDGCRN: two 12-step GRU-style encoders over B=16, N=8600 nodes, HID=64,
with a global node-pooling term (GFS) inside each gate, plus conv heads.

Sharding: data-parallel over batch. B=16 / 8 cores = 2 batch elems per
core; the GFS node-pooling sum is per-batch-element, so no collectives.

Per-core layout: feature-major. All wide tensors are [D, W] where
W = 2*8600 = 17200 columns (col = b*N + n). State tile X is [66, W]:
rows 0:64 = hidden state, row 64 = x_t, row 65 = v (rank-1 pooling row).
Weight rows are host-permuted to match ([W_state; w_x; aug]).

The GFS pooled term affw*(pooled*scale) is rank-1 in (d, n) for the
given inputs (affw==1): C[d,n] = u[d]*v[n]. It is folded into the
"res" matmul by augmenting K: lhsT row 65 = pooled*u (written per cell
via PE-transpose), rhs row 65 = v (constant). Non-rank-1 / nonzero-affb
inputs fall back to explicit DVE ops with C/AB streamed from DRAM.
"""

import numpy as np
import ml_dtypes
from contextlib import ExitStack

import concourse.bass as bass
import concourse.bacc as bacc
import concourse.tile as tile
from concourse import mybir
from concourse.bass_utils import run_bass_kernel_spmd

F32 = mybir.dt.float32
F32R = mybir.dt.float32r
BF16 = mybir.dt.bfloat16
AX = mybir.AxisListType
OP = mybir.AluOpType
AF = mybir.ActivationFunctionType

# Problem constants (hardcoded; kernel.py must be self-contained)
B, T, N_FULL, HID, IN = 16, 12, 8600, 64, 1
GIN = IN + HID
NCORES = 8
BLOC = B // NCORES  # 2


def _chunks(total, size):
    out = []
    off = 0
    while off < total:
        w = min(size, total - off)
        out.append((off, w))
        off += w
    return out


def _rank1(C):
    """C [D, M] -> (u [D], v [M]) with C == outer(u, v), or None."""
    d0, m0 = np.unravel_index(np.argmax(np.abs(C)), C.shape)
    piv = C[d0, m0]
    if abs(piv) < 1e-30:
        return np.zeros(C.shape[0], np.float32), np.zeros(C.shape[1], np.float32)
    u = C[:, m0].astype(np.float64)
    v = C[d0, :].astype(np.float64) / piv
    if not np.allclose(np.outer(u, v), C, rtol=1e-5, atol=1e-7 * abs(piv)):
        return None
    return u.astype(np.float32), v.astype(np.float32)


def _prep_host(inputs, n=N_FULL, t_steps=T):
    """Host-side parameter prep. Returns dict of per-core-shared arrays
    plus flags. All weight matrices get the row permutation
    [rows 1:65 (state); row 0 (x); (aug row 0)]."""
    f32 = np.float32
    H = {"flags": {}}
    shared = {}

    def perm66(w):
        """[65, Dout] -> [66, Dout]: rows 0:64 = state weights, row 64 = 0
        (v-row slot / aug slot), row 65 = x weight."""
        z = np.zeros((1, w.shape[1]), f32)
        return np.concatenate([w[1:], z, w[0:1]], axis=0).astype(f32)

    for e in range(2):
        gaW = np.asarray(inputs["gate_alignW"][e], f32)   # [65, 128]
        gw = np.asarray(inputs["gate_w"][e], f32)         # [65, 128]
        gab = np.asarray(inputs["gate_alignb"][e], f32)   # [128]
        gb = np.asarray(inputs["gate_b"][e], f32)         # [128]
        uaW = np.asarray(inputs["upd_alignW"][e], f32)    # [65, 64]
        uw = np.asarray(inputs["upd_w"][e], f32)          # [65, 64]
        uab = np.asarray(inputs["upd_alignb"][e], f32)    # [64]
        ub = np.asarray(inputs["upd_b"][e], f32)          # [64]

        shared[f"wgh{e}"] = perm66(gw).astype(ml_dtypes.bfloat16)
        shared[f"wga{e}"] = perm66(gaW).astype(ml_dtypes.bfloat16)
        shared[f"wuh{e}"] = perm66(uw).astype(ml_dtypes.bfloat16)
        shared[f"wua{e}"] = perm66(uaW).astype(ml_dtypes.bfloat16)

        # rank-1 pooling factors: C[d, n] = affw[n, d] * aw[n] * nw[n]
        for kind, aff, aw, nw, dout in (
            ("g", inputs["gate_affw"][e], inputs["gate_aw"][e], inputs["gate_nw"][e], 128),
            ("u", inputs["upd_affw"][e], inputs["upd_aw"][e], inputs["upd_nw"][e], 64),
        ):
            scale = (np.asarray(aw, f32)[:, 0] * np.asarray(nw, f32)[0])  # [n]
            C = np.asarray(aff, f32).T * scale[None, :]  # [dout, n]
            r1 = _rank1(C)
            if r1 is None:
                H["flags"][f"cfull_{kind}{e}"] = True
                u = np.zeros(dout, f32)
                v = np.zeros(C.shape[1], f32)
                if kind == "g":
                    shared[f"cg{e}"] = np.ascontiguousarray(C)
                else:
                    shared[f"cu2_{e}"] = np.ascontiguousarray(
                        np.concatenate([C, C], axis=0))
            else:
                H["flags"][f"cfull_{kind}{e}"] = False
                u, v = r1
            if kind == "g":
                ug = u
                vg = v
            else:
                uu = u
                vu = v

        # affb fallback (AB tensors). gab/uab per-partition parts go in ACT bias.
        abg = np.asarray(inputs["gate_affb"][e], f32).T  # [128, n]
        abu = np.asarray(inputs["upd_affb"][e], f32).T   # [64, n]
        H["flags"][f"ab_g{e}"] = bool(np.any(abg))
        H["flags"][f"ab_u{e}"] = bool(np.any(abu))
        if H["flags"][f"ab_g{e}"]:
            shared[f"abg{e}"] = np.ascontiguousarray(abg)
        if H["flags"][f"ab_u{e}"]:
            shared[f"abu2_{e}"] = np.ascontiguousarray(
                np.concatenate([abu, abu], axis=0))

        # v rows, repeated per local batch elem
        shared[f"vg{e}"] = np.tile(vg[None, :], (1, BLOC)).astype(ml_dtypes.bfloat16)
        shared[f"vu{e}"] = np.tile(vu[None, :], (1, BLOC)).astype(ml_dtypes.bfloat16)

        # bias/scale vector columns
        H[f"gb{e}"] = gb
        H[f"bzr{e}"] = np.concatenate([gab[:64], -gab[64:]])
        H[f"ub2_{e}"] = np.concatenate([ub, ub])
        H[f"uab2_{e}"] = np.concatenate([uab, uab])
        H[f"ug{e}"] = ug
        H[f"uu2_{e}"] = np.concatenate([uu, uu])

    cw = np.asarray(inputs["conv_w"], f32)  # [3, 12, 64]
    cb = np.asarray(inputs["conv_b"], f32)  # [3, 12]
    # head order: [src1(12) | out1(12)] so src1 sits at partitions 0:12
    shared["cw01"] = np.concatenate([cw[1].T, cw[0].T], axis=1).astype(
        ml_dtypes.bfloat16)                                       # [64, 2T]
    shared["cw2"] = np.ascontiguousarray(cw[2].T).astype(ml_dtypes.bfloat16)
    shared["ident"] = np.eye(128, dtype=f32)

    # cvec columns
    ncol = 16
    cvec = np.zeros((128, ncol), f32)
    cvec[:64, 0] = 1.0
    cvec[64:, 0] = -1.0
    cols = {"szr": 0}
    ci = 1
    for e in range(2):
        for nm in ("gb", "bzr", "ub2_", "uab2_", "ug", "uu2_"):
            key = f"{nm}{e}"
            arr = H[key]
            cvec[: len(arr), ci] = arr
            cols[key] = ci
            ci += 1
    th = cb.shape[1]
    cvec[: 2 * th, ci] = np.concatenate([cb[1], cb[0]])
    cols["cb01"] = ci
    ci += 1
    cvec[:th, ci] = cb[2]
    cols["cb2"] = ci
    shared["cvec"] = cvec
    H["cols"] = cols
    H["shared"] = shared
    return H


def _build(H, n=N_FULL, t_steps=T, ch=1024, mmc=512):
    """Build the single-core Bass program (same for all cores)."""
    W = BLOC * n
    flags = H["flags"]
    cols = H["cols"]
    nc = bacc.Bacc("TRN2", target_bir_lowering=False, debug=False)

    dram = {}
    for name, arr in H["shared"].items():
        dram[name] = nc.declare_dram_parameter(
            name, list(arr.shape), mybir.dt.from_np(arr.dtype), isOutput=False)
    src32 = nc.declare_dram_parameter("src32", [t_steps, W], F32, isOutput=False)
    srcbf = nc.declare_dram_parameter("srcbf", [t_steps, W], BF16, isOutput=False)
    out_d = nc.declare_dram_parameter("out", [t_steps, W], F32, isOutput=True)
    o1d = nc.dram_tensor("o1d", [t_steps, W], F32)
    xsbf = nc.dram_tensor("xsbf", [t_steps, W], BF16)

    CH_B = _chunks(n, ch)       # chunks within one batch-half
    hch = min(ch, 1024)
    CH_W = _chunks(W, hch)      # chunks over full width (boundary passes)
    nchb = len(CH_B)

    with tile.TileContext(nc) as tc, ExitStack() as ctx:
        # ---- persistent tiles (one pool, distinct tags = one slot each) ----
        pers = ctx.enter_context(tc.tile_pool(name="pers", bufs=1))

        def ptile(shape, dtype, nm):
            return pers.tile(shape, dtype, name=nm, tag=nm)

        X = ptile([66, W], BF16, "X")
        X2 = ptile([66, W], BF16, "X2")
        ZQ = ptile([128, W], BF16, "ZQ")
        CVEC = ptile(list(H["shared"]["cvec"].shape), F32, "CVEC")
        IDENT = ptile([128, 128], F32, "IDENT")
        th = t_steps
        CW01 = ptile([64, 2 * th], BF16, "CW01")
        CW2 = ptile([64, th], BF16, "CW2")
        WGH = {}
        WGA = {}
        WUH = {}
        WUA = {}
        for e in range(2):
            WGH[e] = ptile([66, 128], BF16, f"twgh{e}")
            WUH[e] = ptile([66, 64], BF16, f"twuh{e}")
            WGA[e] = {}
            WUA[e] = {}
            for b in range(BLOC):
                WGA[e][b] = ptile([66, 128], BF16, f"twga{e}{b}")
                WUA[e][b] = ptile([66, 64], BF16, f"twua{e}{b}")

        # (Bacc.generate_event_semaphores legalizes wait counts at compile)
        nc.sync.dma_start(CVEC[:, :], dram["cvec"][:, :])
        nc.sync.dma_start(IDENT[:, :], dram["ident"][:, :])
        nc.sync.dma_start(CW01[:, :], dram["cw01"][:, :])
        nc.sync.dma_start(CW2[:, :], dram["cw2"][:, :])
        for e in range(2):
            nc.sync.dma_start(WGH[e][:, :], dram[f"wgh{e}"][:, :])
            nc.sync.dma_start(WUH[e][:, :], dram[f"wuh{e}"][:, :])
            for b in range(BLOC):
                nc.sync.dma_start(WGA[e][b][:, :], dram[f"wga{e}"][:, :])
                nc.sync.dma_start(WUA[e][b][:, :], dram[f"wua{e}"][:, :])

        def bias(key):
            return CVEC[:, cols[key]:cols[key] + 1]

        # ---- pools ----
        ps = ctx.enter_context(tc.tile_pool(name="ps", bufs=3, space="PSUM"))
        tps = ctx.enter_context(tc.tile_pool(name="tps", bufs=2, space="PSUM"))
        sb = ctx.enter_context(tc.tile_pool(name="sb", bufs=3))
        small = ctx.enter_context(tc.tile_pool(name="small", bufs=2))
        fpool = ctx.enter_context(tc.tile_pool(name="fpool", bufs=2))

        def mm_into(p, lhsT, rhs_tile, rhs_rows, coff, cw_, p_rows=None,
                    cast=None):
            """Matmul chunk [*, cw_] at col offset coff into psum tile p."""
            for m0, mw in _chunks(cw_, mmc):
                lhs_ap = lhsT
                rhs_ap = rhs_tile[rhs_rows, coff + m0:coff + m0 + mw]
                o = p[:, m0:m0 + mw] if p_rows is None else \
                    p[p_rows, m0:m0 + mw]
                if cast is not None:
                    lhs_ap = lhs_ap.bitcast(cast)
                    rhs_ap = rhs_ap.bitcast(cast)
                nc.tensor.matmul(o, lhs_ap, rhs_ap, start=True, stop=True)

        def gfs_gate(e):
            pooled_b = []
            for b in range(BLOC):
                boff = b * n
                parts = small.tile([128, nchb], F32, tag="parts")
                for ci, (c0, cw_) in enumerate(CH_B):
                    p = ps.tile([128, ch], F32, tag="ps")
                    mm_into(p, WGH[e][:, :], X, slice(0, 66), boff + c0, cw_)
                    if False:
                        # DVE relu+bias+accum for engine balance
                        nc.vector.tensor_scalar(p[:, :cw_], p[:, :cw_],
                                                bias(f"gb{e}"), 0.0,
                                                op0=OP.add, op1=OP.max,
                                                accum_out=parts[:, ci:ci + 1])
                    else:
                        nc.scalar.activation(p[:, :cw_], p[:, :cw_], AF.Relu,
                                             bias=bias(f"gb{e}"),
                                             accum_out=parts[:, ci:ci + 1])
                if not flags[f"cfull_g{e}"]:
                    # fused: scratch = parts*u, accum = sum = pooled*u
                    scr = small.tile([128, nchb], F32, tag="pscr")
                    pgs = small.tile([128, 1], F32, tag="pgs")
                    nc.vector.tensor_scalar(scr[:, :], parts[:, :],
                                            bias(f"ug{e}"), 0.0, op0=OP.mult,
                                            op1=OP.add, accum_out=pgs[:, :])
                    tp = tps.tile([1, 128], F32, tag="tp")
                    nc.tensor.transpose(tp[:, :], pgs[:, :], IDENT[:, :])
                    nc.vector.tensor_copy(WGA[e][b][64:65, 0:128],
                                          tp[0:1, 0:128])
                    pooled_b.append(None)
                else:
                    pooled = small.tile([128, 1], F32, tag="pooled")
                    nc.vector.tensor_reduce(pooled[:, :], parts[:, :],
                                            axis=AX.X, op=OP.add)
                    pooled_b.append(pooled)
            # pass 2: res(+aug) matmul -> sigmoid -> ZQ (z rows 0:64, q 64:128)
            for b in range(BLOC):
                boff = b * n
                for c0, cw_ in CH_B:
                    p = ps.tile([128, ch], F32, tag="ps")
                    mm_into(p, WGA[e][b][:, :], X, slice(0, 66), boff + c0,
                            cw_)
                    if flags[f"cfull_g{e}"]:
                        cgc = fpool.tile([128, ch], F32, tag="cgc")
                        nc.sync.dma_start(cgc[:, :cw_],
                                          dram[f"cg{e}"][:, c0:c0 + cw_])
                        nc.vector.scalar_tensor_tensor(
                            p[:, :cw_], cgc[:, :cw_], pooled_b[b][:, :],
                            p[:, :cw_], op0=OP.mult, op1=OP.add)
                    if flags[f"ab_g{e}"]:
                        abc = fpool.tile([128, ch], F32, tag="abc")
                        nc.sync.dma_start(abc[:, :cw_],
                                          dram[f"abg{e}"][:, c0:c0 + cw_])
                        nc.vector.tensor_add(p[:, :cw_], p[:, :cw_],
                                             abc[:, :cw_])
                    nc.scalar.activation(ZQ[:, boff + c0:boff + c0 + cw_],
                                         p[:, :cw_], AF.Sigmoid,
                                         bias=bias(f"bzr{e}"),
                                         scale=bias("szr"))
                    # zs = z * S -> X2 state rows (b0 on gpsimd for balance)
                    zeng = nc.gpsimd if b == 0 else nc.vector
                    zeng.tensor_mul(
                        X2[0:64, boff + c0:boff + c0 + cw_],
                        ZQ[0:64, boff + c0:boff + c0 + cw_],
                        X[0:64, boff + c0:boff + c0 + cw_])

        def gfs_upd(e):
            parts = small.tile([128, nchb], F32, tag="parts")
            for ci, (c0, cw_) in enumerate(CH_B):
                p = ps.tile([128, ch], F32, tag="ps")
                for b in range(BLOC):
                    mm_into(p, WUH[e][:, :], X2, slice(0, 66), b * n + c0,
                            cw_, p_rows=slice(b * 64, b * 64 + 64))
                if ci % 2 == 0:
                    nc.scalar.activation(p[:, :cw_], p[:, :cw_], AF.Relu,
                                         bias=bias(f"ub2_{e}"),
                                         accum_out=parts[:, ci:ci + 1])
                else:
                    # DVE version of relu+bias+accum to balance engines
                    nc.vector.tensor_scalar(p[:, :cw_], p[:, :cw_],
                                            bias(f"ub2_{e}"), 0.0,
                                            op0=OP.add, op1=OP.max,
                                            accum_out=parts[:, ci:ci + 1])
            pooled2 = None
            if not flags[f"cfull_u{e}"]:
                scr = small.tile([128, nchb], F32, tag="pscr")
                pgs = small.tile([128, 1], F32, tag="pgs")
                nc.vector.tensor_scalar(scr[:, :], parts[:, :],
                                        bias(f"uu2_{e}"), 0.0, op0=OP.mult,
                                        op1=OP.add, accum_out=pgs[:, :])
                tp = tps.tile([1, 128], F32, tag="tp")
                nc.tensor.transpose(tp[:, :], pgs[:, :], IDENT[:, :])
                nc.vector.tensor_copy(WUA[e][0][64:65, 0:64], tp[0:1, 0:64])
                nc.vector.tensor_copy(WUA[e][1][64:65, 0:64],
                                      tp[0:1, 64:128])
            else:
                pooled2 = small.tile([128, 1], F32, tag="pooled")
                nc.vector.tensor_reduce(pooled2[:, :], parts[:, :],
                                        axis=AX.X, op=OP.add)
            # full-width q realign: one SBUF->SBUF DMA per batch elem,
            # issued while U1 still runs (sigmoid is long done)
            qts = []
            for b in range(BLOC):
                qt = sb.tile([64, n], BF16, tag="qt", bufs=2)
                nc.gpsimd.dma_start(qt[:, :], ZQ[64:128, b * n:(b + 1) * n])
                qts.append(qt)
            hcf = sb.tile([128, n], BF16, tag="hcf", bufs=2)
            # pass 2: res2(+aug) -> tanh -> combine into state
            for c0, cw_ in CH_B:
                p = ps.tile([128, ch], F32, tag="ps")
                for b in range(BLOC):
                    mm_into(p, WUA[e][b][:, :], X2, slice(0, 66), b * n + c0,
                            cw_, p_rows=slice(b * 64, b * 64 + 64))
                if flags[f"cfull_u{e}"]:
                    cuc = fpool.tile([128, ch], F32, tag="cgc")
                    nc.sync.dma_start(cuc[:, :cw_],
                                      dram[f"cu2_{e}"][:, c0:c0 + cw_])
                    nc.vector.scalar_tensor_tensor(
                        p[:, :cw_], cuc[:, :cw_], pooled2[:, :], p[:, :cw_],
                        op0=OP.mult, op1=OP.add)
                if flags[f"ab_u{e}"]:
                    abc = fpool.tile([128, ch], F32, tag="abc")
                    nc.sync.dma_start(abc[:, :cw_],
                                      dram[f"abu2_{e}"][:, c0:c0 + cw_])
                    nc.vector.tensor_add(p[:, :cw_], p[:, :cw_], abc[:, :cw_])
                nc.scalar.activation(hcf[:, c0:c0 + cw_], p[:, :cw_], AF.Tanh,
                                     bias=bias(f"uab2_{e}"))
                # b0 combine chunk-rolls right behind tanh
                csl = slice(c0, c0 + cw_)
                nc.vector.tensor_sub(hcf[0:64, csl], hcf[0:64, csl],
                                     X[0:64, csl])
                nc.vector.tensor_mul(hcf[0:64, csl], qts[0][:, csl],
                                     hcf[0:64, csl])
                nc.vector.tensor_add(X[0:64, csl], X[0:64, csl],
                                     hcf[0:64, csl])
            # b1: chunked realign (SWDGE; Pool engine is otherwise idle)
            for c0, cw_ in CH_B:
                csl = slice(c0, c0 + cw_)
                xsl = slice(n + c0, n + c0 + cw_)
                hct = sb.tile([64, ch], BF16, tag="hct", bufs=2)
                nc.gpsimd.dma_start(hct[:, :cw_], hcf[64:128, csl])
                nc.vector.tensor_sub(hct[:, :cw_], hct[:, :cw_], X[0:64, xsl])
                nc.vector.tensor_mul(hct[:, :cw_], qts[1][:, csl],
                                     hct[:, :cw_])
                nc.vector.tensor_add(X[0:64, xsl], X[0:64, xsl],
                                     hct[:, :cw_])

        def absorb(src_tile, we):
            # dummy ldweights pre-consuming {engine, DMA} waits on the PE so
            # subsequent real matmuls stay within the 2-slot MM wait limit
            # (real matmuls are self-loading, so this clobbers nothing)
            nc.tensor.ldweights(src_tile[0:66, 0:1])

        def encoder(e, xbf_src):
            nc.vector.memset(X[0:64, :], 0.0)
            nc.sync.dma_start(X2[64:65, :], dram[f"vu{e}"][:, :])
            nc.sync.dma_start(X[64:65, :], dram[f"vg{e}"][:, :])
            absorb(X, WGH[e])
            absorb(X2, WUH[e])
            for t in range(t_steps):
                nc.sync.dma_start(X2[65:66, :], xbf_src[t:t + 1, :])
                nc.sync.dma_start(X[65:66, :], xbf_src[t:t + 1, :])
                absorb(X, WGH[e])
                gfs_gate(e)
                absorb(X2, WUH[e])
                gfs_upd(e)

        # ================= encoder 1 =================
        encoder(0, srcbf)

        # heads 1+2 and encoder-2 input build
        for c0, cw_ in CH_W:
            p = ps.tile([2 * th, hch], F32, tag="ps")
            mm_into(p, CW01[:, :], X, slice(0, 64), c0, cw_)
            oc = sb.tile([2 * th, hch], F32, tag="hc", bufs=2)
            nc.scalar.activation(oc[:, :cw_], p[: 2 * th, :cw_], AF.Identity,
                                 bias=CVEC[0: 2 * th, cols["cb01"]:cols["cb01"] + 1])
            # rows 0:12 = src1 (head 1), rows 12:24 = out1 (head 0)
            nc.sync.dma_start(o1d[:, c0:c0 + cw_], oc[th: 2 * th, :cw_])
            sc = small.tile([th, hch], F32, tag="srcc")
            nc.sync.dma_start(sc[:, :cw_], src32[:, c0:c0 + cw_])
            xbb = sb.tile([th, hch], BF16, tag="xbb", bufs=2)
            nc.vector.tensor_sub(xbb[:, :cw_], sc[:, :cw_], oc[0:th, :cw_])
            nc.sync.dma_start(xsbf[:, c0:c0 + cw_], xbb[:, :cw_])

        # ================= encoder 2 =================
        encoder(1, xsbf)

        # head 3 + final sum
        for c0, cw_ in CH_W:
            p = ps.tile([th, hch], F32, tag="ps")
            mm_into(p, CW2[:, :], X, slice(0, 64), c0, cw_)
            o2 = sb.tile([th, hch], F32, tag="hc", bufs=2)
            nc.scalar.activation(o2[:, :cw_], p[:th, :cw_], AF.Identity,
                                 bias=CVEC[0:th, cols["cb2"]:cols["cb2"] + 1])
            o1c = sb.tile([th, hch], F32, tag="d", bufs=2)
            nc.sync.dma_start(o1c[:, :cw_], o1d[:, c0:c0 + cw_])
            nc.vector.tensor_add(o2[:, :cw_], o2[:, :cw_], o1c[:, :cw_])
            nc.sync.dma_start(out_d[:, c0:c0 + cw_], o2[:, :cw_])

    nc.compile()
    return nc


def _make_in_maps(inputs, H, n=N_FULL, t_steps=T):
    src = np.asarray(inputs["source"], np.float32)[..., 0]  # (B, T, n)
    in_maps = []
    for c in range(NCORES):
        m = dict(H["shared"])
        # src32[t, b*n + i] = src[2c+b, t, i]
        blk = src[BLOC * c: BLOC * (c + 1)]          # (BLOC, T, n)
        s = np.ascontiguousarray(
            blk.transpose(1, 0, 2).reshape(t_steps, BLOC * n))
        m["src32"] = s
        m["srcbf"] = s.astype(ml_dtypes.bfloat16)
        in_maps.append(m)
    return in_maps


def _assemble(results, n=N_FULL, t_steps=T):
    full = np.zeros((B, t_steps, n, 1), np.float32)
    for c in range(NCORES):
        o = np.asarray(results[c]["out"])            # [T, BLOC*n]
        o = o.reshape(t_steps, BLOC, n).transpose(1, 0, 2)
        full[BLOC * c: BLOC * (c + 1), :, :, 0] = o
    return full


# ---------------------------------------------------------------------------
# Fast path: GFS pooled term provably negligible -> drop both pooled matmul
# passes. State is b-packed [128, n] (partitions 64b+d); x_t contributions
# enter via K=1/K=2 accumulating matmuls from a resident [24, n] source tile
# (rows 2t+b), eliminating the per-step single-partition x-row DMAs.
# ---------------------------------------------------------------------------

TAU_MAX = 1e-3  # bound on the dropped pooled term's pre-activation magnitude


def _prep_fast(inputs):
    """Return fast-path host dict, or None if the pooled term is not
    provably negligible / affb nonzero."""
    f32 = np.float32
    n, th = N_FULL, T
    src = np.asarray(inputs["source"], f32)
    cw = np.asarray(inputs["conv_w"], f32)
    cb = np.asarray(inputs["conv_b"], f32)
    if np.any(np.asarray(inputs["gate_affb"])) or np.any(
            np.asarray(inputs["upd_affb"])):
        return None
    xmax0 = float(np.abs(src).max())
    s1max = float(np.max(np.sum(np.abs(cw[1]), axis=1) + np.abs(cb[1])))
    xmax = [xmax0, xmax0 + s1max]
    for e in range(2):
        for w_, b_, aff, aw, nw in (
            (inputs["gate_w"][e], inputs["gate_b"][e], inputs["gate_affw"][e],
             inputs["gate_aw"][e], inputs["gate_nw"][e]),
            (inputs["upd_w"][e], inputs["upd_b"][e], inputs["upd_affw"][e],
             inputs["upd_aw"][e], inputs["upd_nw"][e]),
        ):
            w_ = np.asarray(w_, f32)
            b_ = np.asarray(b_, f32)
            cmax = float(np.abs(np.asarray(aff, f32).T
                                * (np.asarray(aw, f32)[:, 0]
                                   * np.asarray(nw, f32)[0])[None, :]).max())
            premax = np.abs(b_) + np.abs(w_[0]) * xmax[e] + \
                np.sum(np.abs(w_[1:]), axis=0)
            tau = cmax * n * float(np.maximum(premax, 0.0).max())
            if tau > TAU_MAX:
                return None

    bf = ml_dtypes.bfloat16
    H = {"shared": {}}
    sh = H["shared"]
    for e in range(2):
        gaW = np.asarray(inputs["gate_alignW"][e], f32)   # [65, 128]
        gab = np.asarray(inputs["gate_alignb"][e], f32)   # [128]
        uaW = np.asarray(inputs["upd_alignW"][e], f32)    # [65, 64]
        uab = np.asarray(inputs["upd_alignb"][e], f32)    # [64]
        # b1 rows use swapped z/q column blocks so that z lands on psum rows
        # 64:128 for b1 (partition-aligned with the b-packed state) and q' on
        # 0:64; this keeps every gating TensorTensor op same-start-partition.
        swap = np.concatenate([gaW[:, 64:128], gaW[:, 0:64]], axis=1)
        wgp = np.zeros((128, 128), f32)
        wgp[0:64] = gaW[1:65]
        wgp[64:128] = swap[1:65]
        sh[f"wgp{e}"] = wgp.astype(bf)
        wxga = np.zeros((98, 128), f32)
        wxgb = np.zeros((98, 128), f32)
        for q in (0, 32, 64, 96):
            wxga[q] = gaW[0]          # K=1 row for gate b0
            wxgb[q + 1] = swap[0]     # K=2 rows [0; wx_swapped] for gate b1
        sh[f"wxga{e}"] = wxga.astype(bf)
        sh[f"wxgb{e}"] = wxgb.astype(bf)
        wubd = np.zeros((128, 128), f32)
        wubd[0:64, 0:64] = uaW[1:65]
        wubd[64:128, 64:128] = uaW[1:65]
        sh[f"wubd{e}"] = wubd.astype(bf)
        wxu = np.zeros((98, 128), f32)
        for q in (0, 32, 64, 96):
            wxu[q, 0:64] = uaW[0]
            wxu[q + 1, 64:128] = uaW[0]
        sh[f"wxu{e}"] = wxu.astype(bf)
        H[f"bzr{e}"] = np.concatenate([gab[:64], -gab[64:]])
        H[f"bzrS{e}"] = np.concatenate([-gab[64:], gab[:64]])
        H[f"uab2_{e}"] = np.concatenate([uab, uab])

    cwh1a = np.zeros((128, 24), f32)
    cwh1b = np.zeros((128, 24), f32)
    cwh2 = np.zeros((128, 24), f32)
    cb1p = np.zeros(24, f32)
    cb0p = np.zeros(24, f32)
    cb2p = np.zeros(24, f32)
    for b in range(2):
        for o in range(th):
            cwh1a[64 * b:64 * b + 64, 2 * o + b] = cw[1][o]
            cwh1b[64 * b:64 * b + 64, 2 * o + b] = cw[0][o]
            cwh2[64 * b:64 * b + 64, 2 * o + b] = cw[2][o]
            cb1p[2 * o + b] = cb[1][o]
            cb0p[2 * o + b] = cb[0][o]
            cb2p[2 * o + b] = cb[2][o]
    sh["cwh1a"] = cwh1a.astype(bf)
    sh["cwh1b"] = cwh1b.astype(bf)
    sh["cwh2"] = cwh2.astype(bf)

    cvec = np.zeros((128, 11), f32)
    cvec[0:64, 0] = 1.0        # szr (b0): z rows +, q rows -
    cvec[64:128, 0] = -1.0
    cvec[0:64, 1] = -1.0       # szrS (b1 swapped): q rows -, z rows +
    cvec[64:128, 1] = 1.0
    cvec[:, 2] = H["bzr0"]
    cvec[:, 3] = H["bzrS0"]
    cvec[:, 4] = H["bzr1"]
    cvec[:, 5] = H["bzrS1"]
    cvec[:, 6] = H["uab2_0"]
    cvec[:, 7] = H["uab2_1"]
    cvec[0:24, 8] = cb1p
    cvec[0:24, 9] = cb2p
    cvec[0:24, 10] = cb0p
    sh["cvec"] = cvec
    return H


def _build_fast(H, n=N_FULL, t_steps=T, ch=1024, mmc=512):
    nc = bacc.Bacc("TRN2", target_bir_lowering=False, debug=False)
    dram = {}
    for name, arr in H["shared"].items():
        dram[name] = nc.declare_dram_parameter(
            name, list(arr.shape), mybir.dt.from_np(arr.dtype), isOutput=False)
    srcp_d = nc.declare_dram_parameter("srcp", [2 * t_steps, n], BF16,
                                       isOutput=False)
    src32_d = nc.declare_dram_parameter("src32p", [2 * t_steps, n], F32,
                                        isOutput=False)
    out_d = nc.declare_dram_parameter("out", [2 * t_steps, n], F32,
                                      isOutput=True)
    xs2_d = nc.dram_tensor("xs2", [2 * t_steps, n], BF16)
    CH = list(reversed(_chunks(n, ch)))

    with tile.TileContext(nc) as tc, ExitStack() as ctx:
        pers = ctx.enter_context(tc.tile_pool(name="pers", bufs=1))

        def ptile(shape, dtype, nm):
            return pers.tile(shape, dtype, name=nm, tag=nm)

        SP = ptile([128, n], BF16, "SP")
        X2P = ptile([128, n], BF16, "X2P")
        QP = ptile([128, n], BF16, "QP")
        HC = ptile([128, n], BF16, "HC")
        ZQ = ptile([128, 2 * n], BF16, "ZQ")
        XQ = ptile([98, n], BF16, "XQ")
        O1B = ptile([24, n], F32, "O1B")
        CVEC = ptile(list(H["shared"]["cvec"].shape), F32, "CVEC")
        CWH1A = ptile([128, 24], BF16, "CWH1A")
        CWH1B = ptile([128, 24], BF16, "CWH1B")
        CWH2 = ptile([128, 24], BF16, "CWH2")
        WGP, WXGA, WXGB, WUBD, WXU = {}, {}, {}, {}, {}
        for e in range(2):
            WGP[e] = ptile([128, 128], BF16, f"wgp{e}")
            WXGA[e] = ptile([98, 128], BF16, f"wxga{e}")
            WXGB[e] = ptile([98, 128], BF16, f"wxgb{e}")
            WUBD[e] = ptile([128, 128], BF16, f"wubd{e}")
            WXU[e] = ptile([98, 128], BF16, f"wxu{e}")

        nc.sync.dma_start(CVEC[:, :], dram["cvec"][:, :])
        nc.sync.dma_start(CWH1A[:, :], dram["cwh1a"][:, :])
        nc.sync.dma_start(CWH1B[:, :], dram["cwh1b"][:, :])
        nc.sync.dma_start(CWH2[:, :], dram["cwh2"][:, :])
        for e in range(2):
            nc.sync.dma_start(WGP[e][:, :], dram[f"wgp{e}"][:, :])
            nc.sync.dma_start(WXGA[e][:, :], dram[f"wxga{e}"][:, :])
            nc.sync.dma_start(WXGB[e][:, :], dram[f"wxgb{e}"][:, :])
            nc.sync.dma_start(WUBD[e][:, :], dram[f"wubd{e}"][:, :])
            nc.sync.dma_start(WXU[e][:, :], dram[f"wxu{e}"][:, :])

        ps = ctx.enter_context(tc.tile_pool(name="ps", bufs=4, space="PSUM"))
        sb = ctx.enter_context(tc.tile_pool(name="sb", bufs=2))
        ob = ctx.enter_context(tc.tile_pool(name="ob", bufs=4))

        def bias(i, p=128):
            return CVEC[0:p, i:i + 1]

        def xq_load(src_ap, t):
            q = 32 * (t % 4)
            nc.sync.dma_start(XQ[q:q + 2, :], src_ap[2 * t:2 * t + 2, :])

        def cell(e, t, src_ap):
            q = 32 * (t % 4)
            first = t == 0
            # ---- gate: ZQ b-width; b0 cols rows 0:64=z, 64:128=q';
            #      b1 cols (swapped weights) rows 0:64=q', 64:128=z ----
            for b in range(2):
                boff = b * n
                bp = slice(64 * b, 64 * b + 64)
                for c0, cw_ in CH:
                    p = ps.tile([128, ch], F32, tag="ps")
                    for m0, mw in _chunks(cw_, mmc):
                        msl = slice(c0 + m0, c0 + m0 + mw)
                        if not first:
                            nc.tensor.matmul(p[:, m0:m0 + mw], WGP[e][bp, :],
                                             SP[bp, msl], start=True,
                                             stop=False)
                        if b == 0:
                            nc.tensor.matmul(p[:, m0:m0 + mw],
                                             WXGA[e][q:q + 1, :],
                                             XQ[q:q + 1, msl],
                                             start=first, stop=True,
                                             tile_position=(q, 0))
                        else:
                            nc.tensor.matmul(p[:, m0:m0 + mw],
                                             WXGB[e][q:q + 2, :],
                                             XQ[q:q + 2, msl],
                                             start=first, stop=True,
                                             tile_position=(q, 0))
                    osl = slice(boff + c0, boff + c0 + cw_)
                    nc.scalar.activation(ZQ[:, osl], p[:, :cw_], AF.Sigmoid,
                                         bias=bias(2 + 2 * e + b),
                                         scale=bias(b))
                    csl = slice(c0, c0 + cw_)
                    if not first:
                        # zs: z rows co-located with the b-packed state half
                        zeng = nc.vector if b == 0 else nc.gpsimd
                        zeng.tensor_mul(X2P[bp, csl], ZQ[bp, osl],
                                        SP[bp, csl])
                    # realign q' into b-packed QP (cross-partition copy)
                    qp = slice(64 - 64 * b, 128 - 64 * b)
                    nc.vector.tensor_copy(QP[bp, csl], ZQ[qp, osl])
            # ---- upd: hc = tanh(pre + uab2), b-packed; combine ----
            for c0, cw_ in CH:
                p = ps.tile([128, ch], F32, tag="ps")
                for m0, mw in _chunks(cw_, mmc):
                    msl = slice(c0 + m0, c0 + m0 + mw)
                    if not first:
                        nc.tensor.matmul(p[:, m0:m0 + mw], WUBD[e][:, :],
                                         X2P[:, msl], start=True, stop=False)
                    nc.tensor.matmul(p[:, m0:m0 + mw], WXU[e][q:q + 2, :],
                                     XQ[q:q + 2, msl],
                                     start=first, stop=True,
                                     tile_position=(q, 0))
                csl = slice(c0, c0 + cw_)
                nc.scalar.activation(HC[:, csl], p[:, :cw_], AF.Tanh,
                                     bias=bias(6 + e))
                if first:
                    # state was zero: s1 = q' * hc
                    nc.gpsimd.tensor_mul(SP[:, csl], QP[:, csl], HC[:, csl])
                else:
                    nc.vector.tensor_sub(HC[:, csl], HC[:, csl], SP[:, csl])
                    nc.gpsimd.tensor_mul(HC[:, csl], QP[:, csl], HC[:, csl])
                    nc.vector.tensor_add(SP[:, csl], SP[:, csl], HC[:, csl])
            # prefetch x for step t+4 into the quadrant this step just freed
            if t + 4 < t_steps:
                xq_load(src_ap, t + 4)

        # ================= encoder 1 =================
        for t in range(min(4, t_steps)):
            xq_load(srcp_d, t)
        for t in range(t_steps):
            cell(0, t, srcp_d)

        # heads 1+2 and encoder-2 input build
        hch = 1024
        for c0, cw_ in _chunks(n, hch):
            p1 = ps.tile([24, hch], F32, tag="ps")
            p2 = ps.tile([24, hch], F32, tag="ps")
            for m0, mw in _chunks(cw_, mmc):
                msl = slice(c0 + m0, c0 + m0 + mw)
                nc.tensor.matmul(p1[:, m0:m0 + mw], CWH1A[:, :], SP[:, msl],
                                 start=True, stop=True)
                nc.tensor.matmul(p2[:, m0:m0 + mw], CWH1B[:, :], SP[:, msl],
                                 start=True, stop=True)
            csl = slice(c0, c0 + cw_)
            o1a = sb.tile([24, hch], F32, tag="o1a")
            nc.scalar.activation(o1a[:, :cw_], p1[:24, :cw_], AF.Identity,
                                 bias=bias(8, 24))
            nc.scalar.activation(O1B[:, csl], p2[:24, :cw_], AF.Identity,
                                 bias=bias(10, 24))
            sc = sb.tile([24, hch], F32, tag="srcc")
            nc.sync.dma_start(sc[:, :cw_], src32_d[:, csl])
            x2c = sb.tile([24, hch], BF16, tag="x2c")
            nc.vector.tensor_sub(x2c[:, :cw_], sc[:, :cw_], o1a[:, :cw_])
            nc.sync.dma_start(xs2_d[:, csl], x2c[:, :cw_])
            # feed enc2's t=0 x rows straight from SBUF so encoder 2 can
            # begin before the full xs2 round-trip completes
            nc.sync.dma_start(XQ[0:2, csl], x2c[0:2, :cw_])

        # ================= encoder 2 =================
        for t in range(1, min(4, t_steps)):
            xq_load(xs2_d, t)
        for t in range(t_steps):
            cell(1, t, xs2_d)

        # head 3 + final sum
        hch = 1024
        for c0, cw_ in _chunks(n, hch):
            p = ps.tile([24, hch], F32, tag="ps")
            for m0, mw in _chunks(cw_, mmc):
                msl = slice(c0 + m0, c0 + m0 + mw)
                nc.tensor.matmul(p[0:24, m0:m0 + mw], CWH2[:, :], SP[:, msl],
                                 start=True, stop=True)
            csl = slice(c0, c0 + cw_)
            o2 = ob.tile([24, hch], F32, tag="o2")
            nc.scalar.activation(o2[:, :cw_], p[0:24, :cw_], AF.Identity,
                                 bias=bias(9, 24))
            nc.vector.tensor_add(o2[:, :cw_], o2[:, :cw_], O1B[:, csl])
            nc.sync.dma_start(out_d[:, csl], o2[:, :cw_])

    nc.compile()
    return nc


def _make_in_maps_fast(inputs, H, n=N_FULL, t_steps=T):
    src = np.asarray(inputs["source"], np.float32)[..., 0]  # (B, T, n)
    in_maps = []
    for c in range(NCORES):
        m = dict(H["shared"])
        s = np.zeros((2 * t_steps, n), np.float32)
        for b in range(BLOC):
            s[b::2] = src[BLOC * c + b]  # row 2t+b = src[2c+b, t]
        m["src32p"] = s
        m["srcp"] = s.astype(ml_dtypes.bfloat16)
        in_maps.append(m)
    return in_maps


def _assemble_fast(results, n=N_FULL, t_steps=T):
    full = np.zeros((B, t_steps, n, 1), np.float32)
    for c in range(NCORES):
        o = np.asarray(results[c]["out"])            # [2T, n], row 2o+b
        for b in range(BLOC):
            full[BLOC * c + b, :, :, 0] = o[b::2]
    return full


_PROG_CACHE = {}


def _prepare(inputs):
    """Pick fast/exact path; return (nc, in_maps, assemble_fn)."""
    HF = _prep_fast(inputs)
    if HF is not None:
        if "fast" not in _PROG_CACHE:
            _PROG_CACHE["fast"] = _build_fast(HF)
        return _PROG_CACHE["fast"], _make_in_maps_fast(inputs, HF), \
            _assemble_fast
    H = _prep_host(inputs)
    key = tuple(sorted(H["flags"].items()))
    if key not in _PROG_CACHE:
        _PROG_CACHE[key] = _build(H)
    return _PROG_CACHE[key], _make_in_maps(inputs, H), _assemble


def kernel(**inputs) -> np.ndarray:
    nc, in_maps, assemble = _prepare(inputs)
    res = run_bass_kernel_spmd(nc, in_maps, core_ids=list(range(NCORES)))
    return assemble(res.results)

